# revision 1
# baseline (speedup 1.0000x reference)
"""Trainium2 Bass kernel: 4-layer decoder prefill (S=1024, H=2048, NH=16, HD=128,
FFN=5632, V=32000), tensor-parallel over 8 NeuronCores.

- Megatron TP over 8 cores: wq/wk/wv/w1/w3 sharded on output dim (2 heads /
  704 ffn rows per core), wo/w2 sharded on input dim (partials -> AllReduce),
  out_w sharded over vocab (4000 rows/core); only the last token's logits are
  computed.
- The residual stream lives TRANSPOSED in SBUF (xT: [H on partition-chunks,
  S free]); weights are pre-transposed on the host so every matmul contracts
  over the partition dim with no on-device weight transposes. V is re-
  transposed on the PE so attention*V contracts over key tokens.
- Scores come out directly as [ktok, qtok]; softmax sums are ones-vector
  matmuls on the PE; max-subtraction is skipped (scores are O(+-5)).
- All matmuls run in float32r (full-rate fp32, ~1e-4 rel err).
"""

import os
import sys

sys.path.insert(0, "/opt/trn_rl_repo")

import numpy as np

L = int(os.environ.get("KERNEL_DEV_L", "4"))
SKIP = set(os.environ.get("KERNEL_SKIP", "").split(","))
B, S, H, NH, HD = 1, 1024, 2048, 16, 128
V, P = 32000, 5632
NC = 8
FEAT = H // NC          # 256 q/k/v features per core (2 heads)
PC = P // NC            # 704 ffn rows per core
VC = V // NC            # 4000 vocab rows per core
KH = H // 128           # 16 H-chunks
KP = (PC + 127) // 128  # 6 pc-chunks (last is 64)
EPS = 1e-5
SCALE = float(np.sqrt(HD))
INV_SCALE = 1.0 / SCALE

_STATE = {}


def _build():
    import concourse.bass as bass
    import concourse.bacc as bacc
    from concourse import tile, mybir

    F32 = mybir.dt.float32
    F32R = mybir.dt.float32r
    F16 = mybir.dt.float16
    AF = mybir.ActivationFunctionType
    ALU = mybir.AluOpType
    ts = bass.ts

    nc = bacc.Bacc("TRN2", target_bir_lowering=False, debug=False, num_devices=NC)

    xT_h = nc.dram_tensor("xT", [H, S], F32, kind="ExternalInput")
    maskT_h = nc.dram_tensor("maskT", [S, S], F32, kind="ExternalInput")
    C_h = nc.dram_tensor("Cr", [128, S], F32R, kind="ExternalInput")
    S_h = nc.dram_tensor("Sr", [128, S], F32, kind="ExternalInput")
    J_h = nc.dram_tensor("J", [128, 128], F32R, kind="ExternalInput")
    id_h = nc.dram_tensor("ident", [128, 128], F32R, kind="ExternalInput")
    n1w_h = nc.dram_tensor("n1w", [128, L * KH], F32, kind="ExternalInput")
    n2w_h = nc.dram_tensor("n2w", [128, L * KH], F32, kind="ExternalInput")
    fw_h = nc.dram_tensor("fw", [128, KH], F32, kind="ExternalInput")
    # wq|wk|wv concatenated on the last axis: [L, H, 3*FEAT]
    wqkv_h = nc.dram_tensor("wqkvT", [L, H, 3 * FEAT], F32R, kind="ExternalInput")
    woT_h = nc.dram_tensor("woT", [L, FEAT, H], F32R, kind="ExternalInput")
    # w1|w3 interleaved by m-group: [w1 0:384 | w3 0:384 | w1 384:704 | w3 384:704]
    w13_h = nc.dram_tensor("w13T", [L, H, 2 * PC], F32R, kind="ExternalInput")
    w2T_h = nc.dram_tensor("w2T", [L, PC, H], F32R, kind="ExternalInput")
    owT_h = nc.dram_tensor("owT", [H, VC], F32R, kind="ExternalInput")
    out_h = nc.dram_tensor("logits", [1, VC], F32, kind="ExternalOutput")

    MW = [128] * (KP - 1) + [PC - 128 * (KP - 1)]   # 128 x5, 64
    MG_OFF = [0, 384]
    MG_WID = [384, PC - 384]

    from contextlib import ExitStack

    with tile.TileContext(nc) as tc, ExitStack() as _ctx:
        ec = _ctx.enter_context
        p_resid = ec(tc.tile_pool(name="resid", bufs=1))
        p_const = ec(tc.tile_pool(name="consts", bufs=1))
        p_row = ec(tc.tile_pool(name="row", bufs=2))
        p_big = ec(tc.tile_pool(name="big", bufs=4))
        p_vs = ec(tc.tile_pool(name="vsn", bufs=1))
        p_pt = ec(tc.tile_pool(name="ptile", bufs=3))
        p_f32 = ec(tc.tile_pool(name="f32t", bufs=2))
        p_t512 = ec(tc.tile_pool(name="t512", bufs=3))
        p_ns = ec(tc.tile_pool(name="normsc", bufs=3))
        p_wqkv = ec(tc.tile_pool(name="wqkv", bufs=2))
        p_w13 = ec(tc.tile_pool(name="w13", bufs=3))
        p_w2 = ec(tc.tile_pool(name="w2p", bufs=2))
        p_wo = ec(tc.tile_pool(name="wot", bufs=2))
        p_swig = ec(tc.tile_pool(name="swig", bufs=6))
        p_ar = ec(tc.tile_pool(name="ars", bufs=4))
        psum = ec(tc.tile_pool(name="psum", bufs=6, space="PSUM"))
        psum2 = ec(tc.tile_pool(name="psum2", bufs=1, space="PSUM"))
        dram = ec(tc.tile_pool(name="dram", bufs=4, space="DRAM"))

        xT = p_resid.tile([128, KH * S], F32, tag="xT")
        for hc in range(KH):
            nc.sync.dma_start(xT[:, ts(hc, S)], xT_h.ap()[ts(hc, 128), :])

        C_s = p_const.tile([128, S], F32R, tag="C")
        nc.sync.dma_start(C_s[:], C_h.ap())
        S_s = p_const.tile([128, S], F32, tag="S")
        nc.sync.dma_start(S_s[:], S_h.ap())
        J_r = p_const.tile([128, 128], F32R, tag="J")
        nc.sync.dma_start(J_r[:], J_h.ap())
        id_r = p_const.tile([128, 128], F32R, tag="id")
        nc.sync.dma_start(id_r[:], id_h.ap())
        n1w = p_const.tile([128, L * KH], F32, tag="n1w")
        nc.sync.dma_start(n1w[:], n1w_h.ap())
        n2w = p_const.tile([128, L * KH], F32, tag="n2w")
        nc.sync.dma_start(n2w[:], n2w_h.ap())
        fw_s = p_const.tile([128, KH], F32, tag="fw")
        nc.sync.dma_start(fw_s[:], fw_h.ap())
        ones_f = p_const.tile([128, 1], F32, tag="o1f")
        nc.vector.memset(ones_f[:], 1.0)
        ones_col = p_const.tile([128, 1], F32R, tag="o1")
        nc.vector.tensor_copy(ones_col[:], ones_f[:])
        ones_rf = p_const.tile([1, 128], F32, tag="orf")
        nc.vector.memset(ones_rf[:], 1.0)
        ones_row = p_const.tile([1, 128], F32R, tag="or")
        nc.vector.tensor_copy(ones_row[:], ones_rf[:])
        eps_t = p_const.tile([1, 1], F32, tag="eps")
        nc.vector.memset(eps_t[:], EPS)
        eps_p = p_const.tile([128, 1], F32, tag="epsp")
        nc.vector.memset(eps_p[:], EPS)
        ones_mf = p_const.tile([128, 128], F32, tag="omf")
        nc.vector.memset(ones_mf[:], 1.0)
        ones_mat = p_const.tile([128, 128], F32R, tag="om")
        nc.vector.tensor_copy(ones_mat[:], ones_mf[:])

        def norm_half(w_tile, l_, tk):
            """1/rms for tokens [tk*512, tk*512+512), bcast -> bc_s [128,512]."""
            ssum = psum.tile([1, 512], F32, tag="ps512", name="ssum")
            for hc in range(KH):
                sq = p_pt.tile([128, 512], F32R, tag="pt", name="sq")
                sl = slice(hc * S + tk * 512, hc * S + tk * 512 + 512)
                nc.vector.tensor_mul(sq[:], xT[:, sl], xT[:, sl])
                nc.tensor.matmul(ssum[:], ones_col[:], sq[:],
                                 start=(hc == 0), stop=(hc == KH - 1))
            rms = p_row.tile([1, 512], F32, tag="rms")
            nc.scalar.activation(rms[:], ssum[:], AF.Sqrt,
                                 bias=eps_t[:], scale=1.0 / H)
            inv = p_row.tile([1, 512], F32R, tag="inv")
            with nc.allow_low_precision(reason="f32r rounding of 1/rms"):
                nc.vector.reciprocal(inv[:], rms[:])
            bc_ps = psum.tile([128, 512], F32, tag="ps512", name="bcps")
            nc.tensor.matmul(bc_ps[:], ones_row[:], inv[:], start=True, stop=True)
            bc_s = p_f32.tile([128, 512], F32, tag="f32t", name="bcs")
            nc.scalar.activation(bc_s[:], bc_ps[:], AF.Copy)
            return bc_s

        def qkv_half(l_, tk, q_s, k_s, vT_s):
            """QKV for token half tk of layer l_ (writes [:, mt*S + tk*512])."""
            bc = norm_half(n1w, l_, tk)
            qp = [psum.tile([128, 512], F32, tag="ps512", name=f"qp{i}")
                  for i in range(2)]
            kp = [psum.tile([128, 512], F32, tag="ps512", name=f"kp{i}")
                  for i in range(2)]
            vp = [psum.tile([128, 512], F32, tag="ps512", name=f"vp{i}")
                  for i in range(2)]
            for hc in range(KH):
                xn = p_ns.tile([128, 512], F32R, tag="ns", name="xn")
                nc.vector.scalar_tensor_tensor(
                    xn[:], xT[:, hc * S + tk * 512: hc * S + tk * 512 + 512],
                    n1w[:, l_ * KH + hc: l_ * KH + hc + 1],
                    bc[:], op0=ALU.mult, op1=ALU.mult)
                wt = p_wqkv.tile([128, 3 * FEAT], F32R, tag="wqkv", name="wt")
                nc.sync.dma_start(wt[:], wqkv_h.ap()[l_, ts(hc, 128), :])
                st, sp = (hc == 0), (hc == KH - 1)
                for mt in range(2):
                    nc.tensor.matmul(qp[mt][:], wt[:, ts(mt, 128)], xn[:],
                                     start=st, stop=sp)
                    nc.tensor.matmul(kp[mt][:], wt[:, 256 + mt * 128: 384 + mt * 128],
                                     xn[:], start=st, stop=sp)
                    nc.tensor.matmul(vp[mt][:], wt[:, 512 + mt * 128: 640 + mt * 128],
                                     xn[:], start=st, stop=sp)
            for mt in range(2):
                off = mt * S + tk * 512
                nc.vector.tensor_copy(q_s[:, off:off + 512], qp[mt][:])
                nc.vector.tensor_copy(k_s[:, off:off + 512], kp[mt][:])
                nc.vector.tensor_copy(vT_s[:, off:off + 512], vp[mt][:])

        # ---- layer 0 QKV prologue ----
        cur_q = p_big.tile([128, 2 * S], F32R, tag="big", name="q0")
        cur_k = p_big.tile([128, 2 * S], F32R, tag="big", name="k0")
        cur_vT = p_big.tile([128, 2 * S], F32R, tag="big", name="vT0")
        for tk in range(2):
            qkv_half(0, tk, cur_q, cur_k, cur_vT)

        for l in range(L):
            last = (l == L - 1)
            q_s, k_s, vT_s = cur_q, cur_k, cur_vT

            # RoPE in place on q_s, k_s:  out = C*x + S'*(J@x)
            for t_s in (q_s, k_s):
                for mt in range(2):
                    for n in range(2):
                        sl = slice(mt * S + n * 512, mt * S + n * 512 + 512)
                        csl = slice(n * 512, n * 512 + 512)
                        j_ps = psum.tile([128, 512], F32, tag="ps512", name="jps")
                        nc.tensor.matmul(j_ps[:], J_r[:], t_s[:, sl],
                                         start=True, stop=True)
                        tmp = p_t512.tile([128, 512], F32R, tag="t512r",
                                          name="rtmp")
                        nc.vector.tensor_mul(tmp[:], C_s[:, csl], t_s[:, sl])
                        nc.vector.tensor_mul(t_s[:, sl], j_ps[:], S_s[:, csl])
                        nc.vector.tensor_add(t_s[:, sl], t_s[:, sl], tmp[:])

            # V -> natural layout [tok, feat] via PE transpose
            v_s = p_vs.tile([128, 8 * FEAT], F32R, tag="v", name="vs")
            for mt in range(2):
                for tb in range(8):
                    tp = psum.tile([128, 128], F32R, tag="ps512", name="tp")
                    nc.tensor.transpose(
                        tp[:], vT_s[:, mt * S + tb * 128: mt * S + tb * 128 + 128],
                        id_r[:])
                    nc.vector.tensor_copy(
                        v_s[:, tb * FEAT + mt * 128: tb * FEAT + mt * 128 + 128],
                        tp[:])

            attn_s = p_big.tile([128, 2 * S], F32R, tag="big", name="attn")

            if last:
                # only the last token's query matters (2-wide for ISA)
                for h in range(2):
                    at1 = psum.tile([128, 2], F32, tag="ps512", name="at1")
                    rs1 = psum.tile([128, 2], F32, tag="ps512", name="rs1")
                    for kc in range(8):
                        sc1 = psum.tile([128, 2], F32, tag="ps512", name="sc1")
                        nc.tensor.matmul(
                            sc1[:],
                            k_s[:, h * S + kc * 128: h * S + kc * 128 + 128],
                            q_s[:, h * S + S - 2: h * S + S],
                            start=True, stop=True)
                        mk1 = p_t512.tile([128, 2], F32, tag="mk1", name="mk1")
                        nc.sync.dma_start(mk1[:],
                                          maskT_h.ap()[ts(kc, 128), S - 2: S])
                        ex1 = p_t512.tile([128, 2], F32, tag="mk1", name="ex1")
                        nc.vector.scalar_tensor_tensor(
                            ex1[:], sc1[:], INV_SCALE, mk1[:],
                            op0=ALU.mult, op1=ALU.add)
                        pt1 = p_t512.tile([128, 2], F32R, tag="mk1", name="pt1")
                        nc.scalar.activation(pt1[:], ex1[:], AF.Exp)
                        st, sp = (kc == 0), (kc == 7)
                        nc.tensor.matmul(
                            at1[:],
                            v_s[:, kc * FEAT + h * 128: kc * FEAT + h * 128 + 128],
                            pt1[:], start=st, stop=sp)
                        nc.tensor.matmul(rs1[:], ones_mat[:], pt1[:],
                                         start=st, stop=sp)
                    inva = p_t512.tile([128, 2], F32, tag="mk1", name="inva")
                    nc.vector.reciprocal(inva[:], rs1[:])
                    nc.vector.tensor_mul(
                        attn_s[:, h * S + S - 2: h * S + S], at1[:], inva[:])

                # wo -> [H,2] AllReduce -> residual add (last token)
                ar_in = dram.tile([H, 2], F16, tag="arinL", name="arinL")
                ar_out = dram.tile([H, 2], F16, tag="aroutL",
                                   addr_space="Shared", name="aroutL")
                for hcb in range(8):
                    wo_t = [p_wo.tile([128, 256], F32R, tag="wo",
                                      name=f"wotL{i}") for i in range(2)]
                    for fc in range(2):
                        nc.sync.dma_start(
                            wo_t[fc][:],
                            woT_h.ap()[l, ts(fc, 128), hcb * 256: hcb * 256 + 256])
                    for hh in range(2):
                        hc = hcb * 2 + hh
                        poL = psum.tile([128, 2], F32, tag="ps512", name="poL")
                        for fc in range(2):
                            nc.tensor.matmul(
                                poL[:], wo_t[fc][:, ts(hh, 128)],
                                attn_s[:, fc * S + S - 2: fc * S + S],
                                start=(fc == 0), stop=(fc == 1))
                        arL = p_ar.tile([128, 2], F16, tag="arL", name="arL")
                        nc.scalar.activation(arL[:], poL[:], AF.Copy)
                        nc.sync.dma_start(ar_in[ts(hc, 128), :], arL[:])
                nc.gpsimd.collective_compute(
                    "AllReduce", ALU.add, replica_groups=[list(range(NC))],
                    ins=[ar_in[:].opt()], outs=[ar_out[:].opt()])
                for hc in range(KH):
                    ar_t = p_ar.tile([128, 2], F16, tag="arL", name="art")
                    nc.sync.dma_start(ar_t[:], ar_out[ts(hc, 128), :])
                    nc.vector.tensor_add(
                        xT[:, hc * S + S - 2: hc * S + S],
                        xT[:, hc * S + S - 2: hc * S + S], ar_t[:])

                # norm2 + FFN on the last 2 tokens
                sqL = p_row.tile([128, 2 * KH], F32R, tag="sql2")
                for hc in range(KH):
                    col = hc * S + S - 2
                    nc.vector.tensor_mul(sqL[:, 2 * hc:2 * hc + 2],
                                         xT[:, col:col + 2], xT[:, col:col + 2])
                ssL = psum.tile([128, 2 * KH], F32, tag="ps512", name="ssL")
                nc.tensor.matmul(ssL[:], ones_mat[:], sqL[:],
                                 start=True, stop=True)
                ssr = p_row.tile([128, 2], F32, tag="ssr")
                nc.vector.reduce_sum(
                    ssr[:], ssL[:].rearrange("p (c two) -> p two c", two=2),
                    axis=mybir.AxisListType.X)
                rmsL = p_row.tile([128, 2], F32, tag="rmsL")
                nc.scalar.activation(rmsL[:], ssr[:], AF.Sqrt,
                                     bias=eps_p[:], scale=1.0 / H)
                invL = p_row.tile([128, 2], F32, tag="invLc")
                nc.vector.reciprocal(invL[:], rmsL[:])
                hnL = p_row.tile([128, 2 * KH], F32R, tag="hnL")
                tnL = p_row.tile([128, 2], F32, tag="tnL")
                for hc in range(KH):
                    col = hc * S + S - 2
                    nc.vector.tensor_scalar_mul(
                        tnL[:], xT[:, col:col + 2],
                        n2w[:, l * KH + hc: l * KH + hc + 1])
                    nc.vector.tensor_mul(hnL[:, 2 * hc:2 * hc + 2],
                                         tnL[:], invL[:])
                swigL = p_row.tile([128, 2 * KP], F32R, tag="swL")
                for mg in range(2):
                    mts = [0, 1, 2] if mg == 0 else [3, 4, 5]
                    w_off, w_wid = MG_OFF[mg], MG_WID[mg]
                    gL = {mt: psum.tile([128, 2], F32, tag="ps512",
                                        name=f"gL{mt}") for mt in mts}
                    uL = {mt: psum.tile([128, 2], F32, tag="ps512",
                                        name=f"uL{mt}") for mt in mts}
                    for hc in range(KH):
                        wt13 = p_w13.tile([128, 2 * 384], F32R, tag="w13",
                                          name="wt13L")
                        nc.sync.dma_start(
                            wt13[:, :2 * w_wid],
                            w13_h.ap()[l, ts(hc, 128),
                                       2 * w_off: 2 * w_off + 2 * w_wid])
                        st, sp = (hc == 0), (hc == KH - 1)
                        for i, mt in enumerate(mts):
                            w = min(128, w_wid - i * 128)
                            nc.tensor.matmul(
                                gL[mt][:w, :], wt13[:, i * 128: i * 128 + w],
                                hnL[:, 2 * hc:2 * hc + 2], start=st, stop=sp)
                            nc.tensor.matmul(
                                uL[mt][:w, :],
                                wt13[:, w_wid + i * 128: w_wid + i * 128 + w],
                                hnL[:, 2 * hc:2 * hc + 2], start=st, stop=sp)
                    for mt in mts:
                        kw = MW[mt]
                        gsL = p_row.tile([128, 2], F32, tag="gsL")
                        nc.scalar.activation(gsL[:kw, :], gL[mt][:kw, :], AF.Silu)
                        nc.vector.tensor_mul(swigL[:kw, 2 * mt:2 * mt + 2],
                                             uL[mt][:kw, :], gsL[:kw, :])
                ar2_in = dram.tile([H, 2], F16, tag="arinL", name="ar2inL")
                ar2_out = dram.tile([H, 2], F16, tag="aroutL",
                                    addr_space="Shared", name="ar2outL")
                for hc in range(KH):
                    p2L = psum.tile([128, 2], F32, tag="ps512", name="p2L")
                    for kc in range(KP):
                        kw = MW[kc]
                        w2_t = p_w2.tile([128, 128], F32R, tag="w2", name="w2tL")
                        nc.sync.dma_start(
                            w2_t[:kw, :],
                            w2T_h.ap()[l, kc * 128: kc * 128 + kw,
                                       hc * 128: hc * 128 + 128])
                        nc.tensor.matmul(p2L[:], w2_t[:kw, :],
                                         swigL[:kw, 2 * kc:2 * kc + 2],
                                         start=(kc == 0), stop=(kc == KP - 1))
                    a2L = p_ar.tile([128, 2], F16, tag="arL", name="a2L")
                    nc.scalar.activation(a2L[:], p2L[:], AF.Copy)
                    nc.sync.dma_start(ar2_in[ts(hc, 128), :], a2L[:])
                nc.gpsimd.collective_compute(
                    "AllReduce", ALU.add, replica_groups=[list(range(NC))],
                    ins=[ar2_in[:].opt()], outs=[ar2_out[:].opt()])
                for hc in range(KH):
                    ar_t = p_ar.tile([128, 2], F16, tag="arL", name="art2")
                    nc.sync.dma_start(ar_t[:], ar2_out[ts(hc, 128), :])
                    nc.vector.tensor_add(
                        xT[:, hc * S + S - 2: hc * S + S],
                        xT[:, hc * S + S - 2: hc * S + S], ar_t[:])
                continue

            # ---- non-last layer: attention for both halves, then the
            # token-half-pipelined tail (wo->AR1->norm2->FFN->AR2->next QKV)
            for tk in range(2):
                for h in range(2):
                    at_ps = psum.tile([128, 512], F32, tag="ps512", name="atp")
                    rs_ps = psum.tile([1, 512], F32, tag="ps512", name="rsp")
                    for kc in range(8):
                        sc_ps = psum.tile([128, 512], F32, tag="ps512",
                                          name="scp")
                        nc.tensor.matmul(
                            sc_ps[:],
                            k_s[:, h * S + kc * 128: h * S + kc * 128 + 128],
                            q_s[:, h * S + tk * 512: h * S + tk * 512 + 512],
                            start=True, stop=True)
                        mk = p_t512.tile([128, 512], F32, tag="t512f", name="mk")
                        nc.sync.dma_start(
                            mk[:], maskT_h.ap()[ts(kc, 128), ts(tk, 512)])
                        ex = p_t512.tile([128, 512], F32, tag="t512f", name="ex")
                        nc.vector.scalar_tensor_tensor(
                            ex[:], sc_ps[:], INV_SCALE, mk[:],
                            op0=ALU.mult, op1=ALU.add)
                        pt = p_pt.tile([128, 512], F32R, tag="pt", name="ptl")
                        nc.scalar.activation(pt[:], ex[:], AF.Exp)
                        st, sp = (kc == 0), (kc == 7)
                        nc.tensor.matmul(
                            at_ps[:],
                            v_s[:, kc * FEAT + h * 128: kc * FEAT + h * 128 + 128],
                            pt[:], start=st, stop=sp)
                        nc.tensor.matmul(rs_ps[:], ones_col[:], pt[:],
                                         start=st, stop=sp)
                    inv = p_row.tile([1, 512], F32R, tag="inv", name="ainv")
                    with nc.allow_low_precision(reason="f32r 1/sum"):
                        nc.vector.reciprocal(inv[:], rs_ps[:])
                    ib_ps = psum.tile([128, 512], F32, tag="ps512", name="ibp")
                    nc.tensor.matmul(ib_ps[:], ones_row[:], inv[:],
                                     start=True, stop=True)
                    ib_s = p_f32.tile([128, 512], F32, tag="f32t", name="ibs")
                    nc.scalar.activation(ib_s[:], ib_ps[:], AF.Copy)
                    nc.vector.tensor_mul(
                        attn_s[:, h * S + tk * 512: h * S + tk * 512 + 512],
                        at_ps[:], ib_s[:])

            if l + 1 < L:
                nxt_q = p_big.tile([128, 2 * S], F32R, tag="big", name="qn")
                nxt_k = p_big.tile([128, 2 * S], F32R, tag="big", name="kn")
                nxt_vT = p_big.tile([128, 2 * S], F32R, tag="big", name="vTn")

            ar1_bufs = []
            ar2_bufs = []
            for tk in range(2):
                # wo projection for this token half
                ar_in = dram.tile([H, 512], F16, tag="arin", name="arin")
                ar_out = dram.tile([H, 512], F16, tag="arout",
                                   addr_space="Shared", name="arout")
                ar1_bufs.append((ar_in, ar_out))
                for hcb in range(8):
                    wo_t = [p_wo.tile([128, 256], F32R, tag="wo",
                                      name=f"wot{i}") for i in range(2)]
                    for fc in range(2):
                        nc.sync.dma_start(
                            wo_t[fc][:],
                            woT_h.ap()[l, ts(fc, 128), hcb * 256: hcb * 256 + 256])
                    for hh in range(2):
                        hc = hcb * 2 + hh
                        po = psum.tile([128, 512], F32, tag="ps512", name="po")
                        for fc in range(2):
                            nc.tensor.matmul(
                                po[:], wo_t[fc][:, ts(hh, 128)],
                                attn_s[:, fc * S + tk * 512: fc * S + tk * 512 + 512],
                                start=(fc == 0), stop=(fc == 1))
                        ar_sb = p_ar.tile([128, 512], F16, tag="ar", name="arsb")
                        nc.scalar.activation(ar_sb[:], po[:], AF.Copy)
                        nc.sync.dma_start(ar_in[ts(hc, 128), :], ar_sb[:])
                ar_mid = dram.tile([H // NC, 512], F16, tag="armid",
                                   name="armid")
                nc.gpsimd.collective_compute(
                    "ReduceScatter", ALU.add, replica_groups=[list(range(NC))],
                    ins=[ar_in[:].opt()], outs=[ar_mid[:].opt()])
                nc.gpsimd.collective_compute(
                    "AllGather", ALU.bypass, replica_groups=[list(range(NC))],
                    ins=[ar_mid[:].opt()], outs=[ar_out[:].opt()])

            for tk in range(2):
                ar_in, ar_out = ar1_bufs[tk]
                for hc in range(KH):
                    ar_t = p_ar.tile([128, 512], F16, tag="ar", name="art")
                    nc.sync.dma_start(ar_t[:], ar_out[ts(hc, 128), :])
                    nc.vector.tensor_add(
                        xT[:, hc * S + tk * 512: hc * S + tk * 512 + 512],
                        xT[:, hc * S + tk * 512: hc * S + tk * 512 + 512],
                        ar_t[:])

                # norm2 + FFN for this half
                bc2 = norm_half(n2w, l, tk)
                if tk == 0:
                    swig = [p_swig.tile([128, S], F32R, tag="sw",
                                        name=f"swig{i}") for i in range(KP)]
                for mg in range(2):
                    mts = [0, 1, 2] if mg == 0 else [3, 4, 5]
                    w_off, w_wid = MG_OFF[mg], MG_WID[mg]
                    gp = {mt: psum.tile([128, 512], F32, tag="ps512",
                                        name=f"gp{mt}") for mt in mts}
                    up = {mt: psum.tile([128, 512], F32, tag="ps512",
                                        name=f"up{mt}") for mt in mts}
                    for hc in range(KH):
                        hn = p_ns.tile([128, 512], F32R, tag="ns", name="hn")
                        nc.vector.scalar_tensor_tensor(
                            hn[:],
                            xT[:, hc * S + tk * 512: hc * S + tk * 512 + 512],
                            n2w[:, l * KH + hc: l * KH + hc + 1],
                            bc2[:], op0=ALU.mult, op1=ALU.mult)
                        wt13 = p_w13.tile([128, 2 * 384], F32R, tag="w13",
                                          name="wt13")
                        nc.sync.dma_start(
                            wt13[:, :2 * w_wid],
                            w13_h.ap()[l, ts(hc, 128),
                                       2 * w_off: 2 * w_off + 2 * w_wid])
                        st, sp = (hc == 0), (hc == KH - 1)
                        for i, mt in enumerate(mts):
                            w = min(128, w_wid - i * 128)
                            nc.tensor.matmul(
                                gp[mt][:w, :], wt13[:, i * 128: i * 128 + w],
                                hn[:], start=st, stop=sp)
                            nc.tensor.matmul(
                                up[mt][:w, :],
                                wt13[:, w_wid + i * 128: w_wid + i * 128 + w],
                                hn[:], start=st, stop=sp)
                    for i, mt in enumerate(mts):
                        w = MW[mt]
                        gs = p_t512.tile([128, 512], F32, tag="t512f", name="gs")
                        nc.scalar.activation(gs[:w, :], gp[mt][:w, :], AF.Silu)
                        nc.vector.tensor_mul(
                            swig[mt][:w, tk * 512: tk * 512 + 512],
                            up[mt][:w, :], gs[:w, :])

                # down projection for this half
                ar2_in = dram.tile([H, 512], F16, tag="arin", name="ar2in")
                ar2_out = dram.tile([H, 512], F16, tag="arout",
                                    addr_space="Shared", name="ar2out")
                ar2_bufs.append((ar2_in, ar2_out))
                for hcb in range(4):
                    p2 = [psum.tile([128, 512], F32, tag="ps512",
                                    name=f"p2p{i}") for i in range(4)]
                    for kc in range(KP):
                        kw = MW[kc]
                        w2_t = p_w2.tile([128, 512], F32R, tag="w2", name="w2t")
                        nc.sync.dma_start(
                            w2_t[:kw, :],
                            w2T_h.ap()[l, kc * 128: kc * 128 + kw,
                                       hcb * 512: hcb * 512 + 512])
                        for hh in range(4):
                            nc.tensor.matmul(
                                p2[hh][:], w2_t[:kw, ts(hh, 128)],
                                swig[kc][:kw, tk * 512: tk * 512 + 512],
                                start=(kc == 0), stop=(kc == KP - 1))
                    for hh in range(4):
                        hc = hcb * 4 + hh
                        a2 = p_ar.tile([128, 512], F16, tag="ar", name="a2")
                        nc.scalar.activation(a2[:], p2[hh][:], AF.Copy)
                        nc.sync.dma_start(ar2_in[ts(hc, 128), :], a2[:])
                ar2_mid = dram.tile([H // NC, 512], F16, tag="armid",
                                    name="ar2mid")
                nc.gpsimd.collective_compute(
                    "ReduceScatter", ALU.add, replica_groups=[list(range(NC))],
                    ins=[ar2_in[:].opt()], outs=[ar2_mid[:].opt()])
                nc.gpsimd.collective_compute(
                    "AllGather", ALU.bypass, replica_groups=[list(range(NC))],
                    ins=[ar2_mid[:].opt()], outs=[ar2_out[:].opt()])

            for tk in range(2):
                ar2_in, ar2_out = ar2_bufs[tk]
                for hc in range(KH):
                    ar_t = p_ar.tile([128, 512], F16, tag="ar", name="art2")
                    nc.sync.dma_start(ar_t[:], ar2_out[ts(hc, 128), :])
                    nc.vector.tensor_add(
                        xT[:, hc * S + tk * 512: hc * S + tk * 512 + 512],
                        xT[:, hc * S + tk * 512: hc * S + tk * 512 + 512],
                        ar_t[:])

                # next layer's QKV for this half (overlaps the other AR)
                if l + 1 < L:
                    qkv_half(l + 1, tk, nxt_q, nxt_k, nxt_vT)

            if l + 1 < L:
                cur_q, cur_k, cur_vT = nxt_q, nxt_k, nxt_vT

        # ======== final norm (last token only) + logits ========
        sq_l = p_row.tile([128, KH], F32R, tag="sql")
        for hc in range(KH):
            col = hc * S + S - 1
            nc.vector.tensor_mul(sq_l[:, hc:hc + 1], xT[:, col:col + 1],
                                 xT[:, col:col + 1])
        sl_ps = psum.tile([1, KH], F32, tag="ps512", name="slps")
        nc.tensor.matmul(sl_ps[:], ones_col[:], sq_l[:], start=True, stop=True)
        ssc = p_row.tile([1, 1], F32, tag="ssc")
        nc.vector.reduce_sum(ssc[:], sl_ps[:], axis=mybir.AxisListType.X)
        rms_l = p_row.tile([1, 1], F32, tag="rmsl")
        nc.scalar.activation(rms_l[:], ssc[:], AF.Sqrt, bias=eps_t[:],
                             scale=1.0 / H)
        inv_l = p_row.tile([1, 1], F32, tag="invl")
        nc.vector.reciprocal(inv_l[:], rms_l[:])
        xnl = p_row.tile([128, KH], F32R, tag="xnl")
        for hc in range(KH):
            col = hc * S + S - 1
            nc.vector.tensor_mul(xnl[:, hc:hc + 1], xT[:, col:col + 1],
                                 fw_s[:, hc:hc + 1])
        for n in range(8):
            lg_ps = psum.tile([1, 500], F32, tag="ps512", name="lgps")
            for hc in range(KH):
                ow_t = p_w2.tile([128, 500], F32R, tag="w2", name="owt")
                nc.sync.dma_start(
                    ow_t[:], owT_h.ap()[ts(hc, 128), n * 500: n * 500 + 500])
                nc.tensor.matmul(lg_ps[:], xnl[:, hc: hc + 1], ow_t[:],
                                 start=(hc == 0), stop=(hc == KH - 1))
            lg = p_row.tile([1, 500], F32, tag="lg")
            nc.scalar.activation(lg[:], lg_ps[:], AF.Copy, scale=inv_l[:])
            nc.sync.dma_start(out_h.ap()[:, n * 500: n * 500 + 500], lg[:])

    nc.compile()
    return nc


def _shard(inputs):
    x = np.asarray(inputs["x"], np.float32)
    mask = np.asarray(inputs["attn_mask"], np.float32)
    cos = np.asarray(inputs["cos"], np.float32).reshape(S, HD // 2)
    sin = np.asarray(inputs["sin"], np.float32).reshape(S, HD // 2)
    n1 = np.asarray(inputs["norm1_w"], np.float32)[:L]
    n2 = np.asarray(inputs["norm2_w"], np.float32)[:L]
    fw = np.asarray(inputs["final_norm_w"], np.float32)
    wq = np.asarray(inputs["wq"], np.float32)[:L]
    wk = np.asarray(inputs["wk"], np.float32)[:L]
    wv = np.asarray(inputs["wv"], np.float32)[:L]
    wo = np.asarray(inputs["wo"], np.float32)[:L]
    w1 = np.asarray(inputs["w1"], np.float32)[:L]
    w3 = np.asarray(inputs["w3"], np.float32)[:L]
    w2 = np.asarray(inputs["w2"], np.float32)[:L]
    ow = np.asarray(inputs["out_w"], np.float32)

    xT = np.ascontiguousarray(x[0].T)
    maskT = np.ascontiguousarray(mask[0].T)
    C = np.empty((128, S), np.float32)
    C[0::2] = cos.T
    C[1::2] = cos.T
    Sm = np.empty((128, S), np.float32)
    Sm[0::2] = -sin.T
    Sm[1::2] = sin.T
    J = np.zeros((128, 128), np.float32)
    idx = np.arange(0, 128, 2)
    J[idx, idx + 1] = 1.0
    J[idx + 1, idx] = 1.0
    ident = np.eye(128, dtype=np.float32)
    n1w = np.ascontiguousarray(
        n1.reshape(L, KH, 128).transpose(2, 0, 1).reshape(128, L * KH))
    n2w = np.ascontiguousarray(
        n2.reshape(L, KH, 128).transpose(2, 0, 1).reshape(128, L * KH))
    fwh = np.ascontiguousarray(fw.reshape(KH, 128).T)

    common = dict(xT=xT, maskT=maskT, Cr=C, Sr=Sm, J=J, ident=ident,
                  n1w=n1w, n2w=n2w, fw=fwh)
    in_maps = []
    for c in range(NC):
        fs = slice(c * FEAT, (c + 1) * FEAT)
        ps = slice(c * PC, (c + 1) * PC)
        vs = slice(c * VC, (c + 1) * VC)
        m = dict(common)
        wqT = wq[:, fs, :].transpose(0, 2, 1)
        wkT = wk[:, fs, :].transpose(0, 2, 1)
        wvT = wv[:, fs, :].transpose(0, 2, 1)
        m["wqkvT"] = np.ascontiguousarray(
            np.concatenate([wqT, wkT, wvT], axis=2))
        m["woT"] = np.ascontiguousarray(wo[:, :, fs].transpose(0, 2, 1))
        w1T = w1[:, ps, :].transpose(0, 2, 1)
        w3T = w3[:, ps, :].transpose(0, 2, 1)
        m["w13T"] = np.ascontiguousarray(np.concatenate(
            [w1T[:, :, 0:384], w3T[:, :, 0:384],
             w1T[:, :, 384:], w3T[:, :, 384:]], axis=2))
        m["w2T"] = np.ascontiguousarray(w2[:, :, ps].transpose(0, 2, 1))
        m["owT"] = np.ascontiguousarray(ow[vs, :].T)
        in_maps.append(m)
    return in_maps


def kernel(**inputs) -> np.ndarray:
    from concourse import bass_utils

    if "nc" not in _STATE:
        _STATE["nc"] = _build()
    in_maps = _shard(inputs)
    res = bass_utils.run_bass_kernel_spmd(
        _STATE["nc"], in_maps, core_ids=list(range(NC)))
    out = np.concatenate(
        [res.results[c]["logits"] for c in range(NC)], axis=1)
    return out.astype(np.float32)



# revision 3
# speedup vs baseline: 1.6256x; 1.6256x over previous
"""Trainium2 Bass kernel: 4-layer decoder prefill (S=1024, H=2048, NH=16, HD=128,
FFN=5632, V=32000), tensor-parallel over 8 NeuronCores.

- Megatron TP over 8 cores: wq/wk/wv/w1/w3 sharded on output dim (2 heads /
  704 ffn rows per core), wo/w2 sharded on input dim (partials ->
  ReduceScatter+AllGather), out_w sharded over vocab (4000 rows/core); only
  the last token's logits are computed.
- All matmuls in fp16 (weights pre-cast on host, fp32 accumulation in PSUM);
  the residual stream lives TRANSPOSED in SBUF as fp16 (xT: [H on
  partition-chunks, S free]).
- Causal structure exploited: fully-masked score blocks are skipped; diagonal
  blocks use 4 precomputed multiplicative 0/1 mask tiles; 1/sqrt(HD) is
  folded into the Exp activation scale.
- Softmax denominators and rms-norm sums are accumulated as PE matmuls with
  an all-ones [128,128] stationary, which broadcasts the partition-sum to all
  128 partitions directly -- no slow [1,N] single-partition ops; inverses via
  reciprocal_approx_fast (single DVE op).
- wqkv/wo are SBUF-resident per layer; w13/w2 streamed; ffn w1|w3 are
  zero-padded to 768 rows each so all chunks are full 128 partitions.
- Last layer: k/v for all tokens but q/attention/FFN only for the last
  tokens; logits fp16 GEMV streamed over the vocab shard.
"""

import os
import sys

sys.path.insert(0, "/opt/trn_rl_repo")

import numpy as np

L = 4
B, S, H, NH, HD = 1, 1024, 2048, 16, 128
V, P = 32000, 5632
NC = 8
FEAT = H // NC          # 256 q/k/v features per core (2 heads)
PC = P // NC            # 704 ffn rows per core
PCP = 768               # padded to 6 full 128-chunks
VC = V // NC            # 4000 vocab rows per core
KH = H // 128           # 16 H-chunks
EPS = 1e-5
SCALE = float(np.sqrt(HD))
INV_SCALE = 1.0 / SCALE

_STATE = {}


def _build():
    import concourse.bass as bass
    import concourse.bacc as bacc
    from concourse import tile, mybir

    F32 = mybir.dt.float32
    F16 = mybir.dt.float16
    AF = mybir.ActivationFunctionType
    ALU = mybir.AluOpType
    ts = bass.ts

    nc = bacc.Bacc("TRN2", target_bir_lowering=False, debug=False, num_devices=NC)

    xT_h = nc.dram_tensor("xT", [H, S], F16, kind="ExternalInput")
    C_h = nc.dram_tensor("Cr", [128, S], F16, kind="ExternalInput")
    S_h = nc.dram_tensor("Sr", [128, S], F16, kind="ExternalInput")
    J_h = nc.dram_tensor("J", [128, 128], F16, kind="ExternalInput")
    id_h = nc.dram_tensor("ident", [128, 128], F16, kind="ExternalInput")
    dm_h = nc.dram_tensor("dmask", [128, 4 * 512], F16, kind="ExternalInput")
    n1w_h = nc.dram_tensor("n1w", [128, L * KH], F32, kind="ExternalInput")
    n2w_h = nc.dram_tensor("n2w", [128, L * KH], F32, kind="ExternalInput")
    fw_h = nc.dram_tensor("fw", [128, KH], F32, kind="ExternalInput")
    # wq|wk|wv concatenated on the last axis: [L, H, 3*FEAT]
    wqkv_h = nc.dram_tensor("wqkvT", [L, H, 3 * FEAT], F16, kind="ExternalInput")
    woT_h = nc.dram_tensor("woT", [L, FEAT, H], F16, kind="ExternalInput")
    # [w1 | 64pad | w3 | 64pad] on cols: [L, H, 2*PCP]
    w13_h = nc.dram_tensor("w13T", [L, H, 2 * PCP], F16, kind="ExternalInput")
    w2T_h = nc.dram_tensor("w2T", [L, PCP, H], F16, kind="ExternalInput")
    owT_h = nc.dram_tensor("owT", [H, VC], F16, kind="ExternalInput")
    out_h = nc.dram_tensor("logits", [1, VC], F32, kind="ExternalOutput")

    from contextlib import ExitStack

    with tile.TileContext(nc) as tc, ExitStack() as _ctx:
        ec = _ctx.enter_context
        p_resid = ec(tc.tile_pool(name="resid", bufs=1))
        p_const = ec(tc.tile_pool(name="consts", bufs=1))
        p_big = ec(tc.tile_pool(name="big", bufs=3))
        p_vs = ec(tc.tile_pool(name="vsn", bufs=2))
        p_attn = ec(tc.tile_pool(name="attnp", bufs=2))
        p_pt = ec(tc.tile_pool(name="ptile", bufs=3))
        p_ns = ec(tc.tile_pool(name="normsc", bufs=3))
        p_nrm = ec(tc.tile_pool(name="nrm", bufs=2))
        p_gu = ec(tc.tile_pool(name="gup", bufs=2))
        p_wres = ec(tc.tile_pool(name="wres", bufs=1))
        p_w13 = ec(tc.tile_pool(name="w13p", bufs=3))
        p_w2 = ec(tc.tile_pool(name="w2p", bufs=4))
        p_ow = ec(tc.tile_pool(name="owp", bufs=6))
        p_ar = ec(tc.tile_pool(name="ars", bufs=6))
        p_row = ec(tc.tile_pool(name="row", bufs=2))
        psum = ec(tc.tile_pool(name="psum", bufs=7, space="PSUM"))
        dram = ec(tc.tile_pool(name="dram", bufs=4, space="DRAM"))

        # ---- constants / inputs ----
        xT = p_resid.tile([128, KH * S], F16, tag="xT")
        for hc in range(KH):
            nc.sync.dma_start(xT[:, ts(hc, S)], xT_h.ap()[ts(hc, 128), :])

        C_s = p_const.tile([128, S], F16, tag="C")
        nc.sync.dma_start(C_s[:], C_h.ap())
        S_s = p_const.tile([128, S], F16, tag="S")
        nc.sync.dma_start(S_s[:], S_h.ap())
        J_r = p_const.tile([128, 128], F16, tag="J")
        nc.sync.dma_start(J_r[:], J_h.ap())
        id_r = p_const.tile([128, 128], F16, tag="id")
        nc.sync.dma_start(id_r[:], id_h.ap())
        dmask = p_const.tile([128, 4 * 512], F16, tag="dm")
        nc.sync.dma_start(dmask[:], dm_h.ap())
        n1w = p_const.tile([128, L * KH], F32, tag="n1w")
        nc.sync.dma_start(n1w[:], n1w_h.ap())
        n2w = p_const.tile([128, L * KH], F32, tag="n2w")
        nc.sync.dma_start(n2w[:], n2w_h.ap())
        fw_s = p_const.tile([128, KH], F32, tag="fw")
        nc.sync.dma_start(fw_s[:], fw_h.ap())
        ones_mat = p_const.tile([128, 128], F16, tag="om")
        nc.vector.memset(ones_mat[:], 1.0)
        ones_col = p_const.tile([128, 1], F16, tag="o1")
        nc.vector.memset(ones_col[:], 1.0)
        eps_p = p_const.tile([128, 1], F32, tag="epsp")
        nc.vector.memset(eps_p[:], EPS)
        eps_t = p_const.tile([1, 1], F32, tag="eps")
        nc.vector.memset(eps_t[:], EPS)

        def load_wo(l_):
            wo_sb = p_wres.tile([128, 2 * H], F16, tag="wo", name="wosb")
            for fc in range(2):
                nc.sync.dma_start(wo_sb[:, ts(fc, H)],
                                  woT_h.ap()[l_, ts(fc, 128), :])
            return wo_sb

        def load_wqkv(l_):
            wq_sb = p_wres.tile([128, KH * 3 * FEAT], F16, tag="wqkv",
                                name="wqsb")
            for hc in range(KH):
                nc.sync.dma_start(wq_sb[:, ts(hc, 3 * FEAT)],
                                  wqkv_h.ap()[l_, ts(hc, 128), :])
            return wq_sb

        def norm_inv(w_tile, l_, tk):
            """[128,512] fp32 tile of 1/rms for tokens [tk*512, tk*512+512)."""
            nb_ps = psum.tile([128, 512], F32, tag="ps", name="nbps")
            for hc in range(KH):
                sq = p_ns.tile([128, 512], F16, tag="sq", name="sq")
                sl = slice(hc * S + tk * 512, hc * S + tk * 512 + 512)
                nc.vector.tensor_mul(sq[:], xT[:, sl], xT[:, sl])
                nc.tensor.matmul(nb_ps[:], ones_mat[:], sq[:],
                                 start=(hc == 0), stop=(hc == KH - 1))
            rms = p_nrm.tile([128, 512], F32, tag="rms", name="rms")
            nc.scalar.activation(rms[:], nb_ps[:], AF.Sqrt,
                                 bias=eps_p[:], scale=1.0 / H)
            inv = p_nrm.tile([128, 512], F32, tag="inv", name="inv")
            nc.vector.reciprocal_approx_fast(inv[:], rms[:])
            return inv

        def qkv_half(wq_sb, l_, tk, q_s, k_s, vT_s):
            """QKV for token half tk of layer l_ (writes [:, mt*S+tk*512]).

            For the last layer, q is computed only for the last 2 tokens
            (tk==1) into q_s[:, mt*S + S-2 : mt*S + S]."""
            last = (l_ == L - 1)
            inv = norm_inv(n1w, l_, tk)
            kp = [psum.tile([128, 512], F32, tag="ps", name=f"kp{i}")
                  for i in range(2)]
            vp = [psum.tile([128, 512], F32, tag="ps", name=f"vp{i}")
                  for i in range(2)]
            if not last:
                qp = [psum.tile([128, 512], F32, tag="ps", name=f"qp{i}")
                      for i in range(2)]
            elif tk == 1:
                qp = [psum.tile([128, 2], F32, tag="ps", name=f"qL{i}")
                      for i in range(2)]
            else:
                qp = None
            for hc in range(KH):
                xn = p_ns.tile([128, 512], F16, tag="ns", name="xn")
                nc.vector.scalar_tensor_tensor(
                    xn[:], xT[:, hc * S + tk * 512: hc * S + tk * 512 + 512],
                    n1w[:, l_ * KH + hc: l_ * KH + hc + 1],
                    inv[:], op0=ALU.mult, op1=ALU.mult)
                st, sp = (hc == 0), (hc == KH - 1)
                for mt in range(2):
                    if not last:
                        nc.tensor.matmul(
                            qp[mt][:], wq_sb[:, hc * 768 + mt * 128:
                                             hc * 768 + mt * 128 + 128],
                            xn[:], start=st, stop=sp)
                    elif tk == 1:
                        nc.tensor.matmul(
                            qp[mt][:], wq_sb[:, hc * 768 + mt * 128:
                                             hc * 768 + mt * 128 + 128],
                            xn[:, 510:512], start=st, stop=sp)
                    nc.tensor.matmul(
                        kp[mt][:], wq_sb[:, hc * 768 + 256 + mt * 128:
                                         hc * 768 + 256 + mt * 128 + 128],
                        xn[:], start=st, stop=sp)
                    nc.tensor.matmul(
                        vp[mt][:], wq_sb[:, hc * 768 + 512 + mt * 128:
                                         hc * 768 + 512 + mt * 128 + 128],
                        xn[:], start=st, stop=sp)
            for mt in range(2):
                off = mt * S + tk * 512
                if not last:
                    nc.vector.tensor_copy(q_s[:, off:off + 512], qp[mt][:])
                elif tk == 1:
                    nc.vector.tensor_copy(q_s[:, mt * S + S - 2: mt * S + S],
                                          qp[mt][:])
                nc.vector.tensor_copy(k_s[:, off:off + 512], kp[mt][:])
                nc.vector.tensor_copy(vT_s[:, off:off + 512], vp[mt][:])

        def rope_slice(t_s, col, width, ccol):
            """RoPE in place on t_s[:, col:col+width]; cos/sin cols at ccol."""
            j_ps = psum.tile([128, 512], F32, tag="ps", name="jps")
            nc.tensor.matmul(j_ps[:, :width], J_r[:], t_s[:, col:col + width],
                             start=True, stop=True)
            tmp = p_pt.tile([128, 512], F16, tag="rtmp", name="rtmp")
            nc.vector.tensor_mul(tmp[:, :width], C_s[:, ccol:ccol + width],
                                 t_s[:, col:col + width])
            nc.vector.tensor_mul(t_s[:, col:col + width], j_ps[:, :width],
                                 S_s[:, ccol:ccol + width])
            nc.vector.tensor_add(t_s[:, col:col + width],
                                 t_s[:, col:col + width], tmp[:, :width])

        def wo_project(wo_sb, attn_s, tk):
            """wo @ attn for half tk -> DRAM ar_in; RS+AG; returns ar_out."""
            ar_in = dram.tile([H, 512], F16, tag="arin", name="arin")
            ar_out = dram.tile([H, 512], F16, tag="arout",
                               addr_space="Shared", name="arout")
            for hc in range(KH):
                po = psum.tile([128, 512], F32, tag="ps", name="po")
                for fc in range(2):
                    nc.tensor.matmul(
                        po[:], wo_sb[:, fc * H + hc * 128: fc * H + hc * 128 + 128],
                        attn_s[:, fc * S + tk * 512: fc * S + tk * 512 + 512],
                        start=(fc == 0), stop=(fc == 1))
                ar_sb = p_ar.tile([128, 512], F16, tag="ar", name="arsb")
                nc.scalar.activation(ar_sb[:], po[:], AF.Copy)
                nc.sync.dma_start(ar_in[ts(hc, 128), :], ar_sb[:])
            ar_mid = dram.tile([H // NC, 512], F16, tag="armid", name="armid")
            nc.gpsimd.collective_compute(
                "ReduceScatter", ALU.add, replica_groups=[list(range(NC))],
                ins=[ar_in[:].opt()], outs=[ar_mid[:].opt()])
            nc.gpsimd.collective_compute(
                "AllGather", ALU.bypass, replica_groups=[list(range(NC))],
                ins=[ar_mid[:].opt()], outs=[ar_out[:].opt()])
            return ar_out

        def resid_add(ar_out, tk):
            for hc in range(KH):
                ar_t = p_ar.tile([128, 512], F16, tag="ar", name="art")
                nc.sync.dma_start(ar_t[:], ar_out[ts(hc, 128), :])
                sl = slice(hc * S + tk * 512, hc * S + tk * 512 + 512)
                nc.vector.tensor_add(xT[:, sl], xT[:, sl], ar_t[:])

        def ffn_half(l_, tk):
            """norm2 + SwiGLU FFN + down proj for half tk; launches AR2."""
            inv = norm_inv(n2w, l_, tk)
            gu_sb = p_gu.tile([128, 12 * 512], F16, tag="gu", name="gusb")
            for pi, pset in enumerate((range(0, 6), range(6, 12))):
                gus = {c: psum.tile([128, 512], F32, tag="ps", name=f"gu{c}")
                       for c in pset}
                for hc in range(KH):
                    hn = p_ns.tile([128, 512], F16, tag="ns", name="hn")
                    nc.vector.scalar_tensor_tensor(
                        hn[:],
                        xT[:, hc * S + tk * 512: hc * S + tk * 512 + 512],
                        n2w[:, l_ * KH + hc: l_ * KH + hc + 1],
                        inv[:], op0=ALU.mult, op1=ALU.mult)
                    w13_t = p_w13.tile([128, 768], F16, tag="w13", name="w13t")
                    nc.sync.dma_start(
                        w13_t[:], w13_h.ap()[l_, ts(hc, 128),
                                             pi * 768: pi * 768 + 768])
                    st, sp = (hc == 0), (hc == KH - 1)
                    for ci, c in enumerate(pset):
                        nc.tensor.matmul(gus[c][:], w13_t[:, ts(ci, 128)],
                                         hn[:], start=st, stop=sp)
                for c in pset:
                    nc.scalar.activation(gu_sb[:, ts(c, 512)], gus[c][:],
                                         AF.Copy)
            # swig[s] = silu(g[s]) * u[s]  (in place over g chunks 0..5)
            for sch in range(6):
                sg = p_ns.tile([128, 512], F16, tag="ns", name="sg")
                nc.scalar.activation(sg[:], gu_sb[:, ts(sch, 512)], AF.Silu)
                nc.vector.tensor_mul(gu_sb[:, ts(sch, 512)], sg[:],
                                     gu_sb[:, ts(6 + sch, 512)])
            # down projection
            ar2_in = dram.tile([H, 512], F16, tag="arin", name="ar2in")
            ar2_out = dram.tile([H, 512], F16, tag="arout",
                                addr_space="Shared", name="ar2out")
            for hcb in range(4):
                p2 = [psum.tile([128, 512], F32, tag="ps", name=f"p2{i}")
                      for i in range(4)]
                for kc in range(6):
                    w2_t = p_w2.tile([128, 512], F16, tag="w2", name="w2t")
                    nc.sync.dma_start(
                        w2_t[:], w2T_h.ap()[l_, ts(kc, 128),
                                            hcb * 512: hcb * 512 + 512])
                    for hh in range(4):
                        nc.tensor.matmul(p2[hh][:], w2_t[:, ts(hh, 128)],
                                         gu_sb[:, ts(kc, 512)],
                                         start=(kc == 0), stop=(kc == 5))
                for hh in range(4):
                    a2 = p_ar.tile([128, 512], F16, tag="ar", name="a2")
                    nc.scalar.activation(a2[:], p2[hh][:], AF.Copy)
                    nc.sync.dma_start(ar2_in[ts(hcb * 4 + hh, 128), :], a2[:])
            ar_mid = dram.tile([H // NC, 512], F16, tag="armid", name="ar2mid")
            nc.gpsimd.collective_compute(
                "ReduceScatter", ALU.add, replica_groups=[list(range(NC))],
                ins=[ar2_in[:].opt()], outs=[ar_mid[:].opt()])
            nc.gpsimd.collective_compute(
                "AllGather", ALU.bypass, replica_groups=[list(range(NC))],
                ins=[ar_mid[:].opt()], outs=[ar2_out[:].opt()])
            return ar2_out

        # ---- layer 0 prologue ----
        wo_sb = load_wo(0)
        wq_sb = load_wqkv(0)
        cur_q = p_big.tile([128, 2 * S], F16, tag="big", name="q0")
        cur_k = p_big.tile([128, 2 * S], F16, tag="big", name="k0")
        cur_vT = p_big.tile([128, 2 * S], F16, tag="big", name="vT0")
        for tk in range(2):
            qkv_half(wq_sb, 0, tk, cur_q, cur_k, cur_vT)

        for l in range(L):
            last = (l == L - 1)
            q_s, k_s, vT_s = cur_q, cur_k, cur_vT

            v_s = p_vs.tile([128, 8 * FEAT], F16, tag="v", name="vs")
            attn_s = p_attn.tile([128, 2 * S], F16, tag="attn", name="attn")

            if last:
                # RoPE on k (all tokens) and q (last 2 only)
                for mt in range(2):
                    for n in range(2):
                        rope_slice(k_s, mt * S + n * 512, 512, n * 512)
                    rope_slice(q_s, mt * S + S - 2, 2, S - 2)
                # V -> [tok, feat] via PE transpose
                for mt in range(2):
                    for tb in range(8):
                        tp = psum.tile([128, 128], F16, tag="ps", name="tp")
                        nc.tensor.transpose(
                            tp[:],
                            vT_s[:, mt * S + tb * 128: mt * S + tb * 128 + 128],
                            id_r[:])
                        nc.vector.tensor_copy(
                            v_s[:, tb * FEAT + mt * 128:
                                tb * FEAT + mt * 128 + 128], tp[:])
                # attention for the last 2 tokens only
                for h in range(2):
                    at1 = psum.tile([128, 2], F32, tag="ps", name="at1")
                    ib1 = psum.tile([128, 2], F32, tag="ps", name="ib1")
                    for kc in range(8):
                        sc1 = psum.tile([128, 2], F32, tag="ps", name="sc1")
                        nc.tensor.matmul(
                            sc1[:],
                            k_s[:, h * S + kc * 128: h * S + kc * 128 + 128],
                            q_s[:, h * S + S - 2: h * S + S],
                            start=True, stop=True)
                        pt1 = p_pt.tile([128, 2], F16, tag="pt1", name="pt1")
                        nc.scalar.activation(pt1[:], sc1[:], AF.Exp,
                                             scale=INV_SCALE)
                        if kc == 7:
                            nc.vector.tensor_mul(
                                pt1[:], pt1[:],
                                dmask[:, 3 * 512 + 510: 3 * 512 + 512])
                        st, sp = (kc == 0), (kc == 7)
                        nc.tensor.matmul(
                            at1[:],
                            v_s[:, kc * FEAT + h * 128: kc * FEAT + h * 128 + 128],
                            pt1[:], start=st, stop=sp)
                        nc.tensor.matmul(ib1[:], ones_mat[:], pt1[:],
                                         start=st, stop=sp)
                    inva = p_pt.tile([128, 2], F32, tag="pta", name="inva")
                    nc.vector.reciprocal_approx_fast(inva[:], ib1[:])
                    nc.vector.tensor_mul(
                        attn_s[:, h * S + S - 2: h * S + S], at1[:], inva[:])

                # wo -> [H,2] AllReduce -> residual add (last 2 tokens)
                ar_in = dram.tile([H, 2], F16, tag="arinL", name="arinL")
                ar_out = dram.tile([H, 2], F16, tag="aroutL",
                                   addr_space="Shared", name="aroutL")
                for hc in range(KH):
                    poL = psum.tile([128, 2], F32, tag="ps", name="poL")
                    for fc in range(2):
                        nc.tensor.matmul(
                            poL[:],
                            wo_sb[:, fc * H + hc * 128: fc * H + hc * 128 + 128],
                            attn_s[:, fc * S + S - 2: fc * S + S],
                            start=(fc == 0), stop=(fc == 1))
                    arL = p_pt.tile([128, 2], F16, tag="arL", name="arL")
                    nc.scalar.activation(arL[:], poL[:], AF.Copy)
                    nc.sync.dma_start(ar_in[ts(hc, 128), :], arL[:])
                nc.gpsimd.collective_compute(
                    "AllReduce", ALU.add, replica_groups=[list(range(NC))],
                    ins=[ar_in[:].opt()], outs=[ar_out[:].opt()])
                for hc in range(KH):
                    ar_t = p_pt.tile([128, 2], F16, tag="arL", name="art")
                    nc.sync.dma_start(ar_t[:], ar_out[ts(hc, 128), :])
                    nc.vector.tensor_add(
                        xT[:, hc * S + S - 2: hc * S + S],
                        xT[:, hc * S + S - 2: hc * S + S], ar_t[:])

                # norm2 + FFN on the last 2 tokens
                sqL = p_row.tile([128, 2 * KH], F16, tag="sql2")
                for hc in range(KH):
                    col = hc * S + S - 2
                    nc.vector.tensor_mul(sqL[:, 2 * hc:2 * hc + 2],
                                         xT[:, col:col + 2], xT[:, col:col + 2])
                ssL = psum.tile([128, 2 * KH], F32, tag="ps", name="ssL")
                nc.tensor.matmul(ssL[:], ones_mat[:], sqL[:],
                                 start=True, stop=True)
                ssr = p_row.tile([128, 2], F32, tag="ssr")
                nc.vector.reduce_sum(
                    ssr[:], ssL[:].rearrange("p (c two) -> p two c", two=2),
                    axis=mybir.AxisListType.X)
                rmsL = p_row.tile([128, 2], F32, tag="rmsL")
                nc.scalar.activation(rmsL[:], ssr[:], AF.Sqrt,
                                     bias=eps_p[:], scale=1.0 / H)
                invL = p_row.tile([128, 2], F32, tag="invLc")
                nc.vector.reciprocal_approx_fast(invL[:], rmsL[:])
                hnL = p_row.tile([128, 2 * KH], F16, tag="hnL")
                tnL = p_row.tile([128, 2], F32, tag="tnL")
                for hc in range(KH):
                    col = hc * S + S - 2
                    nc.vector.tensor_scalar_mul(
                        tnL[:], xT[:, col:col + 2],
                        n2w[:, l * KH + hc: l * KH + hc + 1])
                    nc.vector.tensor_mul(hnL[:, 2 * hc:2 * hc + 2],
                                         tnL[:], invL[:])
                guL = p_row.tile([128, 12 * 2], F16, tag="guL")
                for pi, pset in enumerate((range(0, 6), range(6, 12))):
                    gps = {c: psum.tile([128, 2], F32, tag="ps",
                                        name=f"gL{c}") for c in pset}
                    for hc in range(KH):
                        w13_t = p_w13.tile([128, 768], F16, tag="w13",
                                           name="w13tL")
                        nc.sync.dma_start(
                            w13_t[:], w13_h.ap()[l, ts(hc, 128),
                                                 pi * 768: pi * 768 + 768])
                        st, sp = (hc == 0), (hc == KH - 1)
                        for ci, c in enumerate(pset):
                            nc.tensor.matmul(
                                gps[c][:], w13_t[:, ts(ci, 128)],
                                hnL[:, 2 * hc:2 * hc + 2], start=st, stop=sp)
                    for c in pset:
                        nc.scalar.activation(guL[:, 2 * c:2 * c + 2],
                                             gps[c][:], AF.Copy)
                swL = p_row.tile([128, 6 * 2], F16, tag="swL")
                for sch in range(6):
                    sgL = p_row.tile([128, 2], F16, tag="sgL")
                    nc.scalar.activation(sgL[:], guL[:, 2 * sch:2 * sch + 2],
                                         AF.Silu)
                    nc.vector.tensor_mul(swL[:, 2 * sch:2 * sch + 2], sgL[:],
                                         guL[:, 2 * (6 + sch):2 * (6 + sch) + 2])
                ar2_in = dram.tile([H, 2], F16, tag="arinL", name="ar2inL")
                ar2_out = dram.tile([H, 2], F16, tag="aroutL",
                                    addr_space="Shared", name="ar2outL")
                for hc in range(KH):
                    p2L = psum.tile([128, 2], F32, tag="ps", name="p2L")
                    for kc in range(6):
                        w2_t = p_w2.tile([128, 128], F16, tag="w2L",
                                         name="w2tL")
                        nc.sync.dma_start(
                            w2_t[:], w2T_h.ap()[l, ts(kc, 128), ts(hc, 128)])
                        nc.tensor.matmul(p2L[:], w2_t[:],
                                         swL[:, 2 * kc:2 * kc + 2],
                                         start=(kc == 0), stop=(kc == 5))
                    a2L = p_pt.tile([128, 2], F16, tag="arL", name="a2L")
                    nc.scalar.activation(a2L[:], p2L[:], AF.Copy)
                    nc.sync.dma_start(ar2_in[ts(hc, 128), :], a2L[:])
                nc.gpsimd.collective_compute(
                    "AllReduce", ALU.add, replica_groups=[list(range(NC))],
                    ins=[ar2_in[:].opt()], outs=[ar2_out[:].opt()])
                for hc in range(KH):
                    ar_t = p_pt.tile([128, 2], F16, tag="arL", name="art2")
                    nc.sync.dma_start(ar_t[:], ar2_out[ts(hc, 128), :])
                    nc.vector.tensor_add(
                        xT[:, hc * S + S - 2: hc * S + S],
                        xT[:, hc * S + S - 2: hc * S + S], ar_t[:])
                continue

            # ---- non-last layer ----
            ar1 = [None, None]
            for tk in range(2):
                # RoPE + V-transpose for this half only
                for t_s in (q_s, k_s):
                    for mt in range(2):
                        rope_slice(t_s, mt * S + tk * 512, 512, tk * 512)
                for mt in range(2):
                    for tb in range(tk * 4, tk * 4 + 4):
                        tp = psum.tile([128, 128], F16, tag="ps", name="tp")
                        nc.tensor.transpose(
                            tp[:],
                            vT_s[:, mt * S + tb * 128: mt * S + tb * 128 + 128],
                            id_r[:])
                        nc.vector.tensor_copy(
                            v_s[:, tb * FEAT + mt * 128:
                                tb * FEAT + mt * 128 + 128], tp[:])
                # attention: only causally visible key blocks
                nvis = (tk + 1) * 4
                for h in range(2):
                    at_ps = psum.tile([128, 512], F32, tag="ps", name="atp")
                    ib_ps = psum.tile([128, 512], F32, tag="ps", name="ibp")
                    for kc in range(nvis):
                        sc_ps = psum.tile([128, 512], F32, tag="ps",
                                          name="scp")
                        nc.tensor.matmul(
                            sc_ps[:],
                            k_s[:, h * S + kc * 128: h * S + kc * 128 + 128],
                            q_s[:, h * S + tk * 512: h * S + tk * 512 + 512],
                            start=True, stop=True)
                        pt = p_pt.tile([128, 512], F16, tag="pt", name="ptl")
                        nc.scalar.activation(pt[:], sc_ps[:], AF.Exp,
                                             scale=INV_SCALE)
                        d = kc * 128 - tk * 512
                        if d >= 0:
                            nc.vector.tensor_mul(
                                pt[:], pt[:], dmask[:, ts(d // 128, 512)])
                        st, sp = (kc == 0), (kc == nvis - 1)
                        nc.tensor.matmul(
                            at_ps[:],
                            v_s[:, kc * FEAT + h * 128: kc * FEAT + h * 128 + 128],
                            pt[:], start=st, stop=sp)
                        nc.tensor.matmul(ib_ps[:], ones_mat[:], pt[:],
                                         start=st, stop=sp)
                    inv_a = p_pt.tile([128, 512], F32, tag="pta", name="inva")
                    nc.vector.reciprocal_approx_fast(inv_a[:], ib_ps[:])
                    nc.vector.tensor_mul(
                        attn_s[:, h * S + tk * 512: h * S + tk * 512 + 512],
                        at_ps[:], inv_a[:])
                ar1[tk] = wo_project(wo_sb, attn_s, tk)

            # prefetch next layer's qkv weights (slot free: qkv(l) done)
            wq_sb = load_wqkv(l + 1)

            ar2 = [None, None]
            for tk in range(2):
                resid_add(ar1[tk], tk)
                ar2[tk] = ffn_half(l, tk)

            # next layer's wo
            wo_sb = load_wo(l + 1)

            nxt_q = p_big.tile([128, 2 * S], F16, tag="big", name="qn")
            nxt_k = p_big.tile([128, 2 * S], F16, tag="big", name="kn")
            nxt_vT = p_big.tile([128, 2 * S], F16, tag="big", name="vTn")
            for tk in range(2):
                resid_add(ar2[tk], tk)
                qkv_half(wq_sb, l + 1, tk, nxt_q, nxt_k, nxt_vT)
            cur_q, cur_k, cur_vT = nxt_q, nxt_k, nxt_vT

        # ======== final norm (last token only) + logits ========
        sq_l = p_row.tile([128, KH], F16, tag="sql")
        for hc in range(KH):
            col = hc * S + S - 1
            nc.vector.tensor_mul(sq_l[:, hc:hc + 1], xT[:, col:col + 1],
                                 xT[:, col:col + 1])
        sl_ps = psum.tile([1, KH], F32, tag="ps", name="slps")
        nc.tensor.matmul(sl_ps[:], ones_col[:], sq_l[:], start=True, stop=True)
        ssc = p_row.tile([1, 1], F32, tag="ssc")
        nc.vector.reduce_sum(ssc[:], sl_ps[:], axis=mybir.AxisListType.X)
        rms_l = p_row.tile([1, 1], F32, tag="rmsl")
        nc.scalar.activation(rms_l[:], ssc[:], AF.Sqrt, bias=eps_t[:],
                             scale=1.0 / H)
        inv_l = p_row.tile([1, 1], F32, tag="invl")
        nc.vector.reciprocal(inv_l[:], rms_l[:])
        xnl = p_row.tile([128, KH], F16, tag="xnl")
        for hc in range(KH):
            col = hc * S + S - 1
            nc.vector.tensor_mul(xnl[:, hc:hc + 1], xT[:, col:col + 1],
                                 fw_s[:, hc:hc + 1])
        for n in range(8):
            lg_ps = psum.tile([1, 500], F32, tag="ps", name="lgps")
            for hc in range(KH):
                ow_t = p_ow.tile([128, 500], F16, tag="ow", name="owt")
                nc.sync.dma_start(
                    ow_t[:], owT_h.ap()[ts(hc, 128), n * 500: n * 500 + 500])
                nc.tensor.matmul(lg_ps[:], xnl[:, hc: hc + 1], ow_t[:],
                                 start=(hc == 0), stop=(hc == KH - 1))
            lg = p_row.tile([1, 500], F32, tag="lg")
            nc.scalar.activation(lg[:], lg_ps[:], AF.Copy, scale=inv_l[:])
            nc.sync.dma_start(out_h.ap()[:, n * 500: n * 500 + 500], lg[:])

    nc.compile()
    return nc


def _shard(inputs):
    f16 = np.float16
    x = np.asarray(inputs["x"], np.float32)
    cos = np.asarray(inputs["cos"], np.float32).reshape(S, HD // 2)
    sin = np.asarray(inputs["sin"], np.float32).reshape(S, HD // 2)
    n1 = np.asarray(inputs["norm1_w"], np.float32)[:L]
    n2 = np.asarray(inputs["norm2_w"], np.float32)[:L]
    fw = np.asarray(inputs["final_norm_w"], np.float32)
    wq = np.asarray(inputs["wq"], np.float32)[:L]
    wk = np.asarray(inputs["wk"], np.float32)[:L]
    wv = np.asarray(inputs["wv"], np.float32)[:L]
    wo = np.asarray(inputs["wo"], np.float32)[:L]
    w1 = np.asarray(inputs["w1"], np.float32)[:L]
    w3 = np.asarray(inputs["w3"], np.float32)[:L]
    w2 = np.asarray(inputs["w2"], np.float32)[:L]
    ow = np.asarray(inputs["out_w"], np.float32)

    xT = np.ascontiguousarray(x[0].T).astype(f16)
    C = np.empty((128, S), np.float32)
    C[0::2] = cos.T
    C[1::2] = cos.T
    Sm = np.empty((128, S), np.float32)
    Sm[0::2] = -sin.T
    Sm[1::2] = sin.T
    J = np.zeros((128, 128), np.float32)
    idx = np.arange(0, 128, 2)
    J[idx, idx + 1] = 1.0
    J[idx + 1, idx] = 1.0
    ident = np.eye(128, dtype=np.float32)
    # diagonal causal masks: pattern di (block offset di*128):
    # mask[kp, q] = 1 if q >= kp + di*128
    dm = np.zeros((128, 4 * 512), np.float32)
    kp = np.arange(128)[:, None]
    qq = np.arange(512)[None, :]
    for di in range(4):
        dm[:, di * 512:(di + 1) * 512] = (qq >= kp + di * 128)
    n1w = np.ascontiguousarray(
        n1.reshape(L, KH, 128).transpose(2, 0, 1).reshape(128, L * KH))
    n2w = np.ascontiguousarray(
        n2.reshape(L, KH, 128).transpose(2, 0, 1).reshape(128, L * KH))
    fwh = np.ascontiguousarray(fw.reshape(KH, 128).T)

    common = dict(xT=xT, Cr=C.astype(f16), Sr=Sm.astype(f16),
                  J=J.astype(f16), ident=ident.astype(f16),
                  dmask=dm.astype(f16), n1w=n1w, n2w=n2w, fw=fwh)
    in_maps = []
    for c in range(NC):
        fs = slice(c * FEAT, (c + 1) * FEAT)
        ps = slice(c * PC, (c + 1) * PC)
        vs = slice(c * VC, (c + 1) * VC)
        m = dict(common)
        wqT = wq[:, fs, :].transpose(0, 2, 1)
        wkT = wk[:, fs, :].transpose(0, 2, 1)
        wvT = wv[:, fs, :].transpose(0, 2, 1)
        m["wqkvT"] = np.ascontiguousarray(
            np.concatenate([wqT, wkT, wvT], axis=2)).astype(f16)
        m["woT"] = np.ascontiguousarray(
            wo[:, :, fs].transpose(0, 2, 1)).astype(f16)
        w1T = w1[:, ps, :].transpose(0, 2, 1)   # [L, H, PC]
        w3T = w3[:, ps, :].transpose(0, 2, 1)
        pad = np.zeros((L, H, PCP - PC), np.float32)
        m["w13T"] = np.ascontiguousarray(np.concatenate(
            [w1T, pad, w3T, pad], axis=2)).astype(f16)
        w2p = np.zeros((L, PCP, H), np.float32)
        w2p[:, :PC, :] = w2[:, :, ps].transpose(0, 2, 1)
        m["w2T"] = np.ascontiguousarray(w2p).astype(f16)
        m["owT"] = np.ascontiguousarray(ow[vs, :].T).astype(f16)
        in_maps.append(m)
    return in_maps


def kernel(**inputs) -> np.ndarray:
    from concourse import bass_utils

    if "nc" not in _STATE:
        _STATE["nc"] = _build()
    in_maps = _shard(inputs)
    res = bass_utils.run_bass_kernel_spmd(
        _STATE["nc"], in_maps, core_ids=list(range(NC)))
    out = np.concatenate(
        [res.results[c]["logits"] for c in range(NC)], axis=1)
    return out.astype(np.float32)


# revision 5
# speedup vs baseline: 1.6447x; 1.0118x over previous
"""Trainium2 Bass kernel: 4-layer decoder prefill (S=1024, H=2048, NH=16, HD=128,
FFN=5632, V=32000), tensor-parallel over 8 NeuronCores.

- Megatron TP over 8 cores: wq/wk/wv/w1/w3 sharded on output dim (2 heads /
  704 ffn rows per core), wo/w2 sharded on input dim (partials ->
  ReduceScatter+AllGather), out_w sharded over vocab (4000 rows/core); only
  the last token's logits are computed.
- All matmuls in fp16 (weights pre-cast on host, fp32 accumulation in PSUM);
  the residual stream lives TRANSPOSED in SBUF as fp16 (xT: [H on
  partition-chunks, S free]).
- Causal structure exploited: fully-masked score blocks are skipped; diagonal
  blocks use 4 precomputed multiplicative 0/1 mask tiles; 1/sqrt(HD) is
  folded into the Exp activation scale.
- Softmax denominators and rms-norm sums are accumulated as PE matmuls with
  an all-ones [128,128] stationary, which broadcasts the partition-sum to all
  128 partitions directly -- no slow [1,N] single-partition ops; inverses via
  reciprocal_approx_fast (single DVE op).
- wqkv/wo are SBUF-resident per layer; w13/w2 streamed; ffn w1|w3 are
  zero-padded to 768 rows each so all chunks are full 128 partitions.
- Last layer: k/v for all tokens but q/attention/FFN only for the last
  tokens; logits fp16 GEMV streamed over the vocab shard.
"""

import os
import sys

sys.path.insert(0, "/opt/trn_rl_repo")

import numpy as np

L = 4
B, S, H, NH, HD = 1, 1024, 2048, 16, 128
V, P = 32000, 5632
NC = 8
FEAT = H // NC          # 256 q/k/v features per core (2 heads)
PC = P // NC            # 704 ffn rows per core
PCP = 768               # padded to 6 full 128-chunks
VC = V // NC            # 4000 vocab rows per core
KH = H // 128           # 16 H-chunks
EPS = 1e-5
SCALE = float(np.sqrt(HD))
INV_SCALE = 1.0 / SCALE

_STATE = {}


def _build():
    import concourse.bass as bass
    import concourse.bacc as bacc
    from concourse import tile, mybir

    F32 = mybir.dt.float32
    F16 = mybir.dt.float16
    AF = mybir.ActivationFunctionType
    ALU = mybir.AluOpType
    ts = bass.ts

    nc = bacc.Bacc("TRN2", target_bir_lowering=False, debug=False, num_devices=NC)

    xT_h = nc.dram_tensor("xT", [H, S], F16, kind="ExternalInput")
    C_h = nc.dram_tensor("Cr", [128, S], F16, kind="ExternalInput")
    S_h = nc.dram_tensor("Sr", [128, S], F16, kind="ExternalInput")
    J_h = nc.dram_tensor("J", [128, 128], F16, kind="ExternalInput")
    id_h = nc.dram_tensor("ident", [128, 128], F16, kind="ExternalInput")
    dm_h = nc.dram_tensor("dmask", [128, 4 * 512], F16, kind="ExternalInput")
    n1w_h = nc.dram_tensor("n1w", [128, L * KH], F32, kind="ExternalInput")
    n2w_h = nc.dram_tensor("n2w", [128, L * KH], F32, kind="ExternalInput")
    fw_h = nc.dram_tensor("fw", [128, KH], F32, kind="ExternalInput")
    # wq|wk|wv concatenated on the last axis: [L, H, 3*FEAT]
    wqkv_h = nc.dram_tensor("wqkvT", [L, H, 3 * FEAT], F16, kind="ExternalInput")
    woT_h = nc.dram_tensor("woT", [L, FEAT, H], F16, kind="ExternalInput")
    # [w1 | 64pad | w3 | 64pad] on cols: [L, H, 2*PCP]
    w13_h = nc.dram_tensor("w13T", [L, H, 2 * PCP], F16, kind="ExternalInput")
    w2T_h = nc.dram_tensor("w2T", [L, PCP, H], F16, kind="ExternalInput")
    owT_h = nc.dram_tensor("owT", [H, VC], F16, kind="ExternalInput")
    out_h = nc.dram_tensor("logits", [1, VC], F32, kind="ExternalOutput")

    from contextlib import ExitStack

    with tile.TileContext(nc) as tc, ExitStack() as _ctx:
        ec = _ctx.enter_context
        p_resid = ec(tc.tile_pool(name="resid", bufs=1))
        p_const = ec(tc.tile_pool(name="consts", bufs=1))
        p_big = ec(tc.tile_pool(name="big", bufs=3))
        p_vs = ec(tc.tile_pool(name="vsn", bufs=2))
        p_attn = ec(tc.tile_pool(name="attnp", bufs=2))
        p_pt = ec(tc.tile_pool(name="ptile", bufs=3))
        p_ns = ec(tc.tile_pool(name="normsc", bufs=3))
        p_nrm = ec(tc.tile_pool(name="nrm", bufs=2))
        p_gu = ec(tc.tile_pool(name="gup", bufs=2))
        p_wres = ec(tc.tile_pool(name="wres", bufs=1))
        p_w13 = ec(tc.tile_pool(name="w13p", bufs=3))
        p_w2 = ec(tc.tile_pool(name="w2p", bufs=4))
        p_ow = ec(tc.tile_pool(name="owp", bufs=6))
        p_ar = ec(tc.tile_pool(name="ars", bufs=6))
        p_row = ec(tc.tile_pool(name="row", bufs=2))
        psum = ec(tc.tile_pool(name="psum", bufs=7, space="PSUM"))
        dram = ec(tc.tile_pool(name="dram", bufs=4, space="DRAM"))

        # ---- constants / inputs ----
        xT = p_resid.tile([128, KH * S], F16, tag="xT")
        for hc in range(KH):
            nc.sync.dma_start(xT[:, ts(hc, S)], xT_h.ap()[ts(hc, 128), :])

        C_s = p_const.tile([128, S], F16, tag="C")
        nc.sync.dma_start(C_s[:], C_h.ap())
        S_s = p_const.tile([128, S], F16, tag="S")
        nc.sync.dma_start(S_s[:], S_h.ap())
        J_r = p_const.tile([128, 128], F16, tag="J")
        nc.sync.dma_start(J_r[:], J_h.ap())
        id_r = p_const.tile([128, 128], F16, tag="id")
        nc.sync.dma_start(id_r[:], id_h.ap())
        dmask = p_const.tile([128, 4 * 512], F16, tag="dm")
        nc.sync.dma_start(dmask[:], dm_h.ap())
        n1w = p_const.tile([128, L * KH], F32, tag="n1w")
        nc.sync.dma_start(n1w[:], n1w_h.ap())
        n2w = p_const.tile([128, L * KH], F32, tag="n2w")
        nc.sync.dma_start(n2w[:], n2w_h.ap())
        fw_s = p_const.tile([128, KH], F32, tag="fw")
        nc.sync.dma_start(fw_s[:], fw_h.ap())
        ones_mat = p_const.tile([128, 128], F16, tag="om")
        nc.vector.memset(ones_mat[:], 1.0)
        ones_col = p_const.tile([128, 1], F16, tag="o1")
        nc.vector.memset(ones_col[:], 1.0)
        eps_p = p_const.tile([128, 1], F32, tag="epsp")
        nc.vector.memset(eps_p[:], EPS)
        eps_t = p_const.tile([1, 1], F32, tag="eps")
        nc.vector.memset(eps_t[:], EPS)

        def load_wo(l_):
            wo_sb = p_wres.tile([128, 2 * H], F16, tag="wo", name="wosb")
            for fc in range(2):
                nc.sync.dma_start(wo_sb[:, ts(fc, H)],
                                  woT_h.ap()[l_, ts(fc, 128), :])
            return wo_sb

        def load_wqkv(l_):
            wq_sb = p_wres.tile([128, KH * 3 * FEAT], F16, tag="wqkv",
                                name="wqsb")
            for hc in range(KH):
                nc.sync.dma_start(wq_sb[:, ts(hc, 3 * FEAT)],
                                  wqkv_h.ap()[l_, ts(hc, 128), :])
            return wq_sb

        def norm_inv(w_tile, l_, tk):
            """[128,512] fp32 tile of 1/rms for tokens [tk*512, tk*512+512)."""
            nb_ps = psum.tile([128, 512], F32, tag="ps", name="nbps")
            for hc in range(KH):
                sq = p_ns.tile([128, 512], F16, tag="sq", name="sq")
                sl = slice(hc * S + tk * 512, hc * S + tk * 512 + 512)
                nc.vector.tensor_mul(sq[:], xT[:, sl], xT[:, sl])
                nc.tensor.matmul(nb_ps[:], ones_mat[:], sq[:],
                                 start=(hc == 0), stop=(hc == KH - 1))
            rms = p_nrm.tile([128, 512], F32, tag="rms", name="rms")
            nc.scalar.activation(rms[:], nb_ps[:], AF.Sqrt,
                                 bias=eps_p[:], scale=1.0 / H)
            inv = p_nrm.tile([128, 512], F32, tag="inv", name="inv")
            nc.vector.reciprocal_approx_fast(inv[:], rms[:])
            return inv

        def qkv_half(wq_sb, l_, tk, q_s, k_s, vT_s):
            """QKV for token half tk of layer l_ (writes [:, mt*S+tk*512]).

            For the last layer, q is computed only for the last 2 tokens
            (tk==1) into q_s[:, mt*S + S-2 : mt*S + S]."""
            last = (l_ == L - 1)
            inv = norm_inv(n1w, l_, tk)
            kp = [psum.tile([128, 512], F32, tag="ps", name=f"kp{i}")
                  for i in range(2)]
            vp = [psum.tile([128, 512], F32, tag="ps", name=f"vp{i}")
                  for i in range(2)]
            if not last:
                qp = [psum.tile([128, 512], F32, tag="ps", name=f"qp{i}")
                      for i in range(2)]
            elif tk == 1:
                qp = [psum.tile([128, 2], F32, tag="ps", name=f"qL{i}")
                      for i in range(2)]
            else:
                qp = None
            for hc in range(KH):
                xn = p_ns.tile([128, 512], F16, tag="ns", name="xn")
                nc.vector.scalar_tensor_tensor(
                    xn[:], xT[:, hc * S + tk * 512: hc * S + tk * 512 + 512],
                    n1w[:, l_ * KH + hc: l_ * KH + hc + 1],
                    inv[:], op0=ALU.mult, op1=ALU.mult)
                st, sp = (hc == 0), (hc == KH - 1)
                for mt in range(2):
                    if not last:
                        nc.tensor.matmul(
                            qp[mt][:], wq_sb[:, hc * 768 + mt * 128:
                                             hc * 768 + mt * 128 + 128],
                            xn[:], start=st, stop=sp)
                    elif tk == 1:
                        nc.tensor.matmul(
                            qp[mt][:], wq_sb[:, hc * 768 + mt * 128:
                                             hc * 768 + mt * 128 + 128],
                            xn[:, 510:512], start=st, stop=sp)
                    nc.tensor.matmul(
                        kp[mt][:], wq_sb[:, hc * 768 + 256 + mt * 128:
                                         hc * 768 + 256 + mt * 128 + 128],
                        xn[:], start=st, stop=sp)
                    nc.tensor.matmul(
                        vp[mt][:], wq_sb[:, hc * 768 + 512 + mt * 128:
                                         hc * 768 + 512 + mt * 128 + 128],
                        xn[:], start=st, stop=sp)
            for mt in range(2):
                off = mt * S + tk * 512
                if not last:
                    nc.vector.tensor_copy(q_s[:, off:off + 512], qp[mt][:])
                elif tk == 1:
                    nc.vector.tensor_copy(q_s[:, mt * S + S - 2: mt * S + S],
                                          qp[mt][:])
                nc.vector.tensor_copy(k_s[:, off:off + 512], kp[mt][:])
                nc.vector.tensor_copy(vT_s[:, off:off + 512], vp[mt][:])

        def rope_slice(t_s, col, width, ccol):
            """RoPE in place on t_s[:, col:col+width]; cos/sin cols at ccol."""
            j_ps = psum.tile([128, 512], F32, tag="ps", name="jps")
            nc.tensor.matmul(j_ps[:, :width], J_r[:], t_s[:, col:col + width],
                             start=True, stop=True)
            tmp = p_pt.tile([128, 512], F16, tag="rtmp", name="rtmp")
            nc.vector.tensor_mul(tmp[:, :width], C_s[:, ccol:ccol + width],
                                 t_s[:, col:col + width])
            nc.vector.tensor_mul(t_s[:, col:col + width], j_ps[:, :width],
                                 S_s[:, ccol:ccol + width])
            nc.vector.tensor_add(t_s[:, col:col + width],
                                 t_s[:, col:col + width], tmp[:, :width])

        def wo_project(wo_sb, attn_s, tk):
            """wo @ attn for half tk -> DRAM ar_in; RS+AG; returns ar_out."""
            ar_in = dram.tile([H, 512], F16, tag="arin", name="arin")
            ar_out = dram.tile([H, 512], F16, tag="arout",
                               addr_space="Shared", name="arout")
            for hc in range(KH):
                po = psum.tile([128, 512], F32, tag="ps", name="po")
                for fc in range(2):
                    nc.tensor.matmul(
                        po[:], wo_sb[:, fc * H + hc * 128: fc * H + hc * 128 + 128],
                        attn_s[:, fc * S + tk * 512: fc * S + tk * 512 + 512],
                        start=(fc == 0), stop=(fc == 1))
                ar_sb = p_ar.tile([128, 512], F16, tag="ar", name="arsb")
                nc.scalar.activation(ar_sb[:], po[:], AF.Copy)
                nc.sync.dma_start(ar_in[ts(hc, 128), :], ar_sb[:])
            ar_mid = dram.tile([H // NC, 512], F16, tag="armid", name="armid")
            nc.gpsimd.collective_compute(
                "ReduceScatter", ALU.add, replica_groups=[list(range(NC))],
                ins=[ar_in[:].opt()], outs=[ar_mid[:].opt()])
            nc.gpsimd.collective_compute(
                "AllGather", ALU.bypass, replica_groups=[list(range(NC))],
                ins=[ar_mid[:].opt()], outs=[ar_out[:].opt()])
            return ar_out

        def resid_add(ar_out, tk):
            for hc in range(KH):
                ar_t = p_ar.tile([128, 512], F16, tag="ar", name="art")
                nc.sync.dma_start(ar_t[:], ar_out[ts(hc, 128), :])
                sl = slice(hc * S + tk * 512, hc * S + tk * 512 + 512)
                nc.vector.tensor_add(xT[:, sl], xT[:, sl], ar_t[:])

        def ffn_half(l_, tk):
            """norm2 + SwiGLU FFN + down proj for half tk; launches AR2."""
            inv = norm_inv(n2w, l_, tk)
            gu_sb = p_gu.tile([128, 12 * 512], F16, tag="gu", name="gusb")
            for pi, pset in enumerate((range(0, 6), range(6, 12))):
                gus = {c: psum.tile([128, 512], F32, tag="ps", name=f"gu{c}")
                       for c in pset}
                for hc in range(KH):
                    hn = p_ns.tile([128, 512], F16, tag="ns", name="hn")
                    nc.vector.scalar_tensor_tensor(
                        hn[:],
                        xT[:, hc * S + tk * 512: hc * S + tk * 512 + 512],
                        n2w[:, l_ * KH + hc: l_ * KH + hc + 1],
                        inv[:], op0=ALU.mult, op1=ALU.mult)
                    w13_t = p_w13.tile([128, 768], F16, tag="w13", name="w13t")
                    nc.sync.dma_start(
                        w13_t[:], w13_h.ap()[l_, ts(hc, 128),
                                             pi * 768: pi * 768 + 768])
                    st, sp = (hc == 0), (hc == KH - 1)
                    for ci, c in enumerate(pset):
                        nc.tensor.matmul(gus[c][:], w13_t[:, ts(ci, 128)],
                                         hn[:], start=st, stop=sp)
                for c in pset:
                    nc.scalar.activation(gu_sb[:, ts(c, 512)], gus[c][:],
                                         AF.Copy)
            # swig[s] = silu(g[s]) * u[s]  (in place over g chunks 0..5)
            for sch in range(6):
                sg = p_ns.tile([128, 512], F16, tag="ns", name="sg")
                nc.scalar.activation(sg[:], gu_sb[:, ts(sch, 512)], AF.Silu)
                nc.vector.tensor_mul(gu_sb[:, ts(sch, 512)], sg[:],
                                     gu_sb[:, ts(6 + sch, 512)])
            # down projection
            ar2_in = dram.tile([H, 512], F16, tag="arin", name="ar2in")
            ar2_out = dram.tile([H, 512], F16, tag="arout",
                                addr_space="Shared", name="ar2out")
            for hcb in range(4):
                p2 = [psum.tile([128, 512], F32, tag="ps", name=f"p2{i}")
                      for i in range(4)]
                for kc in range(6):
                    w2_t = p_w2.tile([128, 512], F16, tag="w2", name="w2t")
                    nc.sync.dma_start(
                        w2_t[:], w2T_h.ap()[l_, ts(kc, 128),
                                            hcb * 512: hcb * 512 + 512])
                    for hh in range(4):
                        nc.tensor.matmul(p2[hh][:], w2_t[:, ts(hh, 128)],
                                         gu_sb[:, ts(kc, 512)],
                                         start=(kc == 0), stop=(kc == 5))
                for hh in range(4):
                    a2 = p_ar.tile([128, 512], F16, tag="ar", name="a2")
                    nc.scalar.activation(a2[:], p2[hh][:], AF.Copy)
                    nc.sync.dma_start(ar2_in[ts(hcb * 4 + hh, 128), :], a2[:])
            ar_mid = dram.tile([H // NC, 512], F16, tag="armid", name="ar2mid")
            nc.gpsimd.collective_compute(
                "ReduceScatter", ALU.add, replica_groups=[list(range(NC))],
                ins=[ar2_in[:].opt()], outs=[ar_mid[:].opt()])
            nc.gpsimd.collective_compute(
                "AllGather", ALU.bypass, replica_groups=[list(range(NC))],
                ins=[ar_mid[:].opt()], outs=[ar2_out[:].opt()])
            return ar2_out

        def rope_vtrans_half(q_s, k_s, vT_s, v_s, tk):
            for t_s in (q_s, k_s):
                for mt in range(2):
                    rope_slice(t_s, mt * S + tk * 512, 512, tk * 512)
            for mt in range(2):
                for tb in range(tk * 4, tk * 4 + 4):
                    tp = psum.tile([128, 128], F16, tag="ps", name="tp")
                    nc.tensor.transpose(
                        tp[:],
                        vT_s[:, mt * S + tb * 128: mt * S + tb * 128 + 128],
                        id_r[:])
                    nc.vector.tensor_copy(
                        v_s[:, tb * FEAT + mt * 128:
                            tb * FEAT + mt * 128 + 128], tp[:])

        def attn_half(q_s, k_s, v_s, attn_s, tk):
            """attention for half tk: only causally visible key blocks."""
            nvis = (tk + 1) * 4
            for h in range(2):
                at_ps = psum.tile([128, 512], F32, tag="ps", name="atp")
                ib_ps = psum.tile([128, 512], F32, tag="ps", name="ibp")
                for kc in range(nvis):
                    sc_ps = psum.tile([128, 512], F32, tag="ps", name="scp")
                    nc.tensor.matmul(
                        sc_ps[:],
                        k_s[:, h * S + kc * 128: h * S + kc * 128 + 128],
                        q_s[:, h * S + tk * 512: h * S + tk * 512 + 512],
                        start=True, stop=True)
                    pt = p_pt.tile([128, 512], F16, tag="pt", name="ptl")
                    nc.scalar.activation(pt[:], sc_ps[:], AF.Exp,
                                         scale=INV_SCALE)
                    d = kc * 128 - tk * 512
                    if d >= 0:
                        nc.vector.tensor_mul(
                            pt[:], pt[:], dmask[:, ts(d // 128, 512)])
                    st, sp = (kc == 0), (kc == nvis - 1)
                    nc.tensor.matmul(
                        at_ps[:],
                        v_s[:, kc * FEAT + h * 128: kc * FEAT + h * 128 + 128],
                        pt[:], start=st, stop=sp)
                    nc.tensor.matmul(ib_ps[:], ones_mat[:], pt[:],
                                     start=st, stop=sp)
                inv_a = p_pt.tile([128, 512], F32, tag="pta", name="inva")
                nc.vector.reciprocal_approx_fast(inv_a[:], ib_ps[:])
                nc.vector.tensor_mul(
                    attn_s[:, h * S + tk * 512: h * S + tk * 512 + 512],
                    at_ps[:], inv_a[:])

        # ---- layer 0 prologue: only the A-half of QKV(0); the B-half is
        # computed inside layer 0 under AR1(A)'s shadow ----
        wo_sb = load_wo(0)
        wq_sb = load_wqkv(0)
        cur_q = p_big.tile([128, 2 * S], F16, tag="big", name="q0")
        cur_k = p_big.tile([128, 2 * S], F16, tag="big", name="k0")
        cur_vT = p_big.tile([128, 2 * S], F16, tag="big", name="vT0")
        qkv_half(wq_sb, 0, 0, cur_q, cur_k, cur_vT)
        ar2_prev_b = None

        for l in range(L):
            last = (l == L - 1)
            q_s, k_s, vT_s = cur_q, cur_k, cur_vT

            v_s = p_vs.tile([128, 8 * FEAT], F16, tag="v", name="vs")
            attn_s = p_attn.tile([128, 2 * S], F16, tag="attn", name="attn")

            if last:
                # B-half QKV of the last layer (k/v all tokens, q last-2)
                if ar2_prev_b is not None:
                    resid_add(ar2_prev_b, 1)
                qkv_half(wq_sb, l, 1, q_s, k_s, vT_s)
                # RoPE on k (all tokens) and q (last 2 only)
                for mt in range(2):
                    for n in range(2):
                        rope_slice(k_s, mt * S + n * 512, 512, n * 512)
                    rope_slice(q_s, mt * S + S - 2, 2, S - 2)
                # V -> [tok, feat] via PE transpose
                for mt in range(2):
                    for tb in range(8):
                        tp = psum.tile([128, 128], F16, tag="ps", name="tp")
                        nc.tensor.transpose(
                            tp[:],
                            vT_s[:, mt * S + tb * 128: mt * S + tb * 128 + 128],
                            id_r[:])
                        nc.vector.tensor_copy(
                            v_s[:, tb * FEAT + mt * 128:
                                tb * FEAT + mt * 128 + 128], tp[:])
                # attention for the last 2 tokens only
                for h in range(2):
                    at1 = psum.tile([128, 2], F32, tag="ps", name="at1")
                    ib1 = psum.tile([128, 2], F32, tag="ps", name="ib1")
                    for kc in range(8):
                        sc1 = psum.tile([128, 2], F32, tag="ps", name="sc1")
                        nc.tensor.matmul(
                            sc1[:],
                            k_s[:, h * S + kc * 128: h * S + kc * 128 + 128],
                            q_s[:, h * S + S - 2: h * S + S],
                            start=True, stop=True)
                        pt1 = p_pt.tile([128, 2], F16, tag="pt1", name="pt1")
                        nc.scalar.activation(pt1[:], sc1[:], AF.Exp,
                                             scale=INV_SCALE)
                        if kc == 7:
                            nc.vector.tensor_mul(
                                pt1[:], pt1[:],
                                dmask[:, 3 * 512 + 510: 3 * 512 + 512])
                        st, sp = (kc == 0), (kc == 7)
                        nc.tensor.matmul(
                            at1[:],
                            v_s[:, kc * FEAT + h * 128: kc * FEAT + h * 128 + 128],
                            pt1[:], start=st, stop=sp)
                        nc.tensor.matmul(ib1[:], ones_mat[:], pt1[:],
                                         start=st, stop=sp)
                    inva = p_pt.tile([128, 2], F32, tag="pta", name="inva")
                    nc.vector.reciprocal_approx_fast(inva[:], ib1[:])
                    nc.vector.tensor_mul(
                        attn_s[:, h * S + S - 2: h * S + S], at1[:], inva[:])

                # wo -> [H,2] AllReduce -> residual add (last 2 tokens)
                ar_in = dram.tile([H, 2], F16, tag="arinL", name="arinL")
                ar_out = dram.tile([H, 2], F16, tag="aroutL",
                                   addr_space="Shared", name="aroutL")
                for hc in range(KH):
                    poL = psum.tile([128, 2], F32, tag="ps", name="poL")
                    for fc in range(2):
                        nc.tensor.matmul(
                            poL[:],
                            wo_sb[:, fc * H + hc * 128: fc * H + hc * 128 + 128],
                            attn_s[:, fc * S + S - 2: fc * S + S],
                            start=(fc == 0), stop=(fc == 1))
                    arL = p_pt.tile([128, 2], F16, tag="arL", name="arL")
                    nc.scalar.activation(arL[:], poL[:], AF.Copy)
                    nc.sync.dma_start(ar_in[ts(hc, 128), :], arL[:])
                nc.gpsimd.collective_compute(
                    "AllReduce", ALU.add, replica_groups=[list(range(NC))],
                    ins=[ar_in[:].opt()], outs=[ar_out[:].opt()])
                for hc in range(KH):
                    ar_t = p_pt.tile([128, 2], F16, tag="arL", name="art")
                    nc.sync.dma_start(ar_t[:], ar_out[ts(hc, 128), :])
                    nc.vector.tensor_add(
                        xT[:, hc * S + S - 2: hc * S + S],
                        xT[:, hc * S + S - 2: hc * S + S], ar_t[:])

                # norm2 + FFN on the last 2 tokens
                sqL = p_row.tile([128, 2 * KH], F16, tag="sql2")
                for hc in range(KH):
                    col = hc * S + S - 2
                    nc.vector.tensor_mul(sqL[:, 2 * hc:2 * hc + 2],
                                         xT[:, col:col + 2], xT[:, col:col + 2])
                ssL = psum.tile([128, 2 * KH], F32, tag="ps", name="ssL")
                nc.tensor.matmul(ssL[:], ones_mat[:], sqL[:],
                                 start=True, stop=True)
                ssr = p_row.tile([128, 2], F32, tag="ssr")
                nc.vector.reduce_sum(
                    ssr[:], ssL[:].rearrange("p (c two) -> p two c", two=2),
                    axis=mybir.AxisListType.X)
                rmsL = p_row.tile([128, 2], F32, tag="rmsL")
                nc.scalar.activation(rmsL[:], ssr[:], AF.Sqrt,
                                     bias=eps_p[:], scale=1.0 / H)
                invL = p_row.tile([128, 2], F32, tag="invLc")
                nc.vector.reciprocal_approx_fast(invL[:], rmsL[:])
                hnL = p_row.tile([128, 2 * KH], F16, tag="hnL")
                tnL = p_row.tile([128, 2], F32, tag="tnL")
                for hc in range(KH):
                    col = hc * S + S - 2
                    nc.vector.tensor_scalar_mul(
                        tnL[:], xT[:, col:col + 2],
                        n2w[:, l * KH + hc: l * KH + hc + 1])
                    nc.vector.tensor_mul(hnL[:, 2 * hc:2 * hc + 2],
                                         tnL[:], invL[:])
                guL = p_row.tile([128, 12 * 2], F16, tag="guL")
                for pi, pset in enumerate((range(0, 6), range(6, 12))):
                    gps = {c: psum.tile([128, 2], F32, tag="ps",
                                        name=f"gL{c}") for c in pset}
                    for hc in range(KH):
                        w13_t = p_w13.tile([128, 768], F16, tag="w13",
                                           name="w13tL")
                        nc.sync.dma_start(
                            w13_t[:], w13_h.ap()[l, ts(hc, 128),
                                                 pi * 768: pi * 768 + 768])
                        st, sp = (hc == 0), (hc == KH - 1)
                        for ci, c in enumerate(pset):
                            nc.tensor.matmul(
                                gps[c][:], w13_t[:, ts(ci, 128)],
                                hnL[:, 2 * hc:2 * hc + 2], start=st, stop=sp)
                    for c in pset:
                        nc.scalar.activation(guL[:, 2 * c:2 * c + 2],
                                             gps[c][:], AF.Copy)
                swL = p_row.tile([128, 6 * 2], F16, tag="swL")
                for sch in range(6):
                    sgL = p_row.tile([128, 2], F16, tag="sgL")
                    nc.scalar.activation(sgL[:], guL[:, 2 * sch:2 * sch + 2],
                                         AF.Silu)
                    nc.vector.tensor_mul(swL[:, 2 * sch:2 * sch + 2], sgL[:],
                                         guL[:, 2 * (6 + sch):2 * (6 + sch) + 2])
                ar2_in = dram.tile([H, 2], F16, tag="arinL", name="ar2inL")
                ar2_out = dram.tile([H, 2], F16, tag="aroutL",
                                    addr_space="Shared", name="ar2outL")
                for hc in range(KH):
                    p2L = psum.tile([128, 2], F32, tag="ps", name="p2L")
                    for kc in range(6):
                        w2_t = p_w2.tile([128, 128], F16, tag="w2L",
                                         name="w2tL")
                        nc.sync.dma_start(
                            w2_t[:], w2T_h.ap()[l, ts(kc, 128), ts(hc, 128)])
                        nc.tensor.matmul(p2L[:], w2_t[:],
                                         swL[:, 2 * kc:2 * kc + 2],
                                         start=(kc == 0), stop=(kc == 5))
                    a2L = p_pt.tile([128, 2], F16, tag="arL", name="a2L")
                    nc.scalar.activation(a2L[:], p2L[:], AF.Copy)
                    nc.sync.dma_start(ar2_in[ts(hc, 128), :], a2L[:])
                nc.gpsimd.collective_compute(
                    "AllReduce", ALU.add, replica_groups=[list(range(NC))],
                    ins=[ar2_in[:].opt()], outs=[ar2_out[:].opt()])
                for hc in range(KH):
                    ar_t = p_pt.tile([128, 2], F16, tag="arL", name="art2")
                    nc.sync.dma_start(ar_t[:], ar2_out[ts(hc, 128), :])
                    nc.vector.tensor_add(
                        xT[:, hc * S + S - 2: hc * S + S],
                        xT[:, hc * S + S - 2: hc * S + S], ar_t[:])
                continue

            # ---- non-last layer: A-half attention first, then the B-half
            # QKV runs in AR1(A)'s shadow ----
            rope_vtrans_half(q_s, k_s, vT_s, v_s, 0)
            attn_half(q_s, k_s, v_s, attn_s, 0)
            ar1_a = wo_project(wo_sb, attn_s, 0)

            if ar2_prev_b is not None:
                resid_add(ar2_prev_b, 1)
            qkv_half(wq_sb, l, 1, q_s, k_s, vT_s)

            rope_vtrans_half(q_s, k_s, vT_s, v_s, 1)
            attn_half(q_s, k_s, v_s, attn_s, 1)
            ar1_b = wo_project(wo_sb, attn_s, 1)

            # prefetch next layer's qkv weights (slot free: qkv(l) done)
            wq_sb = load_wqkv(l + 1)

            resid_add(ar1_a, 0)
            ar2_a = ffn_half(l, 0)
            resid_add(ar1_b, 1)
            ar2_prev_b = ffn_half(l, 1)

            # next layer's wo
            wo_sb = load_wo(l + 1)

            nxt_q = p_big.tile([128, 2 * S], F16, tag="big", name="qn")
            nxt_k = p_big.tile([128, 2 * S], F16, tag="big", name="kn")
            nxt_vT = p_big.tile([128, 2 * S], F16, tag="big", name="vTn")
            resid_add(ar2_a, 0)
            qkv_half(wq_sb, l + 1, 0, nxt_q, nxt_k, nxt_vT)
            cur_q, cur_k, cur_vT = nxt_q, nxt_k, nxt_vT

        # ======== final norm (last token only) + logits ========
        sq_l = p_row.tile([128, KH], F16, tag="sql")
        for hc in range(KH):
            col = hc * S + S - 1
            nc.vector.tensor_mul(sq_l[:, hc:hc + 1], xT[:, col:col + 1],
                                 xT[:, col:col + 1])
        sl_ps = psum.tile([1, KH], F32, tag="ps", name="slps")
        nc.tensor.matmul(sl_ps[:], ones_col[:], sq_l[:], start=True, stop=True)
        ssc = p_row.tile([1, 1], F32, tag="ssc")
        nc.vector.reduce_sum(ssc[:], sl_ps[:], axis=mybir.AxisListType.X)
        rms_l = p_row.tile([1, 1], F32, tag="rmsl")
        nc.scalar.activation(rms_l[:], ssc[:], AF.Sqrt, bias=eps_t[:],
                             scale=1.0 / H)
        inv_l = p_row.tile([1, 1], F32, tag="invl")
        nc.vector.reciprocal(inv_l[:], rms_l[:])
        xnl = p_row.tile([128, KH], F16, tag="xnl")
        for hc in range(KH):
            col = hc * S + S - 1
            nc.vector.tensor_mul(xnl[:, hc:hc + 1], xT[:, col:col + 1],
                                 fw_s[:, hc:hc + 1])
        for n in range(8):
            lg_ps = psum.tile([1, 500], F32, tag="ps", name="lgps")
            for hc in range(KH):
                ow_t = p_ow.tile([128, 500], F16, tag="ow", name="owt")
                nc.sync.dma_start(
                    ow_t[:], owT_h.ap()[ts(hc, 128), n * 500: n * 500 + 500])
                nc.tensor.matmul(lg_ps[:], xnl[:, hc: hc + 1], ow_t[:],
                                 start=(hc == 0), stop=(hc == KH - 1))
            lg = p_row.tile([1, 500], F32, tag="lg")
            nc.scalar.activation(lg[:], lg_ps[:], AF.Copy, scale=inv_l[:])
            nc.sync.dma_start(out_h.ap()[:, n * 500: n * 500 + 500], lg[:])

    nc.compile()
    return nc


def _shard(inputs):
    f16 = np.float16
    x = np.asarray(inputs["x"], np.float32)
    cos = np.asarray(inputs["cos"], np.float32).reshape(S, HD // 2)
    sin = np.asarray(inputs["sin"], np.float32).reshape(S, HD // 2)
    n1 = np.asarray(inputs["norm1_w"], np.float32)[:L]
    n2 = np.asarray(inputs["norm2_w"], np.float32)[:L]
    fw = np.asarray(inputs["final_norm_w"], np.float32)
    wq = np.asarray(inputs["wq"], np.float32)[:L]
    wk = np.asarray(inputs["wk"], np.float32)[:L]
    wv = np.asarray(inputs["wv"], np.float32)[:L]
    wo = np.asarray(inputs["wo"], np.float32)[:L]
    w1 = np.asarray(inputs["w1"], np.float32)[:L]
    w3 = np.asarray(inputs["w3"], np.float32)[:L]
    w2 = np.asarray(inputs["w2"], np.float32)[:L]
    ow = np.asarray(inputs["out_w"], np.float32)

    xT = np.ascontiguousarray(x[0].T).astype(f16)
    C = np.empty((128, S), np.float32)
    C[0::2] = cos.T
    C[1::2] = cos.T
    Sm = np.empty((128, S), np.float32)
    Sm[0::2] = -sin.T
    Sm[1::2] = sin.T
    J = np.zeros((128, 128), np.float32)
    idx = np.arange(0, 128, 2)
    J[idx, idx + 1] = 1.0
    J[idx + 1, idx] = 1.0
    ident = np.eye(128, dtype=np.float32)
    # diagonal causal masks: pattern di (block offset di*128):
    # mask[kp, q] = 1 if q >= kp + di*128
    dm = np.zeros((128, 4 * 512), np.float32)
    kp = np.arange(128)[:, None]
    qq = np.arange(512)[None, :]
    for di in range(4):
        dm[:, di * 512:(di + 1) * 512] = (qq >= kp + di * 128)
    n1w = np.ascontiguousarray(
        n1.reshape(L, KH, 128).transpose(2, 0, 1).reshape(128, L * KH))
    n2w = np.ascontiguousarray(
        n2.reshape(L, KH, 128).transpose(2, 0, 1).reshape(128, L * KH))
    fwh = np.ascontiguousarray(fw.reshape(KH, 128).T)

    common = dict(xT=xT, Cr=C.astype(f16), Sr=Sm.astype(f16),
                  J=J.astype(f16), ident=ident.astype(f16),
                  dmask=dm.astype(f16), n1w=n1w, n2w=n2w, fw=fwh)
    in_maps = []
    for c in range(NC):
        fs = slice(c * FEAT, (c + 1) * FEAT)
        ps = slice(c * PC, (c + 1) * PC)
        vs = slice(c * VC, (c + 1) * VC)
        m = dict(common)
        wqT = wq[:, fs, :].transpose(0, 2, 1)
        wkT = wk[:, fs, :].transpose(0, 2, 1)
        wvT = wv[:, fs, :].transpose(0, 2, 1)
        m["wqkvT"] = np.ascontiguousarray(
            np.concatenate([wqT, wkT, wvT], axis=2)).astype(f16)
        m["woT"] = np.ascontiguousarray(
            wo[:, :, fs].transpose(0, 2, 1)).astype(f16)
        w1T = w1[:, ps, :].transpose(0, 2, 1)   # [L, H, PC]
        w3T = w3[:, ps, :].transpose(0, 2, 1)
        pad = np.zeros((L, H, PCP - PC), np.float32)
        m["w13T"] = np.ascontiguousarray(np.concatenate(
            [w1T, pad, w3T, pad], axis=2)).astype(f16)
        w2p = np.zeros((L, PCP, H), np.float32)
        w2p[:, :PC, :] = w2[:, :, ps].transpose(0, 2, 1)
        m["w2T"] = np.ascontiguousarray(w2p).astype(f16)
        m["owT"] = np.ascontiguousarray(ow[vs, :].T).astype(f16)
        in_maps.append(m)
    return in_maps


def kernel(**inputs) -> np.ndarray:
    from concourse import bass_utils

    if "nc" not in _STATE:
        _STATE["nc"] = _build()
    in_maps = _shard(inputs)
    res = bass_utils.run_bass_kernel_spmd(
        _STATE["nc"], in_maps, core_ids=list(range(NC)))
    out = np.concatenate(
        [res.results[c]["logits"] for c in range(NC)], axis=1)
    return out.astype(np.float32)


# revision 10
# speedup vs baseline: 1.6454x; 1.0004x over previous
"""Trainium2 Bass kernel: 4-layer decoder prefill (S=1024, H=2048, NH=16, HD=128,
FFN=5632, V=32000), tensor-parallel over 8 NeuronCores.

- Megatron TP over 8 cores: wq/wk/wv/w1/w3 sharded on output dim (2 heads /
  704 ffn rows per core), wo/w2 sharded on input dim (partials ->
  ReduceScatter+AllGather), out_w sharded over vocab (4000 rows/core); only
  the last token's logits are computed.
- All matmuls in fp16 (weights pre-cast on host, fp32 accumulation in PSUM);
  the residual stream lives TRANSPOSED in SBUF as fp16 (xT: [H on
  partition-chunks, S free]).
- Causal structure exploited: fully-masked score blocks are skipped; diagonal
  blocks use 4 precomputed multiplicative 0/1 mask tiles; 1/sqrt(HD) is
  folded into the Exp activation scale.
- Softmax denominators and rms-norm sums are accumulated as PE matmuls with
  an all-ones [128,128] stationary, which broadcasts the partition-sum to all
  128 partitions directly -- no slow [1,N] single-partition ops; inverses via
  reciprocal_approx_fast (single DVE op).
- wqkv/wo are SBUF-resident per layer; w13/w2 streamed; ffn w1|w3 are
  zero-padded to 768 rows each so all chunks are full 128 partitions.
- Last layer: k/v for all tokens but q/attention/FFN only for the last
  tokens; logits fp16 GEMV streamed over the vocab shard.
"""

import os
import sys

sys.path.insert(0, "/opt/trn_rl_repo")

import numpy as np

L = 4
B, S, H, NH, HD = 1, 1024, 2048, 16, 128
V, P = 32000, 5632
NC = 8
FEAT = H // NC          # 256 q/k/v features per core (2 heads)
PC = P // NC            # 704 ffn rows per core
PCP = 768               # padded to 6 full 128-chunks
VC = V // NC            # 4000 vocab rows per core
KH = H // 128           # 16 H-chunks
EPS = 1e-5
SCALE = float(np.sqrt(HD))
INV_SCALE = 1.0 / SCALE

_STATE = {}


def _build():
    import concourse.bass as bass
    import concourse.bacc as bacc
    from concourse import tile, mybir

    F32 = mybir.dt.float32
    F16 = mybir.dt.float16
    AF = mybir.ActivationFunctionType
    ALU = mybir.AluOpType
    ts = bass.ts

    nc = bacc.Bacc("TRN2", target_bir_lowering=False, debug=False, num_devices=NC)

    xT_h = nc.dram_tensor("xT", [H, S], F16, kind="ExternalInput")
    C_h = nc.dram_tensor("Cr", [128, S], F16, kind="ExternalInput")
    S_h = nc.dram_tensor("Sr", [128, S], F16, kind="ExternalInput")
    J_h = nc.dram_tensor("J", [128, 128], F16, kind="ExternalInput")
    id_h = nc.dram_tensor("ident", [128, 128], F16, kind="ExternalInput")
    dm_h = nc.dram_tensor("dmask", [128, 4 * 512], F16, kind="ExternalInput")
    n1w_h = nc.dram_tensor("n1w", [128, L * KH], F32, kind="ExternalInput")
    n2w_h = nc.dram_tensor("n2w", [128, L * KH], F32, kind="ExternalInput")
    fw_h = nc.dram_tensor("fw", [128, KH], F32, kind="ExternalInput")
    # wq|wk|wv concatenated on the last axis: [L, H, 3*FEAT]
    wqkv_h = nc.dram_tensor("wqkvT", [L, H, 3 * FEAT], F16, kind="ExternalInput")
    woT_h = nc.dram_tensor("woT", [L, FEAT, H], F16, kind="ExternalInput")
    # [w1 | 64pad | w3 | 64pad] on cols: [L, H, 2*PCP]
    w13_h = nc.dram_tensor("w13T", [L, H, 2 * PCP], F16, kind="ExternalInput")
    w2T_h = nc.dram_tensor("w2T", [L, PCP, H], F16, kind="ExternalInput")
    owT_h = nc.dram_tensor("owT", [H, VC], F16, kind="ExternalInput")
    out_h = nc.dram_tensor("logits", [1, VC], F32, kind="ExternalOutput")

    from contextlib import ExitStack

    with tile.TileContext(nc) as tc, ExitStack() as _ctx:
        ec = _ctx.enter_context
        p_resid = ec(tc.tile_pool(name="resid", bufs=1))
        p_const = ec(tc.tile_pool(name="consts", bufs=1))
        p_big = ec(tc.tile_pool(name="big", bufs=3))
        p_vs = ec(tc.tile_pool(name="vsn", bufs=2))
        p_attn = ec(tc.tile_pool(name="attnp", bufs=2))
        p_pt = ec(tc.tile_pool(name="ptile", bufs=3))
        p_ns = ec(tc.tile_pool(name="normsc", bufs=3))
        p_hn = ec(tc.tile_pool(name="hnp", bufs=17))
        p_nrm = ec(tc.tile_pool(name="nrm", bufs=2))
        p_gu = ec(tc.tile_pool(name="gup", bufs=2))
        p_wres = ec(tc.tile_pool(name="wres", bufs=1))
        p_w13 = ec(tc.tile_pool(name="w13p", bufs=3))
        p_w2 = ec(tc.tile_pool(name="w2p", bufs=4))
        p_ow = ec(tc.tile_pool(name="owp", bufs=6))
        p_ar = ec(tc.tile_pool(name="ars", bufs=6))
        p_row = ec(tc.tile_pool(name="row", bufs=2))
        psum = ec(tc.tile_pool(name="psum", bufs=7, space="PSUM"))
        dram = ec(tc.tile_pool(name="dram", bufs=4, space="DRAM"))

        # ---- constants / inputs ----
        xT = p_resid.tile([128, KH * S], F16, tag="xT")
        for hc in range(KH):
            nc.sync.dma_start(xT[:, ts(hc, S)], xT_h.ap()[ts(hc, 128), :])

        C_s = p_const.tile([128, S], F16, tag="C")
        nc.sync.dma_start(C_s[:], C_h.ap())
        S_s = p_const.tile([128, S], F16, tag="S")
        nc.sync.dma_start(S_s[:], S_h.ap())
        J_r = p_const.tile([128, 128], F16, tag="J")
        nc.sync.dma_start(J_r[:], J_h.ap())
        id_r = p_const.tile([128, 128], F16, tag="id")
        nc.sync.dma_start(id_r[:], id_h.ap())
        dmask = p_const.tile([128, 4 * 512], F16, tag="dm")
        nc.sync.dma_start(dmask[:], dm_h.ap())
        n1w = p_const.tile([128, L * KH], F32, tag="n1w")
        nc.sync.dma_start(n1w[:], n1w_h.ap())
        n2w = p_const.tile([128, L * KH], F32, tag="n2w")
        nc.sync.dma_start(n2w[:], n2w_h.ap())
        fw_s = p_const.tile([128, KH], F32, tag="fw")
        nc.sync.dma_start(fw_s[:], fw_h.ap())
        ones_mat = p_const.tile([128, 128], F16, tag="om")
        nc.vector.memset(ones_mat[:], 1.0)
        ones_col = p_const.tile([128, 1], F16, tag="o1")
        nc.vector.memset(ones_col[:], 1.0)
        eps_p = p_const.tile([128, 1], F32, tag="epsp")
        nc.vector.memset(eps_p[:], EPS)
        eps_t = p_const.tile([1, 1], F32, tag="eps")
        nc.vector.memset(eps_t[:], EPS)

        def load_wo(l_):
            wo_sb = p_wres.tile([128, 2 * H], F16, tag="wo", name="wosb")
            for fc in range(2):
                nc.sync.dma_start(wo_sb[:, ts(fc, H)],
                                  woT_h.ap()[l_, ts(fc, 128), :])
            return wo_sb

        def load_wqkv(l_):
            wq_sb = p_wres.tile([128, KH * 3 * FEAT], F16, tag="wqkv",
                                name="wqsb")
            for hc in range(KH):
                nc.sync.dma_start(wq_sb[:, ts(hc, 3 * FEAT)],
                                  wqkv_h.ap()[l_, ts(hc, 128), :])
            return wq_sb

        def norm_inv(w_tile, l_, tk):
            """[128,512] fp32 tile of 1/rms for tokens [tk*512, tk*512+512)."""
            nb_ps = psum.tile([128, 512], F32, tag="ps", name="nbps")
            for hc in range(KH):
                sq = p_ns.tile([128, 512], F16, tag="sq", name="sq")
                sl = slice(hc * S + tk * 512, hc * S + tk * 512 + 512)
                nc.vector.tensor_mul(sq[:], xT[:, sl], xT[:, sl])
                nc.tensor.matmul(nb_ps[:], ones_mat[:], sq[:],
                                 start=(hc == 0), stop=(hc == KH - 1))
            rms = p_nrm.tile([128, 512], F32, tag="rms", name="rms")
            nc.scalar.activation(rms[:], nb_ps[:], AF.Sqrt,
                                 bias=eps_p[:], scale=1.0 / H)
            inv = p_nrm.tile([128, 512], F32, tag="inv", name="inv")
            nc.vector.reciprocal_approx_fast(inv[:], rms[:])
            return inv

        def qkv_half(wq_sb, l_, tk, q_s, k_s, vT_s):
            """QKV for token half tk of layer l_ (writes [:, mt*S+tk*512]).

            Chunk-outer: 6 independent 16-matmul accumulation chains, one
            live PSUM each, so the PE queue never blocks on evictions.
            For the last layer, q is computed only for the last 2 tokens
            (tk==1) into q_s[:, mt*S + S-2 : mt*S + S]."""
            last = (l_ == L - 1)
            inv = norm_inv(n1w, l_, tk)
            xns = []
            for hc in range(KH):
                xn = p_hn.tile([128, 512], F16, tag="hn", name="xn")
                nc.vector.scalar_tensor_tensor(
                    xn[:], xT[:, hc * S + tk * 512: hc * S + tk * 512 + 512],
                    n1w[:, l_ * KH + hc: l_ * KH + hc + 1],
                    inv[:], op0=ALU.mult, op1=ALU.mult)
                xns.append(xn)
            # j: 0,1 = q heads; 2,3 = k heads; 4,5 = v heads
            for j in range(6):
                mt = j % 2
                if j < 2 and last:
                    if tk == 1:
                        ps = psum.tile([128, 2], F32, tag="ps", name="qLp")
                        for hc in range(KH):
                            nc.tensor.matmul(
                                ps[:], wq_sb[:, hc * 768 + j * 128:
                                             hc * 768 + j * 128 + 128],
                                xns[hc][:, 510:512],
                                start=(hc == 0), stop=(hc == KH - 1))
                        nc.vector.tensor_copy(
                            q_s[:, mt * S + S - 2: mt * S + S], ps[:])
                    continue
                ps = psum.tile([128, 512], F32, tag="ps", name="qkvp")
                for hc in range(KH):
                    nc.tensor.matmul(
                        ps[:], wq_sb[:, hc * 768 + j * 128:
                                     hc * 768 + j * 128 + 128],
                        xns[hc][:], start=(hc == 0), stop=(hc == KH - 1))
                dst = (q_s, q_s, k_s, k_s, vT_s, vT_s)[j]
                off = mt * S + tk * 512
                nc.vector.tensor_copy(dst[:, off:off + 512], ps[:])

        def rope_slice(t_s, col, width, ccol):
            """RoPE in place on t_s[:, col:col+width]; cos/sin cols at ccol."""
            j_ps = psum.tile([128, 512], F32, tag="ps", name="jps")
            nc.tensor.matmul(j_ps[:, :width], J_r[:], t_s[:, col:col + width],
                             start=True, stop=True)
            tmp = p_pt.tile([128, 512], F16, tag="rtmp", name="rtmp")
            nc.vector.tensor_mul(tmp[:, :width], C_s[:, ccol:ccol + width],
                                 t_s[:, col:col + width])
            nc.vector.tensor_mul(t_s[:, col:col + width], j_ps[:, :width],
                                 S_s[:, ccol:ccol + width])
            nc.vector.tensor_add(t_s[:, col:col + width],
                                 t_s[:, col:col + width], tmp[:, :width])

        def wo_project(wo_sb, attn_s, tk):
            """wo @ attn for half tk -> DRAM ar_in; RS+AG; returns ar_out."""
            ar_in = dram.tile([H, 512], F16, tag="arin", name="arin")
            ar_out = dram.tile([H, 512], F16, tag="arout",
                               addr_space="Shared", name="arout")
            for hc in range(KH):
                po = psum.tile([128, 512], F32, tag="ps", name="po")
                for fc in range(2):
                    nc.tensor.matmul(
                        po[:], wo_sb[:, fc * H + hc * 128: fc * H + hc * 128 + 128],
                        attn_s[:, fc * S + tk * 512: fc * S + tk * 512 + 512],
                        start=(fc == 0), stop=(fc == 1))
                ar_sb = p_ar.tile([128, 512], F16, tag="ar", name="arsb")
                nc.scalar.activation(ar_sb[:], po[:], AF.Copy)
                nc.sync.dma_start(ar_in[ts(hc, 128), :], ar_sb[:])
            ar_mid = dram.tile([H // NC, 512], F16, tag="armid", name="armid")
            nc.gpsimd.collective_compute(
                "ReduceScatter", ALU.add, replica_groups=[list(range(NC))],
                ins=[ar_in[:].opt()], outs=[ar_mid[:].opt()])
            nc.gpsimd.collective_compute(
                "AllGather", ALU.bypass, replica_groups=[list(range(NC))],
                ins=[ar_mid[:].opt()], outs=[ar_out[:].opt()])
            return ar_out

        def resid_add(ar_out, tk):
            for hc in range(KH):
                ar_t = p_ar.tile([128, 512], F16, tag="ar", name="art")
                nc.sync.dma_start(ar_t[:], ar_out[ts(hc, 128), :])
                sl = slice(hc * S + tk * 512, hc * S + tk * 512 + 512)
                nc.vector.tensor_add(xT[:, sl], xT[:, sl], ar_t[:])

        def ffn_half(l_, tk):
            """norm2 + SwiGLU FFN + down proj for half tk; launches AR2."""
            inv = norm_inv(n2w, l_, tk)
            hns = []
            for hc in range(KH):
                hn = p_hn.tile([128, 512], F16, tag="hn", name="hn")
                nc.vector.scalar_tensor_tensor(
                    hn[:],
                    xT[:, hc * S + tk * 512: hc * S + tk * 512 + 512],
                    n2w[:, l_ * KH + hc: l_ * KH + hc + 1],
                    inv[:], op0=ALU.mult, op1=ALU.mult)
                hns.append(hn)
            gu_sb = p_gu.tile([128, 12 * 512], F16, tag="gu", name="gusb")
            for pi in range(3):   # passes of 4 chunks: 4 live PSUMs + slack
                pset = range(4 * pi, 4 * pi + 4)
                gus = {c: psum.tile([128, 512], F32, tag="ps", name=f"gu{c}")
                       for c in pset}
                for hc in range(KH):
                    w13_t = p_w13.tile([128, 512], F16, tag="w13", name="w13t")
                    nc.sync.dma_start(
                        w13_t[:], w13_h.ap()[l_, ts(hc, 128),
                                             pi * 512: pi * 512 + 512])
                    st, sp = (hc == 0), (hc == KH - 1)
                    for ci, c in enumerate(pset):
                        nc.tensor.matmul(gus[c][:], w13_t[:, ts(ci, 128)],
                                         hns[hc][:], start=st, stop=sp)
                for c in pset:
                    nc.scalar.activation(gu_sb[:, ts(c, 512)], gus[c][:],
                                         AF.Copy)
            # swig[s] = silu(g[s]) * u[s]  (in place over g chunks 0..5)
            for sch in range(6):
                sg = p_ns.tile([128, 512], F16, tag="ns", name="sg")
                nc.scalar.activation(sg[:], gu_sb[:, ts(sch, 512)], AF.Silu)
                nc.vector.tensor_mul(gu_sb[:, ts(sch, 512)], sg[:],
                                     gu_sb[:, ts(6 + sch, 512)])
            # down projection
            ar2_in = dram.tile([H, 512], F16, tag="arin", name="ar2in")
            ar2_out = dram.tile([H, 512], F16, tag="arout",
                                addr_space="Shared", name="ar2out")
            for hcb in range(4):
                p2 = [psum.tile([128, 512], F32, tag="ps", name=f"p2{i}")
                      for i in range(4)]
                for kc in range(6):
                    w2_t = p_w2.tile([128, 512], F16, tag="w2", name="w2t")
                    nc.sync.dma_start(
                        w2_t[:], w2T_h.ap()[l_, ts(kc, 128),
                                            hcb * 512: hcb * 512 + 512])
                    for hh in range(4):
                        nc.tensor.matmul(p2[hh][:], w2_t[:, ts(hh, 128)],
                                         gu_sb[:, ts(kc, 512)],
                                         start=(kc == 0), stop=(kc == 5))
                for hh in range(4):
                    a2 = p_ar.tile([128, 512], F16, tag="ar", name="a2")
                    nc.scalar.activation(a2[:], p2[hh][:], AF.Copy)
                    nc.sync.dma_start(ar2_in[ts(hcb * 4 + hh, 128), :], a2[:])
            ar_mid = dram.tile([H // NC, 512], F16, tag="armid", name="ar2mid")
            nc.gpsimd.collective_compute(
                "ReduceScatter", ALU.add, replica_groups=[list(range(NC))],
                ins=[ar2_in[:].opt()], outs=[ar_mid[:].opt()])
            nc.gpsimd.collective_compute(
                "AllGather", ALU.bypass, replica_groups=[list(range(NC))],
                ins=[ar_mid[:].opt()], outs=[ar2_out[:].opt()])
            return ar2_out

        def rope_vtrans_half(q_s, k_s, vT_s, v_s, tk):
            for t_s in (q_s, k_s):
                for mt in range(2):
                    rope_slice(t_s, mt * S + tk * 512, 512, tk * 512)
            for mt in range(2):
                for tb in range(tk * 4, tk * 4 + 4):
                    tp = psum.tile([128, 128], F16, tag="ps", name="tp")
                    nc.tensor.transpose(
                        tp[:],
                        vT_s[:, mt * S + tb * 128: mt * S + tb * 128 + 128],
                        id_r[:])
                    nc.vector.tensor_copy(
                        v_s[:, tb * FEAT + mt * 128:
                            tb * FEAT + mt * 128 + 128], tp[:])

        def attn_half(q_s, k_s, v_s, attn_s, tk):
            """attention for half tk: only causally visible key blocks.

            The score matmul + exp for block kc+1 is emitted before the
            AV/sum matmuls of block kc so the PE never waits on the Exp."""
            nvis = (tk + 1) * 4

            def emit_sc(h, kc):
                sc_ps = psum.tile([128, 512], F32, tag="ps", name="scp")
                nc.tensor.matmul(
                    sc_ps[:],
                    k_s[:, h * S + kc * 128: h * S + kc * 128 + 128],
                    q_s[:, h * S + tk * 512: h * S + tk * 512 + 512],
                    start=True, stop=True)
                pt = p_pt.tile([128, 512], F16, tag="pt", name="ptl")
                nc.scalar.activation(pt[:], sc_ps[:], AF.Exp,
                                     scale=INV_SCALE)
                d = kc * 128 - tk * 512
                if d >= 0:
                    nc.vector.tensor_mul(
                        pt[:], pt[:], dmask[:, ts(d // 128, 512)])
                return pt

            for h in range(2):
                at_ps = psum.tile([128, 512], F32, tag="ps", name="atp")
                ib_ps = psum.tile([128, 512], F32, tag="ps", name="ibp")
                pt = emit_sc(h, 0)
                for kc in range(nvis):
                    pt_next = emit_sc(h, kc + 1) if kc + 1 < nvis else None
                    st, sp = (kc == 0), (kc == nvis - 1)
                    nc.tensor.matmul(
                        at_ps[:],
                        v_s[:, kc * FEAT + h * 128: kc * FEAT + h * 128 + 128],
                        pt[:], start=st, stop=sp)
                    nc.tensor.matmul(ib_ps[:], ones_mat[:], pt[:],
                                     start=st, stop=sp)
                    pt = pt_next
                inv_a = p_pt.tile([128, 512], F32, tag="pta", name="inva")
                nc.vector.reciprocal_approx_fast(inv_a[:], ib_ps[:])
                nc.vector.tensor_mul(
                    attn_s[:, h * S + tk * 512: h * S + tk * 512 + 512],
                    at_ps[:], inv_a[:])

        # ---- layer 0 prologue: only the A-half of QKV(0); the B-half is
        # computed inside layer 0 under AR1(A)'s shadow ----
        wo_sb = load_wo(0)
        wq_sb = load_wqkv(0)
        cur_q = p_big.tile([128, 2 * S], F16, tag="big", name="q0")
        cur_k = p_big.tile([128, 2 * S], F16, tag="big", name="k0")
        cur_vT = p_big.tile([128, 2 * S], F16, tag="big", name="vT0")
        qkv_half(wq_sb, 0, 0, cur_q, cur_k, cur_vT)
        ar2_prev_b = None

        for l in range(L):
            last = (l == L - 1)
            q_s, k_s, vT_s = cur_q, cur_k, cur_vT

            v_s = p_vs.tile([128, 8 * FEAT], F16, tag="v", name="vs")
            attn_s = p_attn.tile([128, 2 * S], F16, tag="attn", name="attn")

            if last:
                # B-half QKV of the last layer (k/v all tokens, q last-2)
                if ar2_prev_b is not None:
                    resid_add(ar2_prev_b, 1)
                qkv_half(wq_sb, l, 1, q_s, k_s, vT_s)
                # RoPE on k (all tokens) and q (last 2 only)
                for mt in range(2):
                    for n in range(2):
                        rope_slice(k_s, mt * S + n * 512, 512, n * 512)
                    rope_slice(q_s, mt * S + S - 2, 2, S - 2)
                # V -> [tok, feat] via PE transpose
                for mt in range(2):
                    for tb in range(8):
                        tp = psum.tile([128, 128], F16, tag="ps", name="tp")
                        nc.tensor.transpose(
                            tp[:],
                            vT_s[:, mt * S + tb * 128: mt * S + tb * 128 + 128],
                            id_r[:])
                        nc.vector.tensor_copy(
                            v_s[:, tb * FEAT + mt * 128:
                                tb * FEAT + mt * 128 + 128], tp[:])
                # attention for the last 2 tokens only
                for h in range(2):
                    at1 = psum.tile([128, 2], F32, tag="ps", name="at1")
                    ib1 = psum.tile([128, 2], F32, tag="ps", name="ib1")
                    for kc in range(8):
                        sc1 = psum.tile([128, 2], F32, tag="ps", name="sc1")
                        nc.tensor.matmul(
                            sc1[:],
                            k_s[:, h * S + kc * 128: h * S + kc * 128 + 128],
                            q_s[:, h * S + S - 2: h * S + S],
                            start=True, stop=True)
                        pt1 = p_pt.tile([128, 2], F16, tag="pt1", name="pt1")
                        nc.scalar.activation(pt1[:], sc1[:], AF.Exp,
                                             scale=INV_SCALE)
                        if kc == 7:
                            nc.vector.tensor_mul(
                                pt1[:], pt1[:],
                                dmask[:, 3 * 512 + 510: 3 * 512 + 512])
                        st, sp = (kc == 0), (kc == 7)
                        nc.tensor.matmul(
                            at1[:],
                            v_s[:, kc * FEAT + h * 128: kc * FEAT + h * 128 + 128],
                            pt1[:], start=st, stop=sp)
                        nc.tensor.matmul(ib1[:], ones_mat[:], pt1[:],
                                         start=st, stop=sp)
                    inva = p_pt.tile([128, 2], F32, tag="pta", name="inva")
                    nc.vector.reciprocal_approx_fast(inva[:], ib1[:])
                    nc.vector.tensor_mul(
                        attn_s[:, h * S + S - 2: h * S + S], at1[:], inva[:])

                # wo -> [H,2] AllReduce -> residual add (last 2 tokens)
                ar_in = dram.tile([H, 2], F16, tag="arinL", name="arinL")
                ar_out = dram.tile([H, 2], F16, tag="aroutL",
                                   addr_space="Shared", name="aroutL")
                for hc in range(KH):
                    poL = psum.tile([128, 2], F32, tag="ps", name="poL")
                    for fc in range(2):
                        nc.tensor.matmul(
                            poL[:],
                            wo_sb[:, fc * H + hc * 128: fc * H + hc * 128 + 128],
                            attn_s[:, fc * S + S - 2: fc * S + S],
                            start=(fc == 0), stop=(fc == 1))
                    arL = p_pt.tile([128, 2], F16, tag="arL", name="arL")
                    nc.scalar.activation(arL[:], poL[:], AF.Copy)
                    nc.sync.dma_start(ar_in[ts(hc, 128), :], arL[:])
                nc.gpsimd.collective_compute(
                    "AllReduce", ALU.add, replica_groups=[list(range(NC))],
                    ins=[ar_in[:].opt()], outs=[ar_out[:].opt()])
                for hc in range(KH):
                    ar_t = p_pt.tile([128, 2], F16, tag="arL", name="art")
                    nc.sync.dma_start(ar_t[:], ar_out[ts(hc, 128), :])
                    nc.vector.tensor_add(
                        xT[:, hc * S + S - 2: hc * S + S],
                        xT[:, hc * S + S - 2: hc * S + S], ar_t[:])

                # norm2 + FFN on the last 2 tokens
                sqL = p_row.tile([128, 2 * KH], F16, tag="sql2")
                for hc in range(KH):
                    col = hc * S + S - 2
                    nc.vector.tensor_mul(sqL[:, 2 * hc:2 * hc + 2],
                                         xT[:, col:col + 2], xT[:, col:col + 2])
                ssL = psum.tile([128, 2 * KH], F32, tag="ps", name="ssL")
                nc.tensor.matmul(ssL[:], ones_mat[:], sqL[:],
                                 start=True, stop=True)
                ssr = p_row.tile([128, 2], F32, tag="ssr")
                nc.vector.reduce_sum(
                    ssr[:], ssL[:].rearrange("p (c two) -> p two c", two=2),
                    axis=mybir.AxisListType.X)
                rmsL = p_row.tile([128, 2], F32, tag="rmsL")
                nc.scalar.activation(rmsL[:], ssr[:], AF.Sqrt,
                                     bias=eps_p[:], scale=1.0 / H)
                invL = p_row.tile([128, 2], F32, tag="invLc")
                nc.vector.reciprocal_approx_fast(invL[:], rmsL[:])
                hnL = p_row.tile([128, 2 * KH], F16, tag="hnL")
                tnL = p_row.tile([128, 2], F32, tag="tnL")
                for hc in range(KH):
                    col = hc * S + S - 2
                    nc.vector.tensor_scalar_mul(
                        tnL[:], xT[:, col:col + 2],
                        n2w[:, l * KH + hc: l * KH + hc + 1])
                    nc.vector.tensor_mul(hnL[:, 2 * hc:2 * hc + 2],
                                         tnL[:], invL[:])
                guL = p_row.tile([128, 12 * 2], F16, tag="guL")
                for pi, pset in enumerate((range(0, 6), range(6, 12))):
                    gps = {c: psum.tile([128, 2], F32, tag="ps",
                                        name=f"gL{c}") for c in pset}
                    for hc in range(KH):
                        w13_t = p_w13.tile([128, 768], F16, tag="w13",
                                           name="w13tL")
                        nc.sync.dma_start(
                            w13_t[:], w13_h.ap()[l, ts(hc, 128),
                                                 pi * 768: pi * 768 + 768])
                        st, sp = (hc == 0), (hc == KH - 1)
                        for ci, c in enumerate(pset):
                            nc.tensor.matmul(
                                gps[c][:], w13_t[:, ts(ci, 128)],
                                hnL[:, 2 * hc:2 * hc + 2], start=st, stop=sp)
                    for c in pset:
                        nc.scalar.activation(guL[:, 2 * c:2 * c + 2],
                                             gps[c][:], AF.Copy)
                swL = p_row.tile([128, 6 * 2], F16, tag="swL")
                for sch in range(6):
                    sgL = p_row.tile([128, 2], F16, tag="sgL")
                    nc.scalar.activation(sgL[:], guL[:, 2 * sch:2 * sch + 2],
                                         AF.Silu)
                    nc.vector.tensor_mul(swL[:, 2 * sch:2 * sch + 2], sgL[:],
                                         guL[:, 2 * (6 + sch):2 * (6 + sch) + 2])
                ar2_in = dram.tile([H, 2], F16, tag="arinL", name="ar2inL")
                ar2_out = dram.tile([H, 2], F16, tag="aroutL",
                                    addr_space="Shared", name="ar2outL")
                for hc in range(KH):
                    p2L = psum.tile([128, 2], F32, tag="ps", name="p2L")
                    for kc in range(6):
                        w2_t = p_w2.tile([128, 128], F16, tag="w2L",
                                         name="w2tL")
                        nc.sync.dma_start(
                            w2_t[:], w2T_h.ap()[l, ts(kc, 128), ts(hc, 128)])
                        nc.tensor.matmul(p2L[:], w2_t[:],
                                         swL[:, 2 * kc:2 * kc + 2],
                                         start=(kc == 0), stop=(kc == 5))
                    a2L = p_pt.tile([128, 2], F16, tag="arL", name="a2L")
                    nc.scalar.activation(a2L[:], p2L[:], AF.Copy)
                    nc.sync.dma_start(ar2_in[ts(hc, 128), :], a2L[:])
                nc.gpsimd.collective_compute(
                    "AllReduce", ALU.add, replica_groups=[list(range(NC))],
                    ins=[ar2_in[:].opt()], outs=[ar2_out[:].opt()])
                for hc in range(KH):
                    ar_t = p_pt.tile([128, 2], F16, tag="arL", name="art2")
                    nc.sync.dma_start(ar_t[:], ar2_out[ts(hc, 128), :])
                    nc.vector.tensor_add(
                        xT[:, hc * S + S - 2: hc * S + S],
                        xT[:, hc * S + S - 2: hc * S + S], ar_t[:])
                continue

            # ---- non-last layer: A-half attention first, then the B-half
            # QKV runs in AR1(A)'s shadow ----
            rope_vtrans_half(q_s, k_s, vT_s, v_s, 0)
            attn_half(q_s, k_s, v_s, attn_s, 0)
            ar1_a = wo_project(wo_sb, attn_s, 0)

            if ar2_prev_b is not None:
                resid_add(ar2_prev_b, 1)
            qkv_half(wq_sb, l, 1, q_s, k_s, vT_s)

            rope_vtrans_half(q_s, k_s, vT_s, v_s, 1)
            attn_half(q_s, k_s, v_s, attn_s, 1)
            ar1_b = wo_project(wo_sb, attn_s, 1)

            # prefetch next layer's qkv weights (slot free: qkv(l) done)
            wq_sb = load_wqkv(l + 1)

            resid_add(ar1_a, 0)
            ar2_a = ffn_half(l, 0)
            resid_add(ar1_b, 1)
            ar2_prev_b = ffn_half(l, 1)

            # next layer's wo
            wo_sb = load_wo(l + 1)

            nxt_q = p_big.tile([128, 2 * S], F16, tag="big", name="qn")
            nxt_k = p_big.tile([128, 2 * S], F16, tag="big", name="kn")
            nxt_vT = p_big.tile([128, 2 * S], F16, tag="big", name="vTn")
            resid_add(ar2_a, 0)
            qkv_half(wq_sb, l + 1, 0, nxt_q, nxt_k, nxt_vT)
            cur_q, cur_k, cur_vT = nxt_q, nxt_k, nxt_vT

        # ======== final norm (last token only) + logits ========
        sq_l = p_row.tile([128, KH], F16, tag="sql")
        for hc in range(KH):
            col = hc * S + S - 1
            nc.vector.tensor_mul(sq_l[:, hc:hc + 1], xT[:, col:col + 1],
                                 xT[:, col:col + 1])
        sl_ps = psum.tile([1, KH], F32, tag="ps", name="slps")
        nc.tensor.matmul(sl_ps[:], ones_col[:], sq_l[:], start=True, stop=True)
        ssc = p_row.tile([1, 1], F32, tag="ssc")
        nc.vector.reduce_sum(ssc[:], sl_ps[:], axis=mybir.AxisListType.X)
        rms_l = p_row.tile([1, 1], F32, tag="rmsl")
        nc.scalar.activation(rms_l[:], ssc[:], AF.Sqrt, bias=eps_t[:],
                             scale=1.0 / H)
        inv_l = p_row.tile([1, 1], F32, tag="invl")
        nc.vector.reciprocal(inv_l[:], rms_l[:])
        xnl = p_row.tile([128, KH], F16, tag="xnl")
        for hc in range(KH):
            col = hc * S + S - 1
            nc.vector.tensor_mul(xnl[:, hc:hc + 1], xT[:, col:col + 1],
                                 fw_s[:, hc:hc + 1])
        for n in range(4):
            lg_a = psum.tile([1, 500], F32, tag="ps", name="lga")
            lg_b = psum.tile([1, 500], F32, tag="ps", name="lgb")
            for hc in range(KH):
                ow_t = p_ow.tile([128, 1000], F16, tag="ow", name="owt")
                nc.sync.dma_start(
                    ow_t[:], owT_h.ap()[ts(hc, 128), n * 1000: n * 1000 + 1000])
                st, sp = (hc == 0), (hc == KH - 1)
                nc.tensor.matmul(lg_a[:], xnl[:, hc: hc + 1], ow_t[:, :500],
                                 start=st, stop=sp)
                nc.tensor.matmul(lg_b[:], xnl[:, hc: hc + 1], ow_t[:, 500:],
                                 start=st, stop=sp)
            lg = p_row.tile([1, 1000], F32, tag="lg")
            nc.scalar.activation(lg[:, :500], lg_a[:], AF.Copy, scale=inv_l[:])
            nc.scalar.activation(lg[:, 500:], lg_b[:], AF.Copy, scale=inv_l[:])
            nc.sync.dma_start(out_h.ap()[:, n * 1000: n * 1000 + 1000], lg[:])

    nc.compile()
    return nc


def _shard(inputs):
    f16 = np.float16
    x = np.asarray(inputs["x"], np.float32)
    cos = np.asarray(inputs["cos"], np.float32).reshape(S, HD // 2)
    sin = np.asarray(inputs["sin"], np.float32).reshape(S, HD // 2)
    n1 = np.asarray(inputs["norm1_w"], np.float32)[:L]
    n2 = np.asarray(inputs["norm2_w"], np.float32)[:L]
    fw = np.asarray(inputs["final_norm_w"], np.float32)
    wq = np.asarray(inputs["wq"], np.float32)[:L]
    wk = np.asarray(inputs["wk"], np.float32)[:L]
    wv = np.asarray(inputs["wv"], np.float32)[:L]
    wo = np.asarray(inputs["wo"], np.float32)[:L]
    w1 = np.asarray(inputs["w1"], np.float32)[:L]
    w3 = np.asarray(inputs["w3"], np.float32)[:L]
    w2 = np.asarray(inputs["w2"], np.float32)[:L]
    ow = np.asarray(inputs["out_w"], np.float32)

    xT = np.ascontiguousarray(x[0].T).astype(f16)
    C = np.empty((128, S), np.float32)
    C[0::2] = cos.T
    C[1::2] = cos.T
    Sm = np.empty((128, S), np.float32)
    Sm[0::2] = -sin.T
    Sm[1::2] = sin.T
    J = np.zeros((128, 128), np.float32)
    idx = np.arange(0, 128, 2)
    J[idx, idx + 1] = 1.0
    J[idx + 1, idx] = 1.0
    ident = np.eye(128, dtype=np.float32)
    # diagonal causal masks: pattern di (block offset di*128):
    # mask[kp, q] = 1 if q >= kp + di*128
    dm = np.zeros((128, 4 * 512), np.float32)
    kp = np.arange(128)[:, None]
    qq = np.arange(512)[None, :]
    for di in range(4):
        dm[:, di * 512:(di + 1) * 512] = (qq >= kp + di * 128)
    n1w = np.ascontiguousarray(
        n1.reshape(L, KH, 128).transpose(2, 0, 1).reshape(128, L * KH))
    n2w = np.ascontiguousarray(
        n2.reshape(L, KH, 128).transpose(2, 0, 1).reshape(128, L * KH))
    fwh = np.ascontiguousarray(fw.reshape(KH, 128).T)

    common = dict(xT=xT, Cr=C.astype(f16), Sr=Sm.astype(f16),
                  J=J.astype(f16), ident=ident.astype(f16),
                  dmask=dm.astype(f16), n1w=n1w, n2w=n2w, fw=fwh)
    in_maps = []
    for c in range(NC):
        fs = slice(c * FEAT, (c + 1) * FEAT)
        ps = slice(c * PC, (c + 1) * PC)
        vs = slice(c * VC, (c + 1) * VC)
        m = dict(common)
        wqT = wq[:, fs, :].transpose(0, 2, 1)
        wkT = wk[:, fs, :].transpose(0, 2, 1)
        wvT = wv[:, fs, :].transpose(0, 2, 1)
        m["wqkvT"] = np.ascontiguousarray(
            np.concatenate([wqT, wkT, wvT], axis=2)).astype(f16)
        m["woT"] = np.ascontiguousarray(
            wo[:, :, fs].transpose(0, 2, 1)).astype(f16)
        w1T = w1[:, ps, :].transpose(0, 2, 1)   # [L, H, PC]
        w3T = w3[:, ps, :].transpose(0, 2, 1)
        pad = np.zeros((L, H, PCP - PC), np.float32)
        m["w13T"] = np.ascontiguousarray(np.concatenate(
            [w1T, pad, w3T, pad], axis=2)).astype(f16)
        w2p = np.zeros((L, PCP, H), np.float32)
        w2p[:, :PC, :] = w2[:, :, ps].transpose(0, 2, 1)
        m["w2T"] = np.ascontiguousarray(w2p).astype(f16)
        m["owT"] = np.ascontiguousarray(ow[vs, :].T).astype(f16)
        in_maps.append(m)
    return in_maps


def kernel(**inputs) -> np.ndarray:
    from concourse import bass_utils

    if "nc" not in _STATE:
        _STATE["nc"] = _build()
    in_maps = _shard(inputs)
    res = bass_utils.run_bass_kernel_spmd(
        _STATE["nc"], in_maps, core_ids=list(range(NC)))
    out = np.concatenate(
        [res.results[c]["logits"] for c in range(NC)], axis=1)
    return out.astype(np.float32)


# revision 19
# speedup vs baseline: 1.7287x; 1.0506x over previous
"""Trainium2 Bass kernel: 4-layer decoder prefill (S=1024, H=2048, NH=16, HD=128,
FFN=5632, V=32000), tensor-parallel over 8 NeuronCores.

- Megatron TP over 8 cores: wq/wk/wv/w1/w3 sharded on output dim (2 heads /
  704 ffn rows per core), wo/w2 sharded on input dim (partials ->
  ReduceScatter+AllGather), out_w sharded over vocab (4000 rows/core); only
  the last token's logits are computed.
- All matmuls in bf16 (weights pre-cast on host, fp32 accumulation in PSUM);
  the residual stream lives TRANSPOSED in SBUF as bf16 (xT: [H on
  partition-chunks, S free]).
- Causal structure exploited: fully-masked score blocks are skipped; diagonal
  blocks use 4 precomputed multiplicative 0/1 mask tiles; 1/sqrt(HD) is
  folded into the Exp activation scale.
- Softmax denominators and rms-norm sums are accumulated as PE matmuls with
  an all-ones [128,128] stationary, which broadcasts the partition-sum to all
  128 partitions directly -- no slow [1,N] single-partition ops; inverses via
  reciprocal_approx_fast (single DVE op).
- wqkv/wo are SBUF-resident per layer; w13/w2 streamed; ffn w1|w3 are
  zero-padded to 768 rows each so all chunks are full 128 partitions.
- Last layer: k/v for all tokens but q/attention/FFN only for the last
  tokens; logits bf16 GEMV streamed over the vocab shard.
"""

import os
import sys

sys.path.insert(0, "/opt/trn_rl_repo")

import numpy as np

L = 4
B, S, H, NH, HD = 1, 1024, 2048, 16, 128
V, P = 32000, 5632
NC = 8
FEAT = H // NC          # 256 q/k/v features per core (2 heads)
PC = P // NC            # 704 ffn rows per core
PCP = 768               # padded to 6 full 128-chunks
VC = V // NC            # 4000 vocab rows per core
KH = H // 128           # 16 H-chunks
EPS = 1e-5
SCALE = float(np.sqrt(HD))
INV_SCALE = 1.0 / SCALE

_STATE = {}


def _build():
    import concourse.bass as bass
    import concourse.bacc as bacc
    from concourse import tile, mybir

    F32 = mybir.dt.float32
    F16 = mybir.dt.bfloat16  # bf16: native PE rate
    AF = mybir.ActivationFunctionType
    ALU = mybir.AluOpType
    ts = bass.ts

    nc = bacc.Bacc("TRN2", target_bir_lowering=False, debug=False, num_devices=NC)

    xT_h = nc.dram_tensor("xT", [H, S], F16, kind="ExternalInput")
    C_h = nc.dram_tensor("Cr", [128, S], F16, kind="ExternalInput")
    S_h = nc.dram_tensor("Sr", [128, S], F16, kind="ExternalInput")
    J_h = nc.dram_tensor("J", [128, 128], F16, kind="ExternalInput")
    id_h = nc.dram_tensor("ident", [128, 128], F16, kind="ExternalInput")
    dm_h = nc.dram_tensor("dmask", [128, 4 * 512], F16, kind="ExternalInput")
    n1w_h = nc.dram_tensor("n1w", [128, L * KH], F32, kind="ExternalInput")
    n2w_h = nc.dram_tensor("n2w", [128, L * KH], F32, kind="ExternalInput")
    fw_h = nc.dram_tensor("fw", [128, KH], F32, kind="ExternalInput")
    # wq|wk|wv concatenated on the last axis: [L, H, 3*FEAT]
    wqkv_h = nc.dram_tensor("wqkvT", [L, H, 3 * FEAT], F16, kind="ExternalInput")
    woT_h = nc.dram_tensor("woT", [L, FEAT, H], F16, kind="ExternalInput")
    # [w1 | 64pad | w3 | 64pad] on cols: [L, H, 2*PCP]
    w13_h = nc.dram_tensor("w13T", [L, H, 2 * PCP], F16, kind="ExternalInput")
    w2T_h = nc.dram_tensor("w2T", [L, PCP, H], F16, kind="ExternalInput")
    owT_h = nc.dram_tensor("owT", [H, VC], F16, kind="ExternalInput")
    out_h = nc.dram_tensor("logits", [1, VC], F32, kind="ExternalOutput")

    from contextlib import ExitStack

    with tile.TileContext(nc) as tc, ExitStack() as _ctx:
        ec = _ctx.enter_context
        p_resid = ec(tc.tile_pool(name="resid", bufs=1))
        p_const = ec(tc.tile_pool(name="consts", bufs=1))
        p_big = ec(tc.tile_pool(name="big", bufs=3))
        p_vs = ec(tc.tile_pool(name="vsn", bufs=2))
        p_attn = ec(tc.tile_pool(name="attnp", bufs=2))
        p_pt = ec(tc.tile_pool(name="ptile", bufs=3))
        p_ns = ec(tc.tile_pool(name="normsc", bufs=3))
        p_hn = ec(tc.tile_pool(name="hnp", bufs=17))
        p_nrm = ec(tc.tile_pool(name="nrm", bufs=2))
        p_gu = ec(tc.tile_pool(name="gup", bufs=1))
        p_wres = ec(tc.tile_pool(name="wres", bufs=1))
        p_w13 = ec(tc.tile_pool(name="w13p", bufs=19))
        p_w2 = ec(tc.tile_pool(name="w2p", bufs=4))
        p_ow = ec(tc.tile_pool(name="owp", bufs=6))
        p_ar = ec(tc.tile_pool(name="ars", bufs=6))
        p_row = ec(tc.tile_pool(name="row", bufs=2))
        psum = ec(tc.tile_pool(name="psum", bufs=7, space="PSUM"))
        dram = ec(tc.tile_pool(name="dram", bufs=4, space="DRAM"))

        # ---- constants / inputs ----
        xT = p_resid.tile([128, KH * S], F16, tag="xT")
        for hc in range(KH):
            nc.sync.dma_start(xT[:, ts(hc, S)], xT_h.ap()[ts(hc, 128), :])

        C_s = p_const.tile([128, S], F16, tag="C")
        nc.sync.dma_start(C_s[:], C_h.ap())
        S_s = p_const.tile([128, S], F16, tag="S")
        nc.sync.dma_start(S_s[:], S_h.ap())
        J_r = p_const.tile([128, 128], F16, tag="J")
        nc.sync.dma_start(J_r[:], J_h.ap())
        id_r = p_const.tile([128, 128], F16, tag="id")
        nc.sync.dma_start(id_r[:], id_h.ap())
        dmask = p_const.tile([128, 4 * 512], F16, tag="dm")
        nc.sync.dma_start(dmask[:], dm_h.ap())
        n1w = p_const.tile([128, L * KH], F32, tag="n1w")
        nc.sync.dma_start(n1w[:], n1w_h.ap())
        n2w = p_const.tile([128, L * KH], F32, tag="n2w")
        nc.sync.dma_start(n2w[:], n2w_h.ap())
        fw_s = p_const.tile([128, KH], F32, tag="fw")
        nc.sync.dma_start(fw_s[:], fw_h.ap())
        ones_mat = p_const.tile([128, 128], F16, tag="om")
        nc.vector.memset(ones_mat[:], 1.0)
        ones_col = p_const.tile([128, 1], F16, tag="o1")
        nc.vector.memset(ones_col[:], 1.0)
        eps_p = p_const.tile([128, 1], F32, tag="epsp")
        nc.vector.memset(eps_p[:], EPS)
        eps_t = p_const.tile([1, 1], F32, tag="eps")
        nc.vector.memset(eps_t[:], EPS)

        def load_wo(l_):
            wo_sb = p_wres.tile([128, 2 * H], F16, tag="wo", name="wosb")
            for fc in range(2):
                nc.sync.dma_start(wo_sb[:, ts(fc, H)],
                                  woT_h.ap()[l_, ts(fc, 128), :])
            return wo_sb

        def load_wqkv(l_):
            wq_sb = p_wres.tile([128, KH * 3 * FEAT], F16, tag="wqkv",
                                name="wqsb")
            for hc in range(KH):
                nc.sync.dma_start(wq_sb[:, ts(hc, 3 * FEAT)],
                                  wqkv_h.ap()[l_, ts(hc, 128), :])
            return wq_sb

        def norm_inv(w_tile, l_, tk):
            """[128,512] fp32 tile of 1/rms for tokens [tk*512, tk*512+512)."""
            nb_ps = psum.tile([128, 512], F32, tag="ps", name="nbps")
            for hc in range(KH):
                sq = p_ns.tile([128, 512], F16, tag="sq", name="sq")
                sl = slice(hc * S + tk * 512, hc * S + tk * 512 + 512)
                nc.vector.tensor_mul(sq[:], xT[:, sl], xT[:, sl])
                nc.tensor.matmul(nb_ps[:], ones_mat[:], sq[:],
                                 start=(hc == 0), stop=(hc == KH - 1))
            rms = p_nrm.tile([128, 512], F32, tag="rms", name="rms")
            nc.scalar.activation(rms[:], nb_ps[:], AF.Sqrt,
                                 bias=eps_p[:], scale=1.0 / H)
            inv = p_nrm.tile([128, 512], F32, tag="inv", name="inv")
            nc.vector.reciprocal_approx_fast(inv[:], rms[:])
            return inv

        def qkv_half(wq_sb, l_, tk, q_s, k_s, vT_s):
            """QKV for token half tk of layer l_ (writes [:, mt*S+tk*512]).

            Chunk-outer: 6 independent 16-matmul accumulation chains, one
            live PSUM each, so the PE queue never blocks on evictions.
            For the last layer, q is computed only for the last 2 tokens
            (tk==1) into q_s[:, mt*S + S-2 : mt*S + S]."""
            last = (l_ == L - 1)
            inv = norm_inv(n1w, l_, tk)
            xns = []
            for hc in range(KH):
                xn = p_hn.tile([128, 512], F16, tag="hn", name="xn")
                nc.vector.scalar_tensor_tensor(
                    xn[:], xT[:, hc * S + tk * 512: hc * S + tk * 512 + 512],
                    n1w[:, l_ * KH + hc: l_ * KH + hc + 1],
                    inv[:], op0=ALU.mult, op1=ALU.mult)
                xns.append(xn)
            # j: 0,1 = q heads; 2,3 = k heads; 4,5 = v heads
            for j in range(6):
                mt = j % 2
                if j < 2 and last:
                    if tk == 1:
                        ps = psum.tile([128, 2], F32, tag="ps", name="qLp")
                        for hc in range(KH):
                            nc.tensor.matmul(
                                ps[:], wq_sb[:, hc * 768 + j * 128:
                                             hc * 768 + j * 128 + 128],
                                xns[hc][:, 510:512],
                                start=(hc == 0), stop=(hc == KH - 1))
                        nc.vector.tensor_copy(
                            q_s[:, mt * S + S - 2: mt * S + S], ps[:])
                    continue
                ps = psum.tile([128, 512], F32, tag="ps", name="qkvp")
                for hc in range(KH):
                    nc.tensor.matmul(
                        ps[:], wq_sb[:, hc * 768 + j * 128:
                                     hc * 768 + j * 128 + 128],
                        xns[hc][:], start=(hc == 0), stop=(hc == KH - 1))
                dst = (q_s, q_s, k_s, k_s, vT_s, vT_s)[j]
                off = mt * S + tk * 512
                nc.vector.tensor_copy(dst[:, off:off + 512], ps[:])

        def rope_slice(t_s, col, width, ccol):
            """RoPE in place on t_s[:, col:col+width]; cos/sin cols at ccol."""
            j_ps = psum.tile([128, 512], F32, tag="ps", name="jps")
            nc.tensor.matmul(j_ps[:, :width], J_r[:], t_s[:, col:col + width],
                             start=True, stop=True)
            tmp = p_pt.tile([128, 512], F16, tag="rtmp", name="rtmp")
            nc.vector.tensor_mul(tmp[:, :width], C_s[:, ccol:ccol + width],
                                 t_s[:, col:col + width])
            nc.vector.tensor_mul(t_s[:, col:col + width], j_ps[:, :width],
                                 S_s[:, ccol:ccol + width])
            nc.vector.tensor_add(t_s[:, col:col + width],
                                 t_s[:, col:col + width], tmp[:, :width])

        def wo_project(wo_sb, attn_s, tk):
            """wo @ attn for half tk -> DRAM ar_in; RS+AG; returns ar_out."""
            ar_in = dram.tile([H, 512], F16, tag="arin", name="arin")
            ar_out = dram.tile([H, 512], F16, tag="arout",
                               addr_space="Shared", name="arout")
            for hc in range(KH):
                po = psum.tile([128, 512], F32, tag="ps", name="po")
                for fc in range(2):
                    nc.tensor.matmul(
                        po[:], wo_sb[:, fc * H + hc * 128: fc * H + hc * 128 + 128],
                        attn_s[:, fc * S + tk * 512: fc * S + tk * 512 + 512],
                        start=(fc == 0), stop=(fc == 1))
                ar_sb = p_ar.tile([128, 512], F16, tag="ar", name="arsb")
                nc.scalar.activation(ar_sb[:], po[:], AF.Copy)
                nc.sync.dma_start(ar_in[ts(hc, 128), :], ar_sb[:])
            ar_mid = dram.tile([H // NC, 512], F16, tag="armid", name="armid")
            nc.gpsimd.collective_compute(
                "ReduceScatter", ALU.add, replica_groups=[list(range(NC))],
                ins=[ar_in[:].opt()], outs=[ar_mid[:].opt()])
            nc.gpsimd.collective_compute(
                "AllGather", ALU.bypass, replica_groups=[list(range(NC))],
                ins=[ar_mid[:].opt()], outs=[ar_out[:].opt()])
            return ar_out

        def resid_add(ar_out, tk):
            for hc in range(KH):
                ar_t = p_ar.tile([128, 512], F16, tag="ar", name="art")
                nc.sync.dma_start(ar_t[:], ar_out[ts(hc, 128), :])
                sl = slice(hc * S + tk * 512, hc * S + tk * 512 + 512)
                nc.vector.tensor_add(xT[:, sl], xT[:, sl], ar_t[:])

        def prefetch_w13_pass(l_, pi):
            """Issue the 16 w13 stream loads for pass pi ahead of the
            collective-gated residual loads (avoids DMA-queue head-of-line
            blocking)."""
            tiles = []
            for hc in range(KH):
                t = p_w13.tile([128, 512], F16, tag="w13", name="w13pf")
                nc.sync.dma_start(t[:], w13_h.ap()[l_, ts(hc, 128),
                                                   pi * 512: pi * 512 + 512])
                tiles.append(t)
            return tiles

        def ffn_half(l_, tk, pre0=None):
            """norm2 + SwiGLU FFN + down proj for half tk; launches AR2."""
            inv = norm_inv(n2w, l_, tk)
            hns = []
            for hc in range(KH):
                hn = p_hn.tile([128, 512], F16, tag="hn", name="hn")
                nc.vector.scalar_tensor_tensor(
                    hn[:],
                    xT[:, hc * S + tk * 512: hc * S + tk * 512 + 512],
                    n2w[:, l_ * KH + hc: l_ * KH + hc + 1],
                    inv[:], op0=ALU.mult, op1=ALU.mult)
                hns.append(hn)
            gu_sb = p_gu.tile([128, 12 * 512], F16, tag="gu", name="gusb")
            for pi in range(3):   # passes of 4 chunks: 4 live PSUMs + slack
                pset = range(4 * pi, 4 * pi + 4)
                gus = {c: psum.tile([128, 512], F32, tag="ps", name=f"gu{c}")
                       for c in pset}
                tiles = pre0 if (pi == 0 and pre0) else None
                for hc in range(KH):
                    if tiles is not None:
                        w13_t = tiles[hc]
                    else:
                        w13_t = p_w13.tile([128, 512], F16, tag="w13",
                                           name="w13t")
                        nc.sync.dma_start(
                            w13_t[:], w13_h.ap()[l_, ts(hc, 128),
                                                 pi * 512: pi * 512 + 512])
                    st, sp = (hc == 0), (hc == KH - 1)
                    for ci, c in enumerate(pset):
                        nc.tensor.matmul(gus[c][:], w13_t[:, ts(ci, 128)],
                                         hns[hc][:], start=st, stop=sp)
                for c in pset:
                    nc.scalar.activation(gu_sb[:, ts(c, 512)], gus[c][:],
                                         AF.Copy)
            # swig[s] = silu(g[s]) * u[s]  (in place over g chunks 0..5)
            for sch in range(6):
                sg = p_ns.tile([128, 512], F16, tag="ns", name="sg")
                nc.scalar.activation(sg[:], gu_sb[:, ts(sch, 512)], AF.Silu)
                nc.vector.tensor_mul(gu_sb[:, ts(sch, 512)], sg[:],
                                     gu_sb[:, ts(6 + sch, 512)])
            # down projection
            ar2_in = dram.tile([H, 512], F16, tag="arin", name="ar2in")
            ar2_out = dram.tile([H, 512], F16, tag="arout",
                                addr_space="Shared", name="ar2out")
            for hcb in range(4):
                p2 = [psum.tile([128, 512], F32, tag="ps", name=f"p2{i}")
                      for i in range(4)]
                for kc in range(6):
                    w2_t = p_w2.tile([128, 512], F16, tag="w2", name="w2t")
                    nc.sync.dma_start(
                        w2_t[:], w2T_h.ap()[l_, ts(kc, 128),
                                            hcb * 512: hcb * 512 + 512])
                    for hh in range(4):
                        nc.tensor.matmul(p2[hh][:], w2_t[:, ts(hh, 128)],
                                         gu_sb[:, ts(kc, 512)],
                                         start=(kc == 0), stop=(kc == 5))
                for hh in range(4):
                    a2 = p_ar.tile([128, 512], F16, tag="ar", name="a2")
                    nc.scalar.activation(a2[:], p2[hh][:], AF.Copy)
                    nc.sync.dma_start(ar2_in[ts(hcb * 4 + hh, 128), :], a2[:])
            ar_mid = dram.tile([H // NC, 512], F16, tag="armid", name="ar2mid")
            nc.gpsimd.collective_compute(
                "ReduceScatter", ALU.add, replica_groups=[list(range(NC))],
                ins=[ar2_in[:].opt()], outs=[ar_mid[:].opt()])
            nc.gpsimd.collective_compute(
                "AllGather", ALU.bypass, replica_groups=[list(range(NC))],
                ins=[ar_mid[:].opt()], outs=[ar2_out[:].opt()])
            return ar2_out

        def rope_vtrans_half(q_s, k_s, vT_s, v_s, tk):
            for t_s in (q_s, k_s):
                for mt in range(2):
                    rope_slice(t_s, mt * S + tk * 512, 512, tk * 512)
            for mt in range(2):
                for tb in range(tk * 4, tk * 4 + 4):
                    tp = psum.tile([128, 128], F16, tag="ps", name="tp")
                    nc.tensor.transpose(
                        tp[:],
                        vT_s[:, mt * S + tb * 128: mt * S + tb * 128 + 128],
                        id_r[:])
                    nc.vector.tensor_copy(
                        v_s[:, tb * FEAT + mt * 128:
                            tb * FEAT + mt * 128 + 128], tp[:])

        def attn_half(q_s, k_s, v_s, attn_s, tk):
            """attention for half tk: only causally visible key blocks.

            The score matmul + exp for block kc+1 is emitted before the
            AV/sum matmuls of block kc so the PE never waits on the Exp."""
            nvis = (tk + 1) * 4

            def emit_sc(h, kc):
                sc_ps = psum.tile([128, 512], F32, tag="ps", name="scp")
                nc.tensor.matmul(
                    sc_ps[:],
                    k_s[:, h * S + kc * 128: h * S + kc * 128 + 128],
                    q_s[:, h * S + tk * 512: h * S + tk * 512 + 512],
                    start=True, stop=True)
                pt = p_pt.tile([128, 512], F16, tag="pt", name="ptl")
                nc.scalar.activation(pt[:], sc_ps[:], AF.Exp,
                                     scale=INV_SCALE)
                d = kc * 128 - tk * 512
                if d >= 0:
                    nc.vector.tensor_mul(
                        pt[:], pt[:], dmask[:, ts(d // 128, 512)])
                return pt

            for h in range(2):
                at_ps = psum.tile([128, 512], F32, tag="ps", name="atp")
                ib_ps = psum.tile([128, 512], F32, tag="ps", name="ibp")
                pt = emit_sc(h, 0)
                for kc in range(nvis):
                    pt_next = emit_sc(h, kc + 1) if kc + 1 < nvis else None
                    st, sp = (kc == 0), (kc == nvis - 1)
                    nc.tensor.matmul(
                        at_ps[:],
                        v_s[:, kc * FEAT + h * 128: kc * FEAT + h * 128 + 128],
                        pt[:], start=st, stop=sp)
                    nc.tensor.matmul(ib_ps[:], ones_mat[:], pt[:],
                                     start=st, stop=sp)
                    pt = pt_next
                inv_a = p_pt.tile([128, 512], F32, tag="pta", name="inva")
                nc.vector.reciprocal_approx_fast(inv_a[:], ib_ps[:])
                nc.vector.tensor_mul(
                    attn_s[:, h * S + tk * 512: h * S + tk * 512 + 512],
                    at_ps[:], inv_a[:])

        # ---- layer 0 prologue: only the A-half of QKV(0); the B-half is
        # computed inside layer 0 under AR1(A)'s shadow ----
        wo_sb = load_wo(0)
        wq_sb = load_wqkv(0)
        cur_q = p_big.tile([128, 2 * S], F16, tag="big", name="q0")
        cur_k = p_big.tile([128, 2 * S], F16, tag="big", name="k0")
        cur_vT = p_big.tile([128, 2 * S], F16, tag="big", name="vT0")
        qkv_half(wq_sb, 0, 0, cur_q, cur_k, cur_vT)
        ar2_prev_b = None

        for l in range(L):
            last = (l == L - 1)
            q_s, k_s, vT_s = cur_q, cur_k, cur_vT

            v_s = p_vs.tile([128, 8 * FEAT], F16, tag="v", name="vs")
            attn_s = p_attn.tile([128, 2 * S], F16, tag="attn", name="attn")

            if last:
                # B-half QKV of the last layer (k/v all tokens, q last-2)
                if ar2_prev_b is not None:
                    resid_add(ar2_prev_b, 1)
                qkv_half(wq_sb, l, 1, q_s, k_s, vT_s)
                # prefetch the last-layer ffn weight stream into SBUF while
                # the serial attention/AR tail runs (PE is mostly idle here)
                preL = [prefetch_w13_pass(l, 0)]
                preL1 = []
                for hc in range(KH):
                    t = p_hn.tile([128, 512], F16, tag="hn", name="w13pfL")
                    nc.sync.dma_start(t[:], w13_h.ap()[l, ts(hc, 128),
                                                       512: 1024])
                    preL1.append(t)
                preL.append(preL1)
                # RoPE on k (all tokens) and q (last 2 only)
                for mt in range(2):
                    for n in range(2):
                        rope_slice(k_s, mt * S + n * 512, 512, n * 512)
                    rope_slice(q_s, mt * S + S - 2, 2, S - 2)
                # V -> [tok, feat] via PE transpose
                for mt in range(2):
                    for tb in range(8):
                        tp = psum.tile([128, 128], F16, tag="ps", name="tp")
                        nc.tensor.transpose(
                            tp[:],
                            vT_s[:, mt * S + tb * 128: mt * S + tb * 128 + 128],
                            id_r[:])
                        nc.vector.tensor_copy(
                            v_s[:, tb * FEAT + mt * 128:
                                tb * FEAT + mt * 128 + 128], tp[:])
                # attention for the last 2 tokens only
                for h in range(2):
                    at1 = psum.tile([128, 2], F32, tag="ps", name="at1")
                    ib1 = psum.tile([128, 2], F32, tag="ps", name="ib1")
                    for kc in range(8):
                        sc1 = psum.tile([128, 2], F32, tag="ps", name="sc1")
                        nc.tensor.matmul(
                            sc1[:],
                            k_s[:, h * S + kc * 128: h * S + kc * 128 + 128],
                            q_s[:, h * S + S - 2: h * S + S],
                            start=True, stop=True)
                        pt1 = p_pt.tile([128, 2], F16, tag="pt1", name="pt1")
                        nc.scalar.activation(pt1[:], sc1[:], AF.Exp,
                                             scale=INV_SCALE)
                        if kc == 7:
                            nc.vector.tensor_mul(
                                pt1[:], pt1[:],
                                dmask[:, 3 * 512 + 510: 3 * 512 + 512])
                        st, sp = (kc == 0), (kc == 7)
                        nc.tensor.matmul(
                            at1[:],
                            v_s[:, kc * FEAT + h * 128: kc * FEAT + h * 128 + 128],
                            pt1[:], start=st, stop=sp)
                        nc.tensor.matmul(ib1[:], ones_mat[:], pt1[:],
                                         start=st, stop=sp)
                    inva = p_pt.tile([128, 2], F32, tag="pta", name="inva")
                    nc.vector.reciprocal_approx_fast(inva[:], ib1[:])
                    nc.vector.tensor_mul(
                        attn_s[:, h * S + S - 2: h * S + S], at1[:], inva[:])

                # wo -> [H,2] AllReduce -> residual add (last 2 tokens)
                ar_in = dram.tile([H, 2], F16, tag="arinL", name="arinL")
                ar_out = dram.tile([H, 2], F16, tag="aroutL",
                                   addr_space="Shared", name="aroutL")
                for hc in range(KH):
                    poL = psum.tile([128, 2], F32, tag="ps", name="poL")
                    for fc in range(2):
                        nc.tensor.matmul(
                            poL[:],
                            wo_sb[:, fc * H + hc * 128: fc * H + hc * 128 + 128],
                            attn_s[:, fc * S + S - 2: fc * S + S],
                            start=(fc == 0), stop=(fc == 1))
                    arL = p_pt.tile([128, 2], F16, tag="arL", name="arL")
                    nc.scalar.activation(arL[:], poL[:], AF.Copy)
                    nc.sync.dma_start(ar_in[ts(hc, 128), :], arL[:])
                nc.gpsimd.collective_compute(
                    "AllReduce", ALU.add, replica_groups=[list(range(NC))],
                    ins=[ar_in[:].opt()], outs=[ar_out[:].opt()])
                for hc in range(KH):
                    ar_t = p_pt.tile([128, 2], F16, tag="arL", name="art")
                    nc.sync.dma_start(ar_t[:], ar_out[ts(hc, 128), :])
                    nc.vector.tensor_add(
                        xT[:, hc * S + S - 2: hc * S + S],
                        xT[:, hc * S + S - 2: hc * S + S], ar_t[:])

                # norm2 + FFN on the last 2 tokens
                sqL = p_row.tile([128, 2 * KH], F16, tag="sql2")
                for hc in range(KH):
                    col = hc * S + S - 2
                    nc.vector.tensor_mul(sqL[:, 2 * hc:2 * hc + 2],
                                         xT[:, col:col + 2], xT[:, col:col + 2])
                ssL = psum.tile([128, 2 * KH], F32, tag="ps", name="ssL")
                nc.tensor.matmul(ssL[:], ones_mat[:], sqL[:],
                                 start=True, stop=True)
                ssr = p_row.tile([128, 2], F32, tag="ssr")
                nc.vector.reduce_sum(
                    ssr[:], ssL[:].rearrange("p (c two) -> p two c", two=2),
                    axis=mybir.AxisListType.X)
                rmsL = p_row.tile([128, 2], F32, tag="rmsL")
                nc.scalar.activation(rmsL[:], ssr[:], AF.Sqrt,
                                     bias=eps_p[:], scale=1.0 / H)
                invL = p_row.tile([128, 2], F32, tag="invLc")
                nc.vector.reciprocal_approx_fast(invL[:], rmsL[:])
                hnL = p_row.tile([128, 2 * KH], F16, tag="hnL")
                tnL = p_row.tile([128, 2], F32, tag="tnL")
                for hc in range(KH):
                    col = hc * S + S - 2
                    nc.vector.tensor_scalar_mul(
                        tnL[:], xT[:, col:col + 2],
                        n2w[:, l * KH + hc: l * KH + hc + 1])
                    nc.vector.tensor_mul(hnL[:, 2 * hc:2 * hc + 2],
                                         tnL[:], invL[:])
                guL = p_row.tile([128, 12 * 2], F16, tag="guL")
                for pi in range(3):
                    pset = range(4 * pi, 4 * pi + 4)
                    gps = {c: psum.tile([128, 2], F32, tag="ps",
                                        name=f"gL{c}") for c in pset}
                    for hc in range(KH):
                        if pi < 2:
                            w13_t = preL[pi][hc]
                        else:
                            w13_t = p_w13.tile([128, 512], F16, tag="w13",
                                               name="w13tL")
                            nc.sync.dma_start(
                                w13_t[:], w13_h.ap()[l, ts(hc, 128),
                                                     pi * 512: pi * 512 + 512])
                        st, sp = (hc == 0), (hc == KH - 1)
                        for ci, c in enumerate(pset):
                            nc.tensor.matmul(
                                gps[c][:], w13_t[:, ts(ci, 128)],
                                hnL[:, 2 * hc:2 * hc + 2], start=st, stop=sp)
                    for c in pset:
                        nc.scalar.activation(guL[:, 2 * c:2 * c + 2],
                                             gps[c][:], AF.Copy)
                swL = p_row.tile([128, 6 * 2], F16, tag="swL")
                for sch in range(6):
                    sgL = p_row.tile([128, 2], F16, tag="sgL")
                    nc.scalar.activation(sgL[:], guL[:, 2 * sch:2 * sch + 2],
                                         AF.Silu)
                    nc.vector.tensor_mul(swL[:, 2 * sch:2 * sch + 2], sgL[:],
                                         guL[:, 2 * (6 + sch):2 * (6 + sch) + 2])
                ar2_in = dram.tile([H, 2], F16, tag="arinL", name="ar2inL")
                ar2_out = dram.tile([H, 2], F16, tag="aroutL",
                                    addr_space="Shared", name="ar2outL")
                for hc in range(KH):
                    p2L = psum.tile([128, 2], F32, tag="ps", name="p2L")
                    for kc in range(6):
                        w2_t = p_w2.tile([128, 128], F16, tag="w2L",
                                         name="w2tL")
                        nc.sync.dma_start(
                            w2_t[:], w2T_h.ap()[l, ts(kc, 128), ts(hc, 128)])
                        nc.tensor.matmul(p2L[:], w2_t[:],
                                         swL[:, 2 * kc:2 * kc + 2],
                                         start=(kc == 0), stop=(kc == 5))
                    a2L = p_pt.tile([128, 2], F16, tag="arL", name="a2L")
                    nc.scalar.activation(a2L[:], p2L[:], AF.Copy)
                    nc.sync.dma_start(ar2_in[ts(hc, 128), :], a2L[:])
                nc.gpsimd.collective_compute(
                    "AllReduce", ALU.add, replica_groups=[list(range(NC))],
                    ins=[ar2_in[:].opt()], outs=[ar2_out[:].opt()])
                for hc in range(KH):
                    ar_t = p_pt.tile([128, 2], F16, tag="arL", name="art2")
                    nc.sync.dma_start(ar_t[:], ar2_out[ts(hc, 128), :])
                    nc.vector.tensor_add(
                        xT[:, hc * S + S - 2: hc * S + S],
                        xT[:, hc * S + S - 2: hc * S + S], ar_t[:])
                continue

            # ---- non-last layer: A-half attention first, then the B-half
            # QKV runs in AR1(A)'s shadow ----
            rope_vtrans_half(q_s, k_s, vT_s, v_s, 0)
            attn_half(q_s, k_s, v_s, attn_s, 0)
            ar1_a = wo_project(wo_sb, attn_s, 0)

            if ar2_prev_b is not None:
                resid_add(ar2_prev_b, 1)
            qkv_half(wq_sb, l, 1, q_s, k_s, vT_s)

            rope_vtrans_half(q_s, k_s, vT_s, v_s, 1)
            attn_half(q_s, k_s, v_s, attn_s, 1)
            ar1_b = wo_project(wo_sb, attn_s, 1)

            # prefetch next layer's qkv weights (slot free: qkv(l) done)
            wq_sb = load_wqkv(l + 1)

            pre_a = prefetch_w13_pass(l, 0)
            resid_add(ar1_a, 0)
            ar2_a = ffn_half(l, 0, pre_a)
            pre_b = prefetch_w13_pass(l, 0)
            resid_add(ar1_b, 1)
            ar2_prev_b = ffn_half(l, 1, pre_b)

            # next layer's wo
            wo_sb = load_wo(l + 1)

            nxt_q = p_big.tile([128, 2 * S], F16, tag="big", name="qn")
            nxt_k = p_big.tile([128, 2 * S], F16, tag="big", name="kn")
            nxt_vT = p_big.tile([128, 2 * S], F16, tag="big", name="vTn")
            resid_add(ar2_a, 0)
            qkv_half(wq_sb, l + 1, 0, nxt_q, nxt_k, nxt_vT)
            cur_q, cur_k, cur_vT = nxt_q, nxt_k, nxt_vT

        # ======== final norm (last token only) + logits ========
        sq_l = p_row.tile([128, KH], F16, tag="sql")
        for hc in range(KH):
            col = hc * S + S - 1
            nc.vector.tensor_mul(sq_l[:, hc:hc + 1], xT[:, col:col + 1],
                                 xT[:, col:col + 1])
        sl_ps = psum.tile([1, KH], F32, tag="ps", name="slps")
        nc.tensor.matmul(sl_ps[:], ones_col[:], sq_l[:], start=True, stop=True)
        ssc = p_row.tile([1, 1], F32, tag="ssc")
        nc.vector.reduce_sum(ssc[:], sl_ps[:], axis=mybir.AxisListType.X)
        rms_l = p_row.tile([1, 1], F32, tag="rmsl")
        nc.scalar.activation(rms_l[:], ssc[:], AF.Sqrt, bias=eps_t[:],
                             scale=1.0 / H)
        inv_l = p_row.tile([1, 1], F32, tag="invl")
        nc.vector.reciprocal(inv_l[:], rms_l[:])
        xnl = p_row.tile([128, KH], F16, tag="xnl")
        for hc in range(KH):
            col = hc * S + S - 1
            nc.vector.tensor_mul(xnl[:, hc:hc + 1], xT[:, col:col + 1],
                                 fw_s[:, hc:hc + 1])
        for n in range(4):
            lg_a = psum.tile([1, 500], F32, tag="ps", name="lga")
            lg_b = psum.tile([1, 500], F32, tag="ps", name="lgb")
            for hc in range(KH):
                ow_t = p_ow.tile([128, 1000], F16, tag="ow", name="owt")
                nc.sync.dma_start(
                    ow_t[:], owT_h.ap()[ts(hc, 128), n * 1000: n * 1000 + 1000])
                st, sp = (hc == 0), (hc == KH - 1)
                nc.tensor.matmul(lg_a[:], xnl[:, hc: hc + 1], ow_t[:, :500],
                                 start=st, stop=sp)
                nc.tensor.matmul(lg_b[:], xnl[:, hc: hc + 1], ow_t[:, 500:],
                                 start=st, stop=sp)
            lg = p_row.tile([1, 1000], F32, tag="lg")
            nc.scalar.activation(lg[:, :500], lg_a[:], AF.Copy, scale=inv_l[:])
            nc.scalar.activation(lg[:, 500:], lg_b[:], AF.Copy, scale=inv_l[:])
            nc.sync.dma_start(out_h.ap()[:, n * 1000: n * 1000 + 1000], lg[:])

    nc.compile()
    return nc


def _shard(inputs):
    import ml_dtypes
    f16 = ml_dtypes.bfloat16
    x = np.asarray(inputs["x"], np.float32)
    cos = np.asarray(inputs["cos"], np.float32).reshape(S, HD // 2)
    sin = np.asarray(inputs["sin"], np.float32).reshape(S, HD // 2)
    n1 = np.asarray(inputs["norm1_w"], np.float32)[:L]
    n2 = np.asarray(inputs["norm2_w"], np.float32)[:L]
    fw = np.asarray(inputs["final_norm_w"], np.float32)
    wq = np.asarray(inputs["wq"], np.float32)[:L]
    wk = np.asarray(inputs["wk"], np.float32)[:L]
    wv = np.asarray(inputs["wv"], np.float32)[:L]
    wo = np.asarray(inputs["wo"], np.float32)[:L]
    w1 = np.asarray(inputs["w1"], np.float32)[:L]
    w3 = np.asarray(inputs["w3"], np.float32)[:L]
    w2 = np.asarray(inputs["w2"], np.float32)[:L]
    ow = np.asarray(inputs["out_w"], np.float32)

    xT = np.ascontiguousarray(x[0].T).astype(f16)
    C = np.empty((128, S), np.float32)
    C[0::2] = cos.T
    C[1::2] = cos.T
    Sm = np.empty((128, S), np.float32)
    Sm[0::2] = -sin.T
    Sm[1::2] = sin.T
    J = np.zeros((128, 128), np.float32)
    idx = np.arange(0, 128, 2)
    J[idx, idx + 1] = 1.0
    J[idx + 1, idx] = 1.0
    ident = np.eye(128, dtype=np.float32)
    # diagonal causal masks: pattern di (block offset di*128):
    # mask[kp, q] = 1 if q >= kp + di*128
    dm = np.zeros((128, 4 * 512), np.float32)
    kp = np.arange(128)[:, None]
    qq = np.arange(512)[None, :]
    for di in range(4):
        dm[:, di * 512:(di + 1) * 512] = (qq >= kp + di * 128)
    n1w = np.ascontiguousarray(
        n1.reshape(L, KH, 128).transpose(2, 0, 1).reshape(128, L * KH))
    n2w = np.ascontiguousarray(
        n2.reshape(L, KH, 128).transpose(2, 0, 1).reshape(128, L * KH))
    fwh = np.ascontiguousarray(fw.reshape(KH, 128).T)

    common = dict(xT=xT, Cr=C.astype(f16), Sr=Sm.astype(f16),
                  J=J.astype(f16), ident=ident.astype(f16),
                  dmask=dm.astype(f16), n1w=n1w, n2w=n2w, fw=fwh)
    in_maps = []
    for c in range(NC):
        fs = slice(c * FEAT, (c + 1) * FEAT)
        ps = slice(c * PC, (c + 1) * PC)
        vs = slice(c * VC, (c + 1) * VC)
        m = dict(common)
        wqT = wq[:, fs, :].transpose(0, 2, 1)
        wkT = wk[:, fs, :].transpose(0, 2, 1)
        wvT = wv[:, fs, :].transpose(0, 2, 1)
        m["wqkvT"] = np.ascontiguousarray(
            np.concatenate([wqT, wkT, wvT], axis=2)).astype(f16)
        m["woT"] = np.ascontiguousarray(
            wo[:, :, fs].transpose(0, 2, 1)).astype(f16)
        w1T = w1[:, ps, :].transpose(0, 2, 1)   # [L, H, PC]
        w3T = w3[:, ps, :].transpose(0, 2, 1)
        pad = np.zeros((L, H, PCP - PC), np.float32)
        m["w13T"] = np.ascontiguousarray(np.concatenate(
            [w1T, pad, w3T, pad], axis=2)).astype(f16)
        w2p = np.zeros((L, PCP, H), np.float32)
        w2p[:, :PC, :] = w2[:, :, ps].transpose(0, 2, 1)
        m["w2T"] = np.ascontiguousarray(w2p).astype(f16)
        m["owT"] = np.ascontiguousarray(ow[vs, :].T).astype(f16)
        in_maps.append(m)
    return in_maps


def kernel(**inputs) -> np.ndarray:
    from concourse import bass_utils

    if "nc" not in _STATE:
        _STATE["nc"] = _build()
    in_maps = _shard(inputs)
    res = bass_utils.run_bass_kernel_spmd(
        _STATE["nc"], in_maps, core_ids=list(range(NC)))
    out = np.concatenate(
        [res.results[c]["logits"] for c in range(NC)], axis=1)
    return out.astype(np.float32)


# revision 27
# speedup vs baseline: 2.2087x; 1.2777x over previous
"""Trainium2 Bass kernel: 4-layer decoder prefill (S=1024, H=2048, NH=16, HD=128,
FFN=5632, V=32000), tensor-parallel over 8 NeuronCores.

- Megatron TP over 8 cores: wq/wk/wv/w1/w3 sharded on output dim (2 heads /
  704 ffn rows per core), wo/w2 sharded on input dim (partials ->
  ReduceScatter+AllGather), out_w sharded over vocab (4000 rows/core); only
  the last token's logits are computed.
- All matmuls in bf16 (weights pre-cast on host, fp32 accumulation in PSUM);
  the residual stream lives TRANSPOSED in SBUF as bf16 (xT: [H on
  partition-chunks, S free]).
- Causal structure exploited: fully-masked score blocks are skipped; diagonal
  blocks use 4 precomputed multiplicative 0/1 mask tiles; 1/sqrt(HD) is
  folded into the Exp activation scale.
- Softmax denominators and rms-norm sums are accumulated as PE matmuls with
  an all-ones [128,128] stationary, which broadcasts the partition-sum to all
  128 partitions directly -- no slow [1,N] single-partition ops; inverses via
  reciprocal_approx_fast (single DVE op).
- wqkv/wo are SBUF-resident per layer; w13/w2 streamed; ffn w1|w3 are
  zero-padded to 768 rows each so all chunks are full 128 partitions.
- Last layer: k/v for all tokens but q/attention/FFN only for the last
  tokens; logits bf16 GEMV streamed over the vocab shard.
"""

import os
import sys

sys.path.insert(0, "/opt/trn_rl_repo")

import numpy as np

L = 4
B, S, H, NH, HD = 1, 1024, 2048, 16, 128
V, P = 32000, 5632
NC = 8
FEAT = H // NC          # 256 q/k/v features per core (2 heads)
PC = P // NC            # 704 ffn rows per core
PCP = 768               # padded to 6 full 128-chunks
VC = V // NC            # 4000 vocab rows per core
KH = H // 128           # 16 H-chunks
EPS = 1e-5
SCALE = float(np.sqrt(HD))
INV_SCALE = 1.0 / SCALE

_STATE = {}


def _build():
    import concourse.bass as bass
    import concourse.bacc as bacc
    from concourse import tile, mybir

    F32 = mybir.dt.float32
    F16 = mybir.dt.bfloat16  # bf16: native PE rate
    AF = mybir.ActivationFunctionType
    ALU = mybir.AluOpType
    ts = bass.ts

    nc = bacc.Bacc("TRN2", target_bir_lowering=False, debug=False, num_devices=NC)

    xT_h = nc.dram_tensor("xT", [H, S], F16, kind="ExternalInput")
    C_h = nc.dram_tensor("Cr", [128, S], F16, kind="ExternalInput")
    S_h = nc.dram_tensor("Sr", [128, S], F16, kind="ExternalInput")
    J_h = nc.dram_tensor("J", [128, 128], F16, kind="ExternalInput")
    id_h = nc.dram_tensor("ident", [128, 128], F16, kind="ExternalInput")
    dm_h = nc.dram_tensor("dmask", [128, 4 * 512], F16, kind="ExternalInput")
    # wq|wk|wv concatenated on the last axis: [L, H, 3*FEAT]
    wqkv_h = nc.dram_tensor("wqkvT", [L, H, 3 * FEAT], F16, kind="ExternalInput")
    woT_h = nc.dram_tensor("woT", [L, FEAT, H], F16, kind="ExternalInput")
    # [w1 | 64pad | w3 | 64pad] on cols: [L, H, 2*PCP]
    w13_h = nc.dram_tensor("w13T", [L, H, 2 * PCP], F16, kind="ExternalInput")
    w2T_h = nc.dram_tensor("w2T", [L, PCP, H], F16, kind="ExternalInput")
    owT_h = nc.dram_tensor("owT", [H, VC], F16, kind="ExternalInput")
    out_h = nc.dram_tensor("logits", [1, VC], F32, kind="ExternalOutput")

    from contextlib import ExitStack

    with tile.TileContext(nc) as tc, ExitStack() as _ctx:
        ec = _ctx.enter_context
        p_resid = ec(tc.tile_pool(name="resid", bufs=1))
        p_const = ec(tc.tile_pool(name="consts", bufs=1))
        p_big = ec(tc.tile_pool(name="big", bufs=3))
        p_vs = ec(tc.tile_pool(name="vsn", bufs=2))
        p_attn = ec(tc.tile_pool(name="attnp", bufs=2))
        p_pt = ec(tc.tile_pool(name="ptile", bufs=3))
        p_ns = ec(tc.tile_pool(name="normsc", bufs=3))
        p_hn = ec(tc.tile_pool(name="hnp", bufs=17))
        p_nrm = ec(tc.tile_pool(name="nrm", bufs=2))
        p_gu = ec(tc.tile_pool(name="gup", bufs=1))
        p_wres = ec(tc.tile_pool(name="wres", bufs=1))
        p_w13 = ec(tc.tile_pool(name="w13p", bufs=19))
        p_w2 = ec(tc.tile_pool(name="w2p", bufs=4))
        p_ow = ec(tc.tile_pool(name="owp", bufs=6))
        p_ar = ec(tc.tile_pool(name="ars", bufs=6))
        p_row = ec(tc.tile_pool(name="row", bufs=2))
        psum = ec(tc.tile_pool(name="psum", bufs=7, space="PSUM"))
        dram = ec(tc.tile_pool(name="dram", bufs=4, space="DRAM"))

        # ---- constants / inputs ----
        xT = p_resid.tile([128, KH * S], F16, tag="xT")
        for hc in range(KH):
            nc.sync.dma_start(xT[:, ts(hc, S)], xT_h.ap()[ts(hc, 128), :])

        C_s = p_const.tile([128, S], F16, tag="C")
        nc.sync.dma_start(C_s[:], C_h.ap())
        S_s = p_const.tile([128, S], F16, tag="S")
        nc.sync.dma_start(S_s[:], S_h.ap())
        J_r = p_const.tile([128, 128], F16, tag="J")
        nc.sync.dma_start(J_r[:], J_h.ap())
        id_r = p_const.tile([128, 128], F16, tag="id")
        nc.sync.dma_start(id_r[:], id_h.ap())
        dmask = p_const.tile([128, 4 * 512], F16, tag="dm")
        nc.sync.dma_start(dmask[:], dm_h.ap())
        ones_mat = p_const.tile([128, 128], F16, tag="om")
        nc.vector.memset(ones_mat[:], 1.0)
        ones_col = p_const.tile([128, 1], F16, tag="o1")
        nc.vector.memset(ones_col[:], 1.0)
        eps_p = p_const.tile([128, 1], F32, tag="epsp")
        nc.vector.memset(eps_p[:], EPS)
        eps_t = p_const.tile([1, 1], F32, tag="eps")
        nc.vector.memset(eps_t[:], EPS)

        def load_wo(l_):
            wo_sb = p_wres.tile([128, 2 * H], F16, tag="wo", name="wosb")
            for fc in range(2):
                nc.sync.dma_start(wo_sb[:, ts(fc, H)],
                                  woT_h.ap()[l_, ts(fc, 128), :])
            return wo_sb

        def load_wqkv(l_):
            wq_sb = p_wres.tile([128, KH * 3 * FEAT], F16, tag="wqkv",
                                name="wqsb")
            for hc in range(KH):
                nc.sync.dma_start(wq_sb[:, ts(hc, 3 * FEAT)],
                                  wqkv_h.ap()[l_, ts(hc, 128), :])
            return wq_sb

        def norm_inv(tk):
            """[128,512] fp32 tile of 1/rms for tokens [tk*512, tk*512+512)."""
            nb_ps = psum.tile([128, 512], F32, tag="ps", name="nbps")
            for hc in range(KH):
                sq = p_ns.tile([128, 512], F16, tag="sq", name="sq")
                sl = slice(hc * S + tk * 512, hc * S + tk * 512 + 512)
                nc.vector.tensor_mul(sq[:], xT[:, sl], xT[:, sl])
                nc.tensor.matmul(nb_ps[:], ones_mat[:], sq[:],
                                 start=(hc == 0), stop=(hc == KH - 1))
            rms = p_nrm.tile([128, 512], F32, tag="rms", name="rms")
            nc.scalar.activation(rms[:], nb_ps[:], AF.Sqrt,
                                 bias=eps_p[:], scale=1.0 / H)
            inv = p_nrm.tile([128, 512], F32, tag="inv", name="inv")
            nc.vector.reciprocal_approx_fast(inv[:], rms[:])
            return inv

        def qkv_half(wq_sb, l_, tk, q_s, k_s, vT_s):
            """QKV for token half tk of layer l_ (writes [:, mt*S+tk*512]).

            Chunk-outer: 6 independent 16-matmul accumulation chains, one
            live PSUM each, so the PE queue never blocks on evictions.
            For the last layer, q is computed only for the last 2 tokens
            (tk==1) into q_s[:, mt*S + S-2 : mt*S + S]."""
            last = (l_ == L - 1)
            inv = norm_inv(tk)
            # norm weights are folded into the weights on the host, so the
            # matmul chains read raw xT; 1/rms is applied at PSUM eviction.
            # j: 0,1 = q heads; 2,3 = k heads; 4,5 = v heads
            for j in range(6):
                mt = j % 2
                if j < 2 and last:
                    if tk == 1:
                        ps = psum.tile([128, 2], F32, tag="ps", name="qLp")
                        for hc in range(KH):
                            nc.tensor.matmul(
                                ps[:], wq_sb[:, hc * 768 + j * 128:
                                             hc * 768 + j * 128 + 128],
                                xT[:, hc * S + S - 2: hc * S + S],
                                start=(hc == 0), stop=(hc == KH - 1))
                        nc.vector.tensor_mul(
                            q_s[:, mt * S + S - 2: mt * S + S], ps[:],
                            inv[:, 510:512])
                    continue
                ps = psum.tile([128, 512], F32, tag="ps", name="qkvp")
                for hc in range(KH):
                    nc.tensor.matmul(
                        ps[:], wq_sb[:, hc * 768 + j * 128:
                                     hc * 768 + j * 128 + 128],
                        xT[:, hc * S + tk * 512: hc * S + tk * 512 + 512],
                        start=(hc == 0), stop=(hc == KH - 1))
                dst = (q_s, q_s, k_s, k_s, vT_s, vT_s)[j]
                off = mt * S + tk * 512
                nc.vector.tensor_mul(dst[:, off:off + 512], ps[:], inv[:])

        def rope_slice(t_s, col, width, ccol):
            """RoPE in place on t_s[:, col:col+width]; cos/sin cols at ccol."""
            j_ps = psum.tile([128, 512], F32, tag="ps", name="jps")
            nc.tensor.matmul(j_ps[:, :width], J_r[:], t_s[:, col:col + width],
                             start=True, stop=True)
            tmp = p_pt.tile([128, 512], F16, tag="rtmp", name="rtmp")
            nc.vector.tensor_mul(tmp[:, :width], C_s[:, ccol:ccol + width],
                                 t_s[:, col:col + width])
            nc.vector.tensor_mul(t_s[:, col:col + width], j_ps[:, :width],
                                 S_s[:, ccol:ccol + width])
            nc.vector.tensor_add(t_s[:, col:col + width],
                                 t_s[:, col:col + width], tmp[:, :width])

        def wo_project(wo_sb, attn_s, tk):
            """wo @ attn for half tk -> DRAM ar_in; RS+AG; returns ar_out."""
            ar_in = dram.tile([H, 512], F16, tag="arin", name="arin")
            ar_out = dram.tile([H, 512], F16, tag="arout",
                               addr_space="Shared", name="arout")
            for hc in range(KH):
                po = psum.tile([128, 512], F32, tag="ps", name="po")
                for fc in range(2):
                    nc.tensor.matmul(
                        po[:], wo_sb[:, fc * H + hc * 128: fc * H + hc * 128 + 128],
                        attn_s[:, fc * S + tk * 512: fc * S + tk * 512 + 512],
                        start=(fc == 0), stop=(fc == 1))
                ar_sb = p_ar.tile([128, 512], F16, tag="ar", name="arsb")
                nc.scalar.activation(ar_sb[:], po[:], AF.Copy)
                nc.sync.dma_start(ar_in[ts(hc, 128), :], ar_sb[:])
            ar_mid = dram.tile([H // NC, 512], F16, tag="armid", name="armid")
            nc.gpsimd.collective_compute(
                "ReduceScatter", ALU.add, replica_groups=[list(range(NC))],
                ins=[ar_in[:].opt()], outs=[ar_mid[:].opt()])
            nc.gpsimd.collective_compute(
                "AllGather", ALU.bypass, replica_groups=[list(range(NC))],
                ins=[ar_mid[:].opt()], outs=[ar_out[:].opt()])
            return ar_out

        def resid_add(ar_out, tk):
            for hc in range(KH):
                ar_t = p_ar.tile([128, 512], F16, tag="ar", name="art")
                nc.sync.dma_start(ar_t[:], ar_out[ts(hc, 128), :])
                sl = slice(hc * S + tk * 512, hc * S + tk * 512 + 512)
                nc.vector.tensor_add(xT[:, sl], xT[:, sl], ar_t[:])

        def prefetch_w13_pass(l_, pi):
            """Issue the 16 w13 stream loads for pass pi ahead of the
            collective-gated residual loads (avoids DMA-queue head-of-line
            blocking)."""
            tiles = []
            for hc in range(KH):
                t = p_w13.tile([128, 512], F16, tag="w13", name="w13pf")
                nc.sync.dma_start(t[:], w13_h.ap()[l_, ts(hc, 128),
                                                   pi * 512: pi * 512 + 512])
                tiles.append(t)
            return tiles

        def ffn_half(l_, tk, pre0=None):
            """norm2 + SwiGLU FFN + down proj for half tk; launches AR2."""
            inv = norm_inv(tk)
            gu_sb = p_gu.tile([128, 12 * 512], F16, tag="gu", name="gusb")
            for pi in range(3):   # passes of 4 chunks: 4 live PSUMs + slack
                pset = range(4 * pi, 4 * pi + 4)
                gus = {c: psum.tile([128, 512], F32, tag="ps", name=f"gu{c}")
                       for c in pset}
                tiles = pre0 if (pi == 0 and pre0) else None
                for hc in range(KH):
                    if tiles is not None:
                        w13_t = tiles[hc]
                    else:
                        w13_t = p_w13.tile([128, 512], F16, tag="w13",
                                           name="w13t")
                        nc.sync.dma_start(
                            w13_t[:], w13_h.ap()[l_, ts(hc, 128),
                                                 pi * 512: pi * 512 + 512])
                    st, sp = (hc == 0), (hc == KH - 1)
                    for ci, c in enumerate(pset):
                        nc.tensor.matmul(
                            gus[c][:], w13_t[:, ts(ci, 128)],
                            xT[:, hc * S + tk * 512: hc * S + tk * 512 + 512],
                            start=st, stop=sp)
                for c in pset:
                    nc.vector.tensor_mul(gu_sb[:, ts(c, 512)], gus[c][:],
                                         inv[:])
            # swig[s] = silu(g[s]) * u[s]  (in place over g chunks 0..5)
            for sch in range(6):
                sg = p_ns.tile([128, 512], F16, tag="ns", name="sg")
                nc.scalar.activation(sg[:], gu_sb[:, ts(sch, 512)], AF.Silu)
                nc.vector.tensor_mul(gu_sb[:, ts(sch, 512)], sg[:],
                                     gu_sb[:, ts(6 + sch, 512)])
            # down projection
            ar2_in = dram.tile([H, 512], F16, tag="arin", name="ar2in")
            ar2_out = dram.tile([H, 512], F16, tag="arout",
                                addr_space="Shared", name="ar2out")
            for hcb in range(4):
                p2 = [psum.tile([128, 512], F32, tag="ps", name=f"p2{i}")
                      for i in range(4)]
                for kc in range(6):
                    w2_t = p_w2.tile([128, 512], F16, tag="w2", name="w2t")
                    nc.sync.dma_start(
                        w2_t[:], w2T_h.ap()[l_, ts(kc, 128),
                                            hcb * 512: hcb * 512 + 512])
                    for hh in range(4):
                        nc.tensor.matmul(p2[hh][:], w2_t[:, ts(hh, 128)],
                                         gu_sb[:, ts(kc, 512)],
                                         start=(kc == 0), stop=(kc == 5))
                for hh in range(4):
                    a2 = p_ar.tile([128, 512], F16, tag="ar", name="a2")
                    nc.scalar.activation(a2[:], p2[hh][:], AF.Copy)
                    nc.sync.dma_start(ar2_in[ts(hcb * 4 + hh, 128), :], a2[:])
            ar_mid = dram.tile([H // NC, 512], F16, tag="armid", name="ar2mid")
            nc.gpsimd.collective_compute(
                "ReduceScatter", ALU.add, replica_groups=[list(range(NC))],
                ins=[ar2_in[:].opt()], outs=[ar_mid[:].opt()])
            nc.gpsimd.collective_compute(
                "AllGather", ALU.bypass, replica_groups=[list(range(NC))],
                ins=[ar_mid[:].opt()], outs=[ar2_out[:].opt()])
            return ar2_out

        def rope_vtrans_half(q_s, k_s, vT_s, v_s, tk):
            for t_s in (q_s, k_s):
                for mt in range(2):
                    rope_slice(t_s, mt * S + tk * 512, 512, tk * 512)
            for mt in range(2):
                for tb in range(tk * 4, tk * 4 + 4):
                    tp = psum.tile([128, 128], F16, tag="ps", name="tp")
                    nc.tensor.transpose(
                        tp[:],
                        vT_s[:, mt * S + tb * 128: mt * S + tb * 128 + 128],
                        id_r[:])
                    nc.vector.tensor_copy(
                        v_s[:, tb * FEAT + mt * 128:
                            tb * FEAT + mt * 128 + 128], tp[:])

        def attn_half(q_s, k_s, v_s, attn_s, tk):
            """attention for half tk: only causally visible key blocks.

            The score matmul + exp for block kc+1 is emitted before the
            AV/sum matmuls of block kc so the PE never waits on the Exp."""
            nvis = (tk + 1) * 4

            def emit_sc(h, kc):
                sc_ps = psum.tile([128, 512], F32, tag="ps", name="scp")
                nc.tensor.matmul(
                    sc_ps[:],
                    k_s[:, h * S + kc * 128: h * S + kc * 128 + 128],
                    q_s[:, h * S + tk * 512: h * S + tk * 512 + 512],
                    start=True, stop=True)
                pt = p_pt.tile([128, 512], F16, tag="pt", name="ptl")
                nc.scalar.activation(pt[:], sc_ps[:], AF.Exp,
                                     scale=INV_SCALE)
                d = kc * 128 - tk * 512
                if d >= 0:
                    nc.vector.tensor_mul(
                        pt[:], pt[:], dmask[:, ts(d // 128, 512)])
                return pt

            for h in range(2):
                at_ps = psum.tile([128, 512], F32, tag="ps", name="atp")
                ib_ps = psum.tile([128, 512], F32, tag="ps", name="ibp")
                pt = emit_sc(h, 0)
                for kc in range(nvis):
                    pt_next = emit_sc(h, kc + 1) if kc + 1 < nvis else None
                    st, sp = (kc == 0), (kc == nvis - 1)
                    nc.tensor.matmul(
                        at_ps[:],
                        v_s[:, kc * FEAT + h * 128: kc * FEAT + h * 128 + 128],
                        pt[:], start=st, stop=sp)
                    nc.tensor.matmul(ib_ps[:], ones_mat[:], pt[:],
                                     start=st, stop=sp)
                    pt = pt_next
                inv_a = p_pt.tile([128, 512], F32, tag="pta", name="inva")
                nc.vector.reciprocal_approx_fast(inv_a[:], ib_ps[:])
                nc.vector.tensor_mul(
                    attn_s[:, h * S + tk * 512: h * S + tk * 512 + 512],
                    at_ps[:], inv_a[:])

        # ---- layer 0 prologue: only the A-half of QKV(0); the B-half is
        # computed inside layer 0 under AR1(A)'s shadow ----
        wo_sb = load_wo(0)
        wq_sb = load_wqkv(0)
        cur_q = p_big.tile([128, 2 * S], F16, tag="big", name="q0")
        cur_k = p_big.tile([128, 2 * S], F16, tag="big", name="k0")
        cur_vT = p_big.tile([128, 2 * S], F16, tag="big", name="vT0")
        qkv_half(wq_sb, 0, 0, cur_q, cur_k, cur_vT)
        ar2_prev_b = None

        for l in range(L):
            last = (l == L - 1)
            q_s, k_s, vT_s = cur_q, cur_k, cur_vT

            v_s = p_vs.tile([128, 8 * FEAT], F16, tag="v", name="vs")
            attn_s = p_attn.tile([128, 2 * S], F16, tag="attn", name="attn")

            if last:
                # B-half QKV of the last layer (k/v all tokens, q last-2)
                if ar2_prev_b is not None:
                    resid_add(ar2_prev_b, 1)
                qkv_half(wq_sb, l, 1, q_s, k_s, vT_s)
                # prefetch the last-layer ffn weight stream into SBUF while
                # the serial attention/AR tail runs (PE is mostly idle here)
                preL = [prefetch_w13_pass(l, 0)]
                preL1 = []
                for hc in range(KH):
                    t = p_hn.tile([128, 512], F16, tag="hn", name="w13pfL")
                    nc.sync.dma_start(t[:], w13_h.ap()[l, ts(hc, 128),
                                                       512: 1024])
                    preL1.append(t)
                preL.append(preL1)
                # RoPE on k (all tokens) and q (last 2 only)
                for mt in range(2):
                    for n in range(2):
                        rope_slice(k_s, mt * S + n * 512, 512, n * 512)
                    rope_slice(q_s, mt * S + S - 2, 2, S - 2)
                # V -> [tok, feat] via PE transpose
                for mt in range(2):
                    for tb in range(8):
                        tp = psum.tile([128, 128], F16, tag="ps", name="tp")
                        nc.tensor.transpose(
                            tp[:],
                            vT_s[:, mt * S + tb * 128: mt * S + tb * 128 + 128],
                            id_r[:])
                        nc.vector.tensor_copy(
                            v_s[:, tb * FEAT + mt * 128:
                                tb * FEAT + mt * 128 + 128], tp[:])
                # attention for the last 2 tokens only
                for h in range(2):
                    at1 = psum.tile([128, 2], F32, tag="ps", name="at1")
                    ib1 = psum.tile([128, 2], F32, tag="ps", name="ib1")
                    for kc in range(8):
                        sc1 = psum.tile([128, 2], F32, tag="ps", name="sc1")
                        nc.tensor.matmul(
                            sc1[:],
                            k_s[:, h * S + kc * 128: h * S + kc * 128 + 128],
                            q_s[:, h * S + S - 2: h * S + S],
                            start=True, stop=True)
                        pt1 = p_pt.tile([128, 2], F16, tag="pt1", name="pt1")
                        nc.scalar.activation(pt1[:], sc1[:], AF.Exp,
                                             scale=INV_SCALE)
                        if kc == 7:
                            nc.vector.tensor_mul(
                                pt1[:], pt1[:],
                                dmask[:, 3 * 512 + 510: 3 * 512 + 512])
                        st, sp = (kc == 0), (kc == 7)
                        nc.tensor.matmul(
                            at1[:],
                            v_s[:, kc * FEAT + h * 128: kc * FEAT + h * 128 + 128],
                            pt1[:], start=st, stop=sp)
                        nc.tensor.matmul(ib1[:], ones_mat[:], pt1[:],
                                         start=st, stop=sp)
                    inva = p_pt.tile([128, 2], F32, tag="pta", name="inva")
                    nc.vector.reciprocal_approx_fast(inva[:], ib1[:])
                    nc.vector.tensor_mul(
                        attn_s[:, h * S + S - 2: h * S + S], at1[:], inva[:])

                # wo -> [H,2] AllReduce -> residual add (last 2 tokens).
                # Staged through single batched DMAs (the tail is latency-
                # bound: every DMA hop costs a semaphore round-trip).
                xl2 = xT[:].rearrange("p (c s) -> p c s", s=S)[:, :, S - 2: S]
                ar_in = dram.tile([H, 2], F16, tag="arinL", name="arinL")
                ar_out = dram.tile([H, 2], F16, tag="aroutL",
                                   addr_space="Shared", name="aroutL")
                arL_all = p_row.tile([128, 2 * KH], F16, tag="arLa",
                                     name="arLall")
                for hc in range(KH):
                    poL = psum.tile([128, 2], F32, tag="ps", name="poL")
                    for fc in range(2):
                        nc.tensor.matmul(
                            poL[:],
                            wo_sb[:, fc * H + hc * 128: fc * H + hc * 128 + 128],
                            attn_s[:, fc * S + S - 2: fc * S + S],
                            start=(fc == 0), stop=(fc == 1))
                    nc.scalar.activation(arL_all[:, 2 * hc:2 * hc + 2],
                                         poL[:], AF.Copy)
                nc.sync.dma_start(
                    ar_in[:].rearrange("(c p) t -> p c t", p=128), arL_all[:])
                nc.gpsimd.collective_compute(
                    "AllReduce", ALU.add, replica_groups=[list(range(NC))],
                    ins=[ar_in[:].opt()], outs=[ar_out[:].opt()])
                art_all = p_row.tile([128, 2 * KH], F16, tag="arLb",
                                     name="artall")
                nc.sync.dma_start(
                    art_all[:], ar_out[:].rearrange("(c p) t -> p c t", p=128))
                nc.vector.tensor_add(
                    xl2, xl2, art_all[:].rearrange("p (c t) -> p c t", t=2))

                # norm2 + FFN on the last 2 tokens
                sqL = p_row.tile([128, 2 * KH], F16, tag="sql2")
                nc.vector.tensor_mul(
                    sqL[:].rearrange("p (c t) -> p c t", t=2), xl2, xl2)
                ssL = psum.tile([128, 2 * KH], F32, tag="ps", name="ssL")
                nc.tensor.matmul(ssL[:], ones_mat[:], sqL[:],
                                 start=True, stop=True)
                ssr = p_row.tile([128, 2], F32, tag="ssr")
                nc.vector.reduce_sum(
                    ssr[:], ssL[:].rearrange("p (c two) -> p two c", two=2),
                    axis=mybir.AxisListType.X)
                rmsL = p_row.tile([128, 2], F32, tag="rmsL")
                nc.scalar.activation(rmsL[:], ssr[:], AF.Sqrt,
                                     bias=eps_p[:], scale=1.0 / H)
                invL = p_row.tile([128, 2], F32, tag="invLc")
                nc.vector.reciprocal_approx_fast(invL[:], rmsL[:])
                hnL = p_row.tile([128, 2 * KH], F16, tag="hnL")
                for hc in range(KH):
                    col = hc * S + S - 2
                    nc.vector.tensor_mul(hnL[:, 2 * hc:2 * hc + 2],
                                         xT[:, col:col + 2], invL[:])
                guL = p_row.tile([128, 12 * 2], F16, tag="guL")
                for pi in range(3):
                    pset = range(4 * pi, 4 * pi + 4)
                    gps = {c: psum.tile([128, 2], F32, tag="ps",
                                        name=f"gL{c}") for c in pset}
                    for hc in range(KH):
                        if pi < 2:
                            w13_t = preL[pi][hc]
                        else:
                            w13_t = p_w13.tile([128, 512], F16, tag="w13",
                                               name="w13tL")
                            nc.sync.dma_start(
                                w13_t[:], w13_h.ap()[l, ts(hc, 128),
                                                     pi * 512: pi * 512 + 512])
                        st, sp = (hc == 0), (hc == KH - 1)
                        for ci, c in enumerate(pset):
                            nc.tensor.matmul(
                                gps[c][:], w13_t[:, ts(ci, 128)],
                                hnL[:, 2 * hc:2 * hc + 2], start=st, stop=sp)
                    for c in pset:
                        nc.scalar.activation(guL[:, 2 * c:2 * c + 2],
                                             gps[c][:], AF.Copy)
                swL = p_row.tile([128, 6 * 2], F16, tag="swL")
                for sch in range(6):
                    sgL = p_row.tile([128, 2], F16, tag="sgL")
                    nc.scalar.activation(sgL[:], guL[:, 2 * sch:2 * sch + 2],
                                         AF.Silu)
                    nc.vector.tensor_mul(swL[:, 2 * sch:2 * sch + 2], sgL[:],
                                         guL[:, 2 * (6 + sch):2 * (6 + sch) + 2])
                ar2_in = dram.tile([H, 2], F16, tag="arinL", name="ar2inL")
                ar2_out = dram.tile([H, 2], F16, tag="aroutL",
                                    addr_space="Shared", name="ar2outL")
                a2_all = p_row.tile([128, 2 * KH], F16, tag="arLa",
                                    name="a2all")
                for hcb in range(4):
                    p2L = [psum.tile([128, 2], F32, tag="ps", name=f"p2L{i}")
                           for i in range(4)]
                    for kc in range(6):
                        w2_t = p_w2.tile([128, 512], F16, tag="w2",
                                         name="w2tL")
                        nc.sync.dma_start(
                            w2_t[:], w2T_h.ap()[l, ts(kc, 128),
                                                hcb * 512: hcb * 512 + 512])
                        for hh in range(4):
                            nc.tensor.matmul(p2L[hh][:], w2_t[:, ts(hh, 128)],
                                             swL[:, 2 * kc:2 * kc + 2],
                                             start=(kc == 0), stop=(kc == 5))
                    for hh in range(4):
                        hc = hcb * 4 + hh
                        nc.scalar.activation(a2_all[:, 2 * hc:2 * hc + 2],
                                             p2L[hh][:], AF.Copy)
                nc.sync.dma_start(
                    ar2_in[:].rearrange("(c p) t -> p c t", p=128), a2_all[:])
                nc.gpsimd.collective_compute(
                    "AllReduce", ALU.add, replica_groups=[list(range(NC))],
                    ins=[ar2_in[:].opt()], outs=[ar2_out[:].opt()])
                art2_all = p_row.tile([128, 2 * KH], F16, tag="arLb",
                                      name="art2all")
                nc.sync.dma_start(
                    art2_all[:],
                    ar2_out[:].rearrange("(c p) t -> p c t", p=128))
                nc.vector.tensor_add(
                    xl2, xl2, art2_all[:].rearrange("p (c t) -> p c t", t=2))
                continue

            # ---- non-last layer: A-half attention first, then the B-half
            # QKV runs in AR1(A)'s shadow ----
            rope_vtrans_half(q_s, k_s, vT_s, v_s, 0)
            attn_half(q_s, k_s, v_s, attn_s, 0)
            ar1_a = wo_project(wo_sb, attn_s, 0)

            if ar2_prev_b is not None:
                resid_add(ar2_prev_b, 1)
            qkv_half(wq_sb, l, 1, q_s, k_s, vT_s)

            rope_vtrans_half(q_s, k_s, vT_s, v_s, 1)
            attn_half(q_s, k_s, v_s, attn_s, 1)
            ar1_b = wo_project(wo_sb, attn_s, 1)

            # prefetch next layer's qkv weights (slot free: qkv(l) done)
            wq_sb = load_wqkv(l + 1)

            pre_a = prefetch_w13_pass(l, 0)
            resid_add(ar1_a, 0)
            ar2_a = ffn_half(l, 0, pre_a)
            pre_b = prefetch_w13_pass(l, 0)
            resid_add(ar1_b, 1)
            ar2_prev_b = ffn_half(l, 1, pre_b)

            # next layer's wo
            wo_sb = load_wo(l + 1)

            nxt_q = p_big.tile([128, 2 * S], F16, tag="big", name="qn")
            nxt_k = p_big.tile([128, 2 * S], F16, tag="big", name="kn")
            nxt_vT = p_big.tile([128, 2 * S], F16, tag="big", name="vTn")
            resid_add(ar2_a, 0)
            qkv_half(wq_sb, l + 1, 0, nxt_q, nxt_k, nxt_vT)
            cur_q, cur_k, cur_vT = nxt_q, nxt_k, nxt_vT

        # ======== final norm (last token only) + logits ========
        # final_norm_w is folded into out_w on the host, so the GEMV
        # stationary is the raw xT column; 1/rms is applied at eviction.
        xl1 = xT[:].rearrange("p (c s) -> p c s", s=S)[:, :, S - 1: S]
        sq_l = p_row.tile([128, KH], F16, tag="sql")
        nc.vector.tensor_mul(
            sq_l[:].rearrange("p (c t) -> p c t", t=1), xl1, xl1)
        sl_ps = psum.tile([1, KH], F32, tag="ps", name="slps")
        nc.tensor.matmul(sl_ps[:], ones_col[:], sq_l[:], start=True, stop=True)
        ssc = p_row.tile([1, 1], F32, tag="ssc")
        nc.vector.reduce_sum(ssc[:], sl_ps[:], axis=mybir.AxisListType.X)
        rms_l = p_row.tile([1, 1], F32, tag="rmsl")
        nc.scalar.activation(rms_l[:], ssc[:], AF.Sqrt, bias=eps_t[:],
                             scale=1.0 / H)
        inv_l = p_row.tile([1, 1], F32, tag="invl")
        nc.vector.reciprocal(inv_l[:], rms_l[:])
        for n in range(4):
            lg_a = psum.tile([1, 500], F32, tag="ps", name="lga")
            lg_b = psum.tile([1, 500], F32, tag="ps", name="lgb")
            for hc in range(KH):
                ow_t = p_ow.tile([128, 1000], F16, tag="ow", name="owt")
                nc.sync.dma_start(
                    ow_t[:], owT_h.ap()[ts(hc, 128), n * 1000: n * 1000 + 1000])
                st, sp = (hc == 0), (hc == KH - 1)
                xcol = xT[:, hc * S + S - 1: hc * S + S]
                nc.tensor.matmul(lg_a[:], xcol, ow_t[:, :500],
                                 start=st, stop=sp)
                nc.tensor.matmul(lg_b[:], xcol, ow_t[:, 500:],
                                 start=st, stop=sp)
            lg = p_row.tile([1, 1000], F32, tag="lg")
            nc.scalar.activation(lg[:, :500], lg_a[:], AF.Copy, scale=inv_l[:])
            nc.scalar.activation(lg[:, 500:], lg_b[:], AF.Copy, scale=inv_l[:])
            nc.sync.dma_start(out_h.ap()[:, n * 1000: n * 1000 + 1000], lg[:])

    nc.compile()
    return nc


def _shard(inputs):
    import ml_dtypes
    f16 = ml_dtypes.bfloat16
    x = np.asarray(inputs["x"], np.float32)
    cos = np.asarray(inputs["cos"], np.float32).reshape(S, HD // 2)
    sin = np.asarray(inputs["sin"], np.float32).reshape(S, HD // 2)
    n1 = np.asarray(inputs["norm1_w"], np.float32)[:L]
    n2 = np.asarray(inputs["norm2_w"], np.float32)[:L]
    fw = np.asarray(inputs["final_norm_w"], np.float32)
    wq = np.asarray(inputs["wq"], np.float32)[:L]
    wk = np.asarray(inputs["wk"], np.float32)[:L]
    wv = np.asarray(inputs["wv"], np.float32)[:L]
    wo = np.asarray(inputs["wo"], np.float32)[:L]
    w1 = np.asarray(inputs["w1"], np.float32)[:L]
    w3 = np.asarray(inputs["w3"], np.float32)[:L]
    w2 = np.asarray(inputs["w2"], np.float32)[:L]
    ow = np.asarray(inputs["out_w"], np.float32)

    xT = np.ascontiguousarray(x[0].T).astype(f16)
    C = np.empty((128, S), np.float32)
    C[0::2] = cos.T
    C[1::2] = cos.T
    Sm = np.empty((128, S), np.float32)
    Sm[0::2] = -sin.T
    Sm[1::2] = sin.T
    J = np.zeros((128, 128), np.float32)
    idx = np.arange(0, 128, 2)
    J[idx, idx + 1] = 1.0
    J[idx + 1, idx] = 1.0
    ident = np.eye(128, dtype=np.float32)
    # diagonal causal masks: pattern di (block offset di*128):
    # mask[kp, q] = 1 if q >= kp + di*128
    dm = np.zeros((128, 4 * 512), np.float32)
    kp = np.arange(128)[:, None]
    qq = np.arange(512)[None, :]
    for di in range(4):
        dm[:, di * 512:(di + 1) * 512] = (qq >= kp + di * 128)
    # fold the rmsnorm weights into the following projection weights (the
    # device then applies only the 1/rms scale)
    wq = wq * n1[:, None, :]
    wk = wk * n1[:, None, :]
    wv = wv * n1[:, None, :]
    w1 = w1 * n2[:, None, :]
    w3 = w3 * n2[:, None, :]
    ow = ow * fw[None, :]

    common = dict(xT=xT, Cr=C.astype(f16), Sr=Sm.astype(f16),
                  J=J.astype(f16), ident=ident.astype(f16),
                  dmask=dm.astype(f16))
    in_maps = []
    for c in range(NC):
        fs = slice(c * FEAT, (c + 1) * FEAT)
        ps = slice(c * PC, (c + 1) * PC)
        vs = slice(c * VC, (c + 1) * VC)
        m = dict(common)
        wqT = wq[:, fs, :].transpose(0, 2, 1)
        wkT = wk[:, fs, :].transpose(0, 2, 1)
        wvT = wv[:, fs, :].transpose(0, 2, 1)
        m["wqkvT"] = np.ascontiguousarray(
            np.concatenate([wqT, wkT, wvT], axis=2)).astype(f16)
        m["woT"] = np.ascontiguousarray(
            wo[:, :, fs].transpose(0, 2, 1)).astype(f16)
        w1T = w1[:, ps, :].transpose(0, 2, 1)   # [L, H, PC]
        w3T = w3[:, ps, :].transpose(0, 2, 1)
        pad = np.zeros((L, H, PCP - PC), np.float32)
        m["w13T"] = np.ascontiguousarray(np.concatenate(
            [w1T, pad, w3T, pad], axis=2)).astype(f16)
        w2p = np.zeros((L, PCP, H), np.float32)
        w2p[:, :PC, :] = w2[:, :, ps].transpose(0, 2, 1)
        m["w2T"] = np.ascontiguousarray(w2p).astype(f16)
        m["owT"] = np.ascontiguousarray(ow[vs, :].T).astype(f16)
        in_maps.append(m)
    return in_maps


def kernel(**inputs) -> np.ndarray:
    from concourse import bass_utils

    if "nc" not in _STATE:
        _STATE["nc"] = _build()
    in_maps = _shard(inputs)
    res = bass_utils.run_bass_kernel_spmd(
        _STATE["nc"], in_maps, core_ids=list(range(NC)))
    out = np.concatenate(
        [res.results[c]["logits"] for c in range(NC)], axis=1)
    return out.astype(np.float32)
